# revision 1
# baseline (speedup 1.0000x reference)
"""MoE-ALU (add with carry + xor over one-hot byte encodings) on 8 NeuronCores.

Semantics (validated against the jax reference bit-exactly): inputs a, b are
exact one-hot byte encodings [B, 4, 256] (little-endian bytes of 32-bit ints);
with SCALE=100 every softmax in the reference collapses to an exact one-hot, so

    out[0] = one_hot bytes of (a_int + b_int) mod 2^32
    out[1] = one_hot bytes of (a_int ^ b_int)

Device kernel (pure data parallel, batch sharded over 8 cores), raw Bass
(this toolchain's walrus encodes at most ONE sync wait per instruction, so
Tile-generated schedules don't compile; manual sems with standalone waits do):

  per 128-row tile (a|b side by side in one [128, 2048] SBUF tile):
    decode  4x scalar_tensor_tensor with accum_out: multiply one 512-col
            segment by the [0..255 | 0,256,...,65280] pattern and reduce in
            one op -> a_lo a_hi b_lo b_hi (16-bit halves, f32-exact)
    add     int halves add; carry folded in via one (s_lo>=65536)+s_hi STT
    xor     int32 xor of the halves
    bytes   int32 shift/mask -> 8 byte indices
    encode  single is_equal [128, 8, 256] of the int iota table against the
            stride-0-broadcast indices, writing f32 one-hots directly

  engines: SyncE issues input DMAs, ScalarE issues output DMAs, VectorE
  computes. Rotating per-buffer-slot semaphores make DMA-queue completion
  order irrelevant (slot reuse is gated by the compute semaphore).

  DVE ops overlap in the engine pipe and do NOT self-interlock (measured:
  removing sync gives stale reads), so every same-engine RAW step waits on a
  monotonically counted DVE semaphore; per-tile temporaries are
  parity-double-buffered so consecutive tiles can overlap, with cross-parity
  reuse gated by the compute semaphore of tile i-1.
"""
from contextlib import ExitStack

import numpy as np

import concourse.bass as bass
from concourse import mybir
from concourse.bass_utils import run_bass_kernel_spmd

F32 = mybir.dt.float32
I32 = mybir.dt.int32

P = 128
N_CORES = 8
B = 32768
B_LOC = B // N_CORES          # 4096 rows per core
ROW = 4 * 256                 # 1024 f32 per row per tensor
N_TILES = B_LOC // P          # 32

NBUF = 8                      # input buffer slots
OBUF = 7                      # output buffer slots

TABI_COLS = 2048 + 8          # encode iota x8 | shift pattern


def _build_nc() -> bass.Bass:
    nc = bass.Bass(trn_type="TRN2")
    a_d = nc.dram_tensor("a", [B_LOC, ROW], F32, kind="ExternalInput")
    b_d = nc.dram_tensor("b", [B_LOC, ROW], F32, kind="ExternalInput")
    tabf_d = nc.dram_tensor("tabf", [P, 512], F32, kind="ExternalInput")
    tabi_d = nc.dram_tensor("tabi", [P, TABI_COLS], I32, kind="ExternalInput")
    out_d = nc.dram_tensor("out", [2, B_LOC, ROW], F32, kind="ExternalOutput")

    with ExitStack() as ctx:
        sb = lambda name, shape, dt: ctx.enter_context(
            nc.sbuf_tensor(name, shape, dt))
        tabf_t = sb("tabf_t", [P, 512], F32)
        tabi_t = sb("tabi_t", [P, TABI_COLS], I32)
        ab_t = [sb(f"ab_t{k}", [P, 2 * ROW], F32) for k in range(NBUF)]
        out_t = [sb(f"out_t{k}", [P, 2 * ROW], F32) for k in range(OBUF)]
        dump = [[sb(f"dump{p}_{k}", [P, 512], F32) for k in range(4)]
                for p in range(2)]
        # parity-double-buffered per-tile temporaries
        t6 = [sb(f"t6_{p}", [P, 6], F32) for p in range(2)]
        iv = [sb(f"iv_{p}", [P, 6], I32) for p in range(2)]   # a16 b16 s16
        v4 = [sb(f"v4_{p}", [P, 4], I32) for p in range(2)]   # slo shi xlo xhi
        sh8 = [sb(f"sh8_{p}", [P, 8], I32) for p in range(2)]
        idx8 = [sb(f"idx8_{p}", [P, 8], I32) for p in range(2)]

        dec = tabf_t[:]
        enc = tabi_t[:, 0:2048].rearrange("p (e k) -> p e k", k=256)
        shifts = tabi_t[:, 2048:2056].rearrange("p (a two) -> p a two", two=2)

        s_tab = ctx.enter_context(nc.semaphore("s_tab"))
        s_tab2 = ctx.enter_context(nc.semaphore("s_tab2"))
        s_load = [ctx.enter_context(nc.semaphore(f"s_load{j}"))
                  for j in range(NBUF)]
        s_store = [ctx.enter_context(nc.semaphore(f"s_store{j}"))
                   for j in range(OBUF)]
        s_comp = ctx.enter_context(nc.semaphore("s_comp"))
        s_dve = ctx.enter_context(nc.semaphore("s_dve"))

        block = ctx.enter_context(nc.Block())

        @block.sync
        def _(sync: bass.BassEngine):
            sync.dma_start(out=tabf_t[:], in_=tabf_d[:]).then_inc(s_tab, 16)
            for i in range(N_TILES):
                j = i % NBUF
                if i >= NBUF:
                    # slot reuse: tile i-NBUF must be fully consumed
                    sync.wait_ge(s_comp, 2 * (i - NBUF + 1))
                r0 = i * P
                sync.dma_start(
                    out=ab_t[j][:, 0:ROW], in_=a_d[r0:r0 + P, :]
                ).then_inc(s_load[j], 16)
                sync.dma_start(
                    out=ab_t[j][:, ROW:2 * ROW], in_=b_d[r0:r0 + P, :]
                ).then_inc(s_load[j], 16)
                if i == 0:
                    # big enc/shift table: after tile-0 data so compute
                    # starts sooner; needed only ~3us into tile 0
                    sync.dma_start(
                        out=tabi_t[:], in_=tabi_d[:]).then_inc(s_tab2, 16)

        @block.scalar
        def _(scalar: bass.BassEngine):
            for i in range(N_TILES):
                j = i % OBUF
                r0 = i * P
                scalar.wait_ge(s_comp, 2 * i + 1)
                scalar.dma_start(
                    out=out_d[0, r0:r0 + P, :], in_=out_t[j][:, 0:ROW]
                ).then_inc(s_store[j], 16)
                scalar.wait_ge(s_comp, 2 * i + 2)
                scalar.dma_start(
                    out=out_d[1, r0:r0 + P, :], in_=out_t[j][:, ROW:2 * ROW]
                ).then_inc(s_store[j], 16)

        @block.vector
        def _(vector: bass.BassEngine):
            n = 0  # statically tracked s_dve count

            vector.wait_ge(s_tab, 16)   # dec table (loaded first)
            for i in range(N_TILES):
                j = i % NBUF
                jo = i % OBUF
                pr = i % 2
                if i >= 2:
                    # tile i-2 (same parity) fully retired, incl. its encode,
                    # before its temporaries are reused
                    vector.wait_ge(s_comp, 2 * (i - 1))
                vector.wait_ge(s_load[j], 32 * (i // NBUF + 1))
                if i >= OBUF:
                    vector.wait_ge(s_store[jo], 32 * (i // OBUF))

                # decode: a_lo a_hi b_lo b_hi as f32 accumulators
                for k in range(4):
                    vector.scalar_tensor_tensor(
                        out=dump[pr][k][:],
                        in0=ab_t[j][:, 512 * k:512 * k + 512],
                        scalar=1.0,
                        in1=dec,
                        op0=mybir.AluOpType.mult,
                        op1=mybir.AluOpType.mult,
                        accum_out=t6[pr][:, k:k + 1],
                    ).then_inc(s_dve, 1)
                n += 4
                vector.wait_ge(s_dve, n)
                # int cast of the four halves
                vector.tensor_copy(iv[pr][:, 0:4], t6[pr][:, 0:4]).then_inc(
                    s_dve, 1)
                n += 1
                vector.wait_ge(s_dve, n)
                # s16 halves sum and xor halves
                vector.tensor_tensor(
                    out=iv[pr][:, 4:6], in0=iv[pr][:, 0:2],
                    in1=iv[pr][:, 2:4],
                    op=mybir.AluOpType.add).then_inc(s_dve, 1)
                vector.tensor_tensor(
                    out=v4[pr][:, 2:4], in0=iv[pr][:, 0:2],
                    in1=iv[pr][:, 2:4],
                    op=mybir.AluOpType.bitwise_xor).then_inc(s_dve, 1)
                n += 2
                vector.wait_ge(s_dve, n)
                # carry lo->hi: s_lo' = s_lo & 65535 ; s_hi' = (s_lo>=2^16)+s_hi
                vector.tensor_scalar(
                    out=v4[pr][:, 0:1], in0=iv[pr][:, 4:5], scalar1=65535,
                    scalar2=None,
                    op0=mybir.AluOpType.bitwise_and).then_inc(s_dve, 1)
                vector.scalar_tensor_tensor(
                    out=v4[pr][:, 1:2], in0=iv[pr][:, 4:5], scalar=65536,
                    in1=iv[pr][:, 5:6],
                    op0=mybir.AluOpType.is_ge,
                    op1=mybir.AluOpType.add).then_inc(s_dve, 1)
                n += 2
                vector.wait_ge(s_dve, n)
                if i == 0:
                    vector.wait_ge(s_tab2, 16)  # shift/enc table ready
                vector.tensor_tensor(
                    out=sh8[pr][:],
                    in0=v4[pr][:, :, None].to_broadcast((P, 4, 2)),
                    in1=shifts,
                    op=mybir.AluOpType.logical_shift_right).then_inc(s_dve, 1)
                n += 1
                vector.wait_ge(s_dve, n)
                vector.tensor_scalar(
                    out=idx8[pr][:], in0=sh8[pr][:], scalar1=255,
                    scalar2=None,
                    op0=mybir.AluOpType.bitwise_and).then_inc(s_dve, 1)
                n += 1
                vector.wait_ge(s_dve, n)
                # encode in two halves so the add-half store releases early
                vector.tensor_tensor(
                    out=out_t[jo][:, 0:ROW].rearrange(
                        "p (e k) -> p e k", k=256),
                    in0=enc[:, 0:4, :],
                    in1=idx8[pr][:, 0:4, None].to_broadcast((P, 4, 256)),
                    op=mybir.AluOpType.is_equal,
                ).then_inc(s_comp, 1)
                vector.tensor_tensor(
                    out=out_t[jo][:, ROW:2 * ROW].rearrange(
                        "p (e k) -> p e k", k=256),
                    in0=enc[:, 4:8, :],
                    in1=idx8[pr][:, 4:8, None].to_broadcast((P, 4, 256)),
                    op=mybir.AluOpType.is_equal,
                ).then_inc(s_comp, 1)

    return nc


def _make_tables():
    dec = np.concatenate([np.arange(256), np.arange(256) * 256]).astype(np.float32)
    tabf = np.tile(dec[None, :], (P, 1))
    enc = np.tile(np.arange(256, dtype=np.int64), 8)
    shifts = np.array([0, 8] * 4, np.int64)
    tabi = np.tile(np.concatenate([enc, shifts]).astype(np.int32)[None, :],
                   (P, 1))
    return tabf, tabi


_NC_CACHE = {}


def _get_nc(variant: str = "main"):
    if variant not in _NC_CACHE:
        _NC_CACHE[variant] = _build_nc()
    return _NC_CACHE[variant]


def _run(a: np.ndarray, b: np.ndarray, **spmd_kwargs):
    assert a.shape == (B, 4, 256) and b.shape == (B, 4, 256)
    a2 = np.ascontiguousarray(a, dtype=np.float32).reshape(B, ROW)
    b2 = np.ascontiguousarray(b, dtype=np.float32).reshape(B, ROW)
    tabf, tabi = _make_tables()
    in_maps = [
        {
            "a": a2[i * B_LOC:(i + 1) * B_LOC],
            "b": b2[i * B_LOC:(i + 1) * B_LOC],
            "tabf": tabf,
            "tabi": tabi,
        }
        for i in range(N_CORES)
    ]
    nc = _get_nc()
    kr = run_bass_kernel_spmd(nc, in_maps, list(range(N_CORES)), **spmd_kwargs)
    shards = [kr.results[i]["out"] for i in range(N_CORES)]
    out = np.concatenate(shards, axis=1).reshape(2, B, 4, 256)
    return out, kr


def kernel(a: np.ndarray, b: np.ndarray) -> np.ndarray:
    out, _ = _run(a, b)
    return out



# revision 6
# speedup vs baseline: 1.3275x; 1.3275x over previous
"""MoE-ALU (add with carry + xor over one-hot byte encodings) on 8 NeuronCores.

Semantics (validated against the jax reference bit-exactly): inputs a, b are
exact one-hot byte encodings [B, 4, 256] (little-endian bytes of 32-bit ints);
with SCALE=100 every softmax in the reference collapses to an exact one-hot, so

    out[0] = one_hot bytes of (a_int + b_int) mod 2^32
    out[1] = one_hot bytes of (a_int ^ b_int)

V3 layout: the host stores the one-hot inputs slab-major+transposed as fp8
([slab, position, batch], values 0.0/1.0 are exact in fp8e4) and the outputs
as uint8 one-hots (exact 0/1), so the device moves 8 MiB in + 8 MiB out per
core instead of 32+32 for f32 batch-major. All compute still happens on
device; the host only reorders/recodes losslessly.

Device pipeline per 512-row batch group (8 groups per core):
  decode  TensorE: 16 accumulating matmuls (K=128 chunk each) of the fp8
          one-hot slabs against bf16 iota/256*iota weight columns produce
          PSUM [4, 512] = (a_lo16, a_hi16, b_lo16, b_hi16) 16-bit halves,
          exact in f32.
  stage   ScalarE copies PSUM -> SBUF f32 (frees the bank for group g+2).
  flip    TensorE transposes [4, 128] -> PSUM [128, 4] per 128-row tile.
  alu     VectorE per tile: int32 cast, halves add / xor, carry fold,
          shift/mask -> 8 byte indices; ops are [128, <=8] wide.
  encode  8x tensor_scalar(is_equal) with per-partition scalar pointer
          against a u8 iota table -> uint8 one-hot [128, 2048] (single-src
          op: runs in the DVE 2x_2p perf mode).
  store   ScalarE issues the two output DMAs (add half / xor half).

Raw Bass (one sync wait per instruction); rotating per-slot semaphores gate
buffer reuse; DVE same-engine RAW steps wait on a monotonically counted
semaphore (DVE ops do not self-interlock).
"""
from contextlib import ExitStack

import numpy as np
import ml_dtypes

import concourse.bass as bass
from concourse import mybir
from concourse.bass_utils import run_bass_kernel_spmd

F32 = mybir.dt.float32
I32 = mybir.dt.int32
U8 = mybir.dt.uint8
BF16 = mybir.dt.bfloat16
FP8 = mybir.dt.float8e4

P = 128
N_CORES = 8
B = 32768
B_LOC = B // N_CORES          # 4096 rows per core
ROW = 4 * 256                 # 1024 per row per tensor
NG = 512                      # batch rows per matmul group (one PSUM bank)
G = B_LOC // NG               # 8 groups
N_TILES = B_LOC // P          # 32 tiles of 128 rows
NCH = 16                      # K-chunks: 8 slabs (a0..a3,b0..b3) x 2 halves

NBUF = 4                      # input group-buffer slots
OBUF = 8                      # output tile-buffer slots

DVE_OPS = 8                   # s_dve increments per tile (chain ops)


def _build_nc() -> bass.Bass:
    nc = bass.Bass(trn_type="TRN2")
    ab_d = nc.dram_tensor("abt", [NCH, P, B_LOC], FP8, kind="ExternalInput")
    tabw_d = nc.dram_tensor("tabw", [P, NCH * 4], BF16, kind="ExternalInput")
    tabio_d = nc.dram_tensor("tabio", [P, 256], U8, kind="ExternalInput")
    tabsh_d = nc.dram_tensor("tabsh", [P, 8], I32, kind="ExternalInput")
    tabid_d = nc.dram_tensor("tabid", [4, 4], F32, kind="ExternalInput")
    out_d = nc.dram_tensor("out", [2, B_LOC, ROW], U8, kind="ExternalOutput")

    with ExitStack() as ctx:
        sb = lambda name, shape, dt: ctx.enter_context(
            nc.sbuf_tensor(name, shape, dt))
        tabw_t = sb("tabw_t", [P, NCH * 4], BF16)
        tabio_t = sb("tabio_t", [P, 256], U8)
        tabsh_t = sb("tabsh_t", [P, 8], I32)
        tabid_t = sb("tabid_t", [4, 4], F32)
        in_t = [sb(f"in_t{k}", [P, NCH * NG], FP8) for k in range(NBUF)]
        sval = [sb(f"sval{k}", [4, NG], F32) for k in range(2)]
        out_t = [sb(f"out_t{k}", [P, 2 * ROW], U8) for k in range(OBUF)]
        # parity-double-buffered per-tile temporaries
        iv = [sb(f"iv_{p}", [P, 6], I32) for p in range(2)]   # halves + s_raw
        v4 = [sb(f"v4_{p}", [P, 4], I32) for p in range(2)]   # slo shi xlo xhi
        sh8 = [sb(f"sh8_{p}", [P, 8], I32) for p in range(2)]
        idx8 = [sb(f"idx8_{p}", [P, 8], I32) for p in range(2)]
        idxf = [sb(f"idxf_{p}", [P, 8], F32) for p in range(2)]

        pv = [ctx.enter_context(nc.psum_tensor(f"pv{k}", [4, NG], F32))
              for k in range(2)]
        pt = [ctx.enter_context(nc.psum_tensor(f"pt{k}", [P, 16], F32))
              for k in range(2)]

        shifts = tabsh_t[:].rearrange("p (a two) -> p a two", two=2)

        s_tab = ctx.enter_context(nc.semaphore("s_tab"))
        s_load = [ctx.enter_context(nc.semaphore(f"s_load{j}"))
                  for j in range(NBUF)]
        s_store = [ctx.enter_context(nc.semaphore(f"s_store{j}"))
                   for j in range(OBUF)]
        s_mm = ctx.enter_context(nc.semaphore("s_mm"))      # matmul groups done
        s_sv = ctx.enter_context(nc.semaphore("s_sv"))      # psum->sbuf copies
        s_T = ctx.enter_context(nc.semaphore("s_T"))        # transposes done
        s_comp = ctx.enter_context(nc.semaphore("s_comp"))  # encode halves done
        s_dve = ctx.enter_context(nc.semaphore("s_dve"))    # chain ops done

        block = ctx.enter_context(nc.Block())

        @block.sync
        def _(sync: bass.BassEngine):
            sync.dma_start(out=tabw_t[:], in_=tabw_d[:]).then_inc(s_tab, 16)
            sync.dma_start(out=tabio_t[:], in_=tabio_d[:]).then_inc(s_tab, 16)
            sync.dma_start(out=tabsh_t[:], in_=tabsh_d[:]).then_inc(s_tab, 16)
            sync.dma_start(out=tabid_t[:], in_=tabid_d[:]).then_inc(s_tab, 16)
            for g in range(G):
                j = g % NBUF
                if g >= NBUF:
                    # slot reuse: matmuls of group g-NBUF consumed it
                    sync.wait_ge(s_mm, g - NBUF + 1)
                for c in range(NCH):
                    sync.dma_start(
                        out=in_t[j][:, NG * c:NG * (c + 1)],
                        in_=ab_d[c, :, NG * g:NG * (g + 1)],
                    ).then_inc(s_load[j], 16)

        @block.tensor
        def _(tensor: bass.BassEngine):
            tensor.wait_ge(s_tab, 64)
            for g in range(G + 1):
                if g < G:
                    j = g % NBUF
                    tensor.wait_ge(s_load[j], 16 * NCH * (g // NBUF + 1))
                    if g >= 2:
                        # pv[g%2] freed once ScalarE copied group g-2
                        tensor.wait_ge(s_sv, g - 1)
                    for c in range(NCH):
                        ins = tensor.matmul(
                            out=pv[g % 2][:, :],
                            lhsT=tabw_t[:, 4 * c:4 * (c + 1)],
                            rhs=in_t[j][:, NG * c:NG * (c + 1)],
                            start=(c == 0),
                            stop=(c == NCH - 1),
                        )
                        if c == NCH - 1:
                            ins.then_inc(s_mm, 1)
                q = g - 1
                if q >= 0:
                    tensor.wait_ge(s_sv, q + 1)
                    if q >= 2:
                        # pt[q%2] freed once DVE's iv-copy of the last tile
                        # of group q-2 retired (copy is op 1 of DVE_OPS)
                        need_tile = 4 * (q - 2) + 3
                        tensor.wait_ge(s_dve, DVE_OPS * need_tile + 1)
                    for k in range(4):
                        tensor.transpose(
                            out=pt[q % 2][:, 4 * k:4 * (k + 1)],
                            in_=sval[q % 2][:, P * k:P * (k + 1)],
                            identity=tabid_t[:],
                        ).then_inc(s_T, 1)

        @block.scalar
        def _(scalar: bass.BassEngine):
            for g in range(G + 1):
                if g < G:
                    scalar.wait_ge(s_mm, g + 1)
                    if g >= 2:
                        # sval[g%2] freed once transposes of group g-2 done
                        scalar.wait_ge(s_T, 4 * (g - 1))
                    scalar.activation(
                        out=sval[g % 2][:, :], in_=pv[g % 2][:, :],
                        func=mybir.ActivationFunctionType.Copy,
                    ).then_inc(s_sv, 1)
                q = g - 1
                if q >= 0:
                    for k in range(4):
                        t = 4 * q + k
                        jo = t % OBUF
                        r0 = t * P
                        scalar.wait_ge(s_comp, 2 * t + 1)
                        scalar.dma_start(
                            out=out_d[0, r0:r0 + P, :], in_=out_t[jo][:, 0:ROW]
                        ).then_inc(s_store[jo], 16)
                        scalar.wait_ge(s_comp, 2 * t + 2)
                        scalar.dma_start(
                            out=out_d[1, r0:r0 + P, :],
                            in_=out_t[jo][:, ROW:2 * ROW]
                        ).then_inc(s_store[jo], 16)

        @block.vector
        def _(vector: bass.BassEngine):
            n = 0  # statically tracked s_dve count
            for t in range(N_TILES):
                q = t // 4
                k = t % 4
                pr = t % 2
                jo = t % OBUF
                vector.wait_ge(s_T, t + 1)
                if t >= 2:
                    # tile t-2 (same parity) fully retired before its
                    # temporaries are reused
                    vector.wait_ge(s_comp, 2 * (t - 1))
                if t >= OBUF:
                    vector.wait_ge(s_store[jo], 32 * (t // OBUF))
                # int cast of the four 16-bit halves [a_lo a_hi b_lo b_hi]
                vector.tensor_copy(
                    iv[pr][:, 0:4], pt[q % 2][:, 4 * k:4 * (k + 1)]
                ).then_inc(s_dve, 1)
                n += 1
                vector.wait_ge(s_dve, n)
                # raw 17-bit sums of halves, and xor of halves
                vector.tensor_tensor(
                    out=iv[pr][:, 4:6], in0=iv[pr][:, 0:2],
                    in1=iv[pr][:, 2:4],
                    op=mybir.AluOpType.add).then_inc(s_dve, 1)
                vector.tensor_tensor(
                    out=v4[pr][:, 2:4], in0=iv[pr][:, 0:2],
                    in1=iv[pr][:, 2:4],
                    op=mybir.AluOpType.bitwise_xor).then_inc(s_dve, 1)
                n += 2
                vector.wait_ge(s_dve, n)
                # carry lo->hi: s_lo' = s_lo & 65535 ; s_hi' = (s_lo>=2^16)+s_hi
                vector.tensor_scalar(
                    out=v4[pr][:, 0:1], in0=iv[pr][:, 4:5], scalar1=65535,
                    scalar2=None,
                    op0=mybir.AluOpType.bitwise_and).then_inc(s_dve, 1)
                vector.scalar_tensor_tensor(
                    out=v4[pr][:, 1:2], in0=iv[pr][:, 4:5], scalar=65536,
                    in1=iv[pr][:, 5:6],
                    op0=mybir.AluOpType.is_ge,
                    op1=mybir.AluOpType.add).then_inc(s_dve, 1)
                n += 2
                vector.wait_ge(s_dve, n)
                # [slo slo shi shi xlo xlo xhi xhi] >> [0 8 0 8 ...]
                vector.tensor_tensor(
                    out=sh8[pr][:],
                    in0=v4[pr][:, :, None].to_broadcast((P, 4, 2)),
                    in1=shifts,
                    op=mybir.AluOpType.logical_shift_right).then_inc(s_dve, 1)
                n += 1
                vector.wait_ge(s_dve, n)
                vector.tensor_scalar(
                    out=idx8[pr][:], in0=sh8[pr][:], scalar1=255,
                    scalar2=None,
                    op0=mybir.AluOpType.bitwise_and).then_inc(s_dve, 1)
                n += 1
                vector.wait_ge(s_dve, n)
                vector.tensor_copy(idxf[pr][:], idx8[pr][:]).then_inc(s_dve, 1)
                n += 1
                vector.wait_ge(s_dve, n)
                # encode: 8 single-src is_equal ops against the u8 iota table,
                # one per output byte, per-partition scalar = that byte's value
                for e in range(8):
                    ins = vector.tensor_scalar(
                        out=out_t[jo][:, 256 * e:256 * (e + 1)],
                        in0=tabio_t[:],
                        scalar1=idxf[pr][:, e:e + 1],
                        scalar2=None,
                        op0=mybir.AluOpType.is_equal,
                    )
                    if e == 3 or e == 7:
                        ins.then_inc(s_comp, 1)

    return nc


def _make_tables():
    pos = np.arange(P, dtype=np.float64)
    w = np.zeros((NCH, P, 4), np.float64)
    for s in range(8):
        col = s // 2 if s < 4 else 2 + (s - 4) // 2
        mul = 1.0 if (s % 2 == 0) else 256.0
        for h in range(2):
            c = 2 * s + h
            w[c, :, col] = (pos + 128.0 * h) * mul
    tabw = w.transpose(1, 0, 2).reshape(P, NCH * 4).astype(ml_dtypes.bfloat16)
    tabio = np.tile(np.arange(256, dtype=np.uint8)[None, :], (P, 1))
    tabsh = np.tile(np.array([0, 8] * 4, np.int32)[None, :], (P, 1))
    tabid = np.eye(4, dtype=np.float32)
    return tabw, tabio, tabsh, tabid


def _pack_inputs(a: np.ndarray, b: np.ndarray) -> np.ndarray:
    """[B,4,256] f32 x2 -> [NCH, P, B] fp8 slab-major transposed chunks."""
    at = np.ascontiguousarray(a.reshape(B, 4, 256).transpose(1, 2, 0))
    bt = np.ascontiguousarray(b.reshape(B, 4, 256).transpose(1, 2, 0))
    ab = np.concatenate([at, bt], axis=0)            # [8, 256, B]
    return ab.reshape(NCH, P, B).astype(ml_dtypes.float8_e4m3)


_NC_CACHE = {}


def _get_nc(variant: str = "main"):
    if variant not in _NC_CACHE:
        _NC_CACHE[variant] = _build_nc()
    return _NC_CACHE[variant]


def _run(a: np.ndarray, b: np.ndarray, **spmd_kwargs):
    assert a.shape == (B, 4, 256) and b.shape == (B, 4, 256)
    abt = _pack_inputs(np.asarray(a, np.float32), np.asarray(b, np.float32))
    tabw, tabio, tabsh, tabid = _make_tables()
    in_maps = [
        {
            "abt": np.ascontiguousarray(abt[:, :, i * B_LOC:(i + 1) * B_LOC]),
            "tabw": tabw,
            "tabio": tabio,
            "tabsh": tabsh,
            "tabid": tabid,
        }
        for i in range(N_CORES)
    ]
    nc = _get_nc()
    kr = run_bass_kernel_spmd(nc, in_maps, list(range(N_CORES)), **spmd_kwargs)
    shards = [kr.results[i]["out"] for i in range(N_CORES)]
    out = np.concatenate(shards, axis=1).astype(np.float32)
    return out.reshape(2, B, 4, 256), kr


def kernel(a: np.ndarray, b: np.ndarray) -> np.ndarray:
    out, _ = _run(a, b)
    return out


def run_sim():
    """CoreSim one core vs numpy oracle (invoked by test.py --sim)."""
    from concourse.bass_interp import CoreSim

    rng = np.random.default_rng(1)
    Bl = B_LOC
    ai = rng.integers(0, 256, (Bl, 4))
    bi = rng.integers(0, 256, (Bl, 4))
    ai[0] = [255] * 4
    bi[0] = [255] * 4
    ai[1] = [255, 255, 255, 255]
    bi[1] = [1, 0, 0, 0]
    a = np.zeros((Bl, 4, 256), np.float32)
    b = np.zeros((Bl, 4, 256), np.float32)
    r = np.arange(Bl)[:, None]
    j = np.arange(4)[None, :]
    a[r, j, ai] = 1.0
    b[r, j, bi] = 1.0

    at = np.ascontiguousarray(a.transpose(1, 2, 0))
    bt = np.ascontiguousarray(b.transpose(1, 2, 0))
    abt = np.concatenate([at, bt], 0).reshape(NCH, P, Bl).astype(
        ml_dtypes.float8_e4m3)
    tabw, tabio, tabsh, tabid = _make_tables()

    nc = _get_nc()
    sim = CoreSim(nc)
    sim.tensor("abt")[:] = abt
    sim.tensor("tabw")[:] = tabw
    sim.tensor("tabio")[:] = tabio
    sim.tensor("tabsh")[:] = tabsh
    sim.tensor("tabid")[:] = tabid
    sim.simulate()
    out = np.array(sim.tensor("out")).astype(np.float32).reshape(2, Bl, 4, 256)

    # numpy oracle
    pw = (256 ** np.arange(4)).astype(np.int64)
    a32 = (ai * pw).sum(-1)
    b32 = (bi * pw).sum(-1)
    s32 = (a32 + b32) % (2 ** 32)
    x32 = a32 ^ b32
    sb_ = np.stack([(s32 >> (8 * i)) & 255 for i in range(4)], -1)
    xb_ = np.stack([(x32 >> (8 * i)) & 255 for i in range(4)], -1)
    exp = np.zeros((2, Bl, 4, 256), np.float32)
    exp[0, r, j, sb_] = 1.0
    exp[1, r, j, xb_] = 1.0
    err = np.abs(out - exp).max()
    print(f"SIM max abs err: {err}")
    assert err == 0.0, "sim mismatch"
    print("SIM PASS")


# revision 8
# speedup vs baseline: 1.5176x; 1.1432x over previous
"""MoE-ALU (add with carry + xor over one-hot byte encodings) on 8 NeuronCores.

Semantics (validated against the jax reference bit-exactly): inputs a, b are
exact one-hot byte encodings [B, 4, 256] (little-endian bytes of 32-bit ints);
with SCALE=100 every softmax in the reference collapses to an exact one-hot, so

    out[0] = one_hot bytes of (a_int + b_int) mod 2^32
    out[1] = one_hot bytes of (a_int ^ b_int)

V3 layout: the host stores the one-hot inputs slab-major+transposed as fp8
([slab, position, batch], values 0.0/1.0 are exact in fp8e4) and the outputs
as uint8 one-hots (exact 0/1), so the device moves 8 MiB in + 8 MiB out per
core instead of 32+32 for f32 batch-major. All compute still happens on
device; the host only reorders/recodes losslessly.

Device pipeline per 512-row batch group (8 groups per core):
  decode  TensorE: 16 accumulating matmuls (K=128 chunk each) of the fp8
          one-hot slabs against bf16 iota/256*iota weight columns produce
          PSUM [4, 512] = (a_lo16, a_hi16, b_lo16, b_hi16) 16-bit halves,
          exact in f32.
  stage   ScalarE copies PSUM -> SBUF f32 (frees the bank for group g+2).
  flip    TensorE transposes [4, 128] -> PSUM [128, 4] per 128-row tile.
  alu     VectorE per tile: int32 cast, halves add / xor, carry fold,
          shift/mask -> 8 byte indices; ops are [128, <=8] wide.
  encode  8x tensor_scalar(is_equal) with per-partition scalar pointer
          against a u8 iota table -> uint8 one-hot [128, 2048] (single-src
          op: runs in the DVE 2x_2p perf mode).
  store   ScalarE issues the two output DMAs (add half / xor half).

Raw Bass (one sync wait per instruction); rotating per-slot semaphores gate
buffer reuse; DVE same-engine RAW steps wait on a monotonically counted
semaphore (DVE ops do not self-interlock).
"""
from contextlib import ExitStack

import numpy as np
import ml_dtypes

import concourse.bass as bass
from concourse import mybir
from concourse.bass_utils import run_bass_kernel_spmd

F32 = mybir.dt.float32
I32 = mybir.dt.int32
U8 = mybir.dt.uint8
BF16 = mybir.dt.bfloat16
FP8 = mybir.dt.float8e4

P = 128
N_CORES = 8
B = 32768
B_LOC = B // N_CORES          # 4096 rows per core
ROW = 4 * 256                 # 1024 per row per tensor
NG = 512                      # batch rows per matmul group (one PSUM bank)
G = B_LOC // NG               # 8 groups
N_TILES = B_LOC // P          # 32 tiles of 128 rows
NCH = 16                      # K-chunks: 8 slabs (a0..a3,b0..b3) x 2 halves

NBUF = 4                      # input group-buffer slots
OBUF = 3                      # output group-buffer slots

DVE_OPS = 8                   # s_dve increments per tile (chain ops)


def _build_nc() -> bass.Bass:
    nc = bass.Bass(trn_type="TRN2")
    ab_d = nc.dram_tensor("abt", [NCH, P, B_LOC], FP8, kind="ExternalInput")
    tabw_d = nc.dram_tensor("tabw", [P, NCH * 4], BF16, kind="ExternalInput")
    tabio_d = nc.dram_tensor("tabio", [P, 256], U8, kind="ExternalInput")
    tabsh_d = nc.dram_tensor("tabsh", [P, 8], I32, kind="ExternalInput")
    tabid_d = nc.dram_tensor("tabid", [4, 4], F32, kind="ExternalInput")
    out_d = nc.dram_tensor("out", [2, B_LOC, ROW], U8, kind="ExternalOutput")

    with ExitStack() as ctx:
        sb = lambda name, shape, dt: ctx.enter_context(
            nc.sbuf_tensor(name, shape, dt))
        tabw_t = sb("tabw_t", [P, NCH * 4], BF16)
        tabio_t = sb("tabio_t", [P, 256], U8)
        tabsh_t = sb("tabsh_t", [P, 8], I32)
        tabid_t = sb("tabid_t", [4, 4], F32)
        in_t = [sb(f"in_t{k}", [P, NCH * NG], FP8) for k in range(NBUF)]
        sval = [sb(f"sval{k}", [4, NG], F32) for k in range(2)]
        og = [sb(f"og{k}", [P, 4 * 2 * ROW], U8) for k in range(OBUF)]
        # parity-double-buffered per-tile temporaries
        iv = [sb(f"iv_{p}", [P, 6], I32) for p in range(2)]   # halves + s_raw
        v4 = [sb(f"v4_{p}", [P, 4], I32) for p in range(2)]   # slo shi xlo xhi
        idx8 = [sb(f"idx8_{p}", [P, 8], I32) for p in range(2)]
        idxf = [sb(f"idxf_{p}", [P, 8], F32) for p in range(2)]

        pv = [ctx.enter_context(nc.psum_tensor(f"pv{k}", [4, NG], F32))
              for k in range(2)]
        pt = [ctx.enter_context(nc.psum_tensor(f"pt{k}", [P, 16], F32))
              for k in range(2)]

        s_tab = ctx.enter_context(nc.semaphore("s_tab"))
        s_load = [ctx.enter_context(nc.semaphore(f"s_load{j}"))
                  for j in range(NBUF)]
        s_store = [ctx.enter_context(nc.semaphore(f"s_store{j}"))
                   for j in range(OBUF)]
        s_mm = ctx.enter_context(nc.semaphore("s_mm"))      # matmul groups done
        s_sv = ctx.enter_context(nc.semaphore("s_sv"))      # psum->sbuf copies
        s_T = ctx.enter_context(nc.semaphore("s_T"))        # transposes done
        s_comp = ctx.enter_context(nc.semaphore("s_comp"))  # encode halves done
        s_dve = ctx.enter_context(nc.semaphore("s_dve"))    # chain ops done

        block = ctx.enter_context(nc.Block())

        @block.sync
        def _(sync: bass.BassEngine):
            sync.dma_start(out=tabw_t[:], in_=tabw_d[:]).then_inc(s_tab, 16)
            sync.dma_start(out=tabio_t[:], in_=tabio_d[:]).then_inc(s_tab, 16)
            sync.dma_start(out=tabsh_t[:], in_=tabsh_d[:]).then_inc(s_tab, 16)
            sync.dma_start(out=tabid_t[:], in_=tabid_d[:]).then_inc(s_tab, 16)
            for g in range(G):
                j = g % NBUF
                if g >= NBUF:
                    # slot reuse: matmuls of group g-NBUF consumed it
                    sync.wait_ge(s_mm, g - NBUF + 1)
                sync.dma_start(
                    out=in_t[j][:].rearrange("p (c n) -> p c n", c=NCH),
                    in_=ab_d[:, :, NG * g:NG * (g + 1)].rearrange(
                        "c p n -> p c n"),
                ).then_inc(s_load[j], 16)

        @block.tensor
        def _(tensor: bass.BassEngine):
            tensor.wait_ge(s_tab, 64)
            for g in range(G + 1):
                if g < G:
                    j = g % NBUF
                    tensor.wait_ge(s_load[j], 16 * (g // NBUF + 1))
                    if g >= 2:
                        # pv[g%2] freed once ScalarE copied group g-2
                        tensor.wait_ge(s_sv, g - 1)
                    for c in range(NCH):
                        ins = tensor.matmul(
                            out=pv[g % 2][:, :],
                            lhsT=tabw_t[:, 4 * c:4 * (c + 1)],
                            rhs=in_t[j][:, NG * c:NG * (c + 1)],
                            start=(c == 0),
                            stop=(c == NCH - 1),
                        )
                        if c == NCH - 1:
                            ins.then_inc(s_mm, 1)
                q = g - 1
                if q >= 0:
                    tensor.wait_ge(s_sv, q + 1)
                    if q >= 2:
                        # pt[q%2] freed once DVE's iv-copy of the last tile
                        # of group q-2 retired (copy is op 1 of DVE_OPS)
                        need_tile = 4 * (q - 2) + 3
                        tensor.wait_ge(s_dve, DVE_OPS * need_tile + 1)
                    for k in range(4):
                        tensor.transpose(
                            out=pt[q % 2][:, 4 * k:4 * (k + 1)],
                            in_=sval[q % 2][:, P * k:P * (k + 1)],
                            identity=tabid_t[:],
                        ).then_inc(s_T, 1)

        @block.scalar
        def _(scalar: bass.BassEngine):
            for g in range(G + 1):
                if g < G:
                    scalar.wait_ge(s_mm, g + 1)
                    if g >= 2:
                        # sval[g%2] freed once transposes of group g-2 done
                        scalar.wait_ge(s_T, 4 * (g - 1))
                    scalar.activation(
                        out=sval[g % 2][:, :], in_=pv[g % 2][:, :],
                        func=mybir.ActivationFunctionType.Copy,
                    ).then_inc(s_sv, 1)
                q = g - 1
                if q >= 0:
                    jo = q % OBUF
                    r0 = q * NG
                    src = og[jo][:].rearrange("p (t two r) -> p t two r",
                                              t=4, two=2)
                    dst0 = out_d[0, r0:r0 + NG, :].rearrange(
                        "(t p) r -> p t r", p=P)
                    dst1 = out_d[1, r0:r0 + NG, :].rearrange(
                        "(t p) r -> p t r", p=P)
                    scalar.wait_ge(s_comp, 4 * (q + 1))
                    scalar.dma_start(
                        out=dst0, in_=src[:, :, 0, :]
                    ).then_inc(s_store[jo], 16)
                    scalar.dma_start(
                        out=dst1, in_=src[:, :, 1, :]
                    ).then_inc(s_store[jo], 16)

        @block.vector
        def _(vector: bass.BassEngine):
            n = 0  # statically tracked s_dve count
            for t in range(N_TILES):
                q = t // 4
                k = t % 4
                pr = t % 2
                jo = q % OBUF
                vector.wait_ge(s_T, t + 1)
                if t >= 2:
                    # tile t-2 (same parity) fully retired before its
                    # temporaries are reused
                    vector.wait_ge(s_comp, t - 1)
                if k == 0 and q >= OBUF:
                    vector.wait_ge(s_store[jo], 32 * (q // OBUF))
                # int cast of the four 16-bit halves [a_lo a_hi b_lo b_hi]
                vector.tensor_copy(
                    iv[pr][:, 0:4], pt[q % 2][:, 4 * k:4 * (k + 1)]
                ).then_inc(s_dve, 1)
                n += 1
                vector.wait_ge(s_dve, n)
                # raw 17-bit sums of halves, and xor of halves
                vector.tensor_tensor(
                    out=iv[pr][:, 4:6], in0=iv[pr][:, 0:2],
                    in1=iv[pr][:, 2:4],
                    op=mybir.AluOpType.add).then_inc(s_dve, 1)
                vector.tensor_tensor(
                    out=v4[pr][:, 2:4], in0=iv[pr][:, 0:2],
                    in1=iv[pr][:, 2:4],
                    op=mybir.AluOpType.bitwise_xor).then_inc(s_dve, 1)
                n += 2
                vector.wait_ge(s_dve, n)
                # carry lo->hi: s_lo' = s_lo & 65535 ; s_hi' = (s_lo>=2^16)+s_hi
                vector.tensor_scalar(
                    out=v4[pr][:, 0:1], in0=iv[pr][:, 4:5], scalar1=65535,
                    scalar2=None,
                    op0=mybir.AluOpType.bitwise_and).then_inc(s_dve, 1)
                vector.scalar_tensor_tensor(
                    out=v4[pr][:, 1:2], in0=iv[pr][:, 4:5], scalar=65536,
                    in1=iv[pr][:, 5:6],
                    op0=mybir.AluOpType.is_ge,
                    op1=mybir.AluOpType.add).then_inc(s_dve, 1)
                n += 2
                vector.wait_ge(s_dve, n)
                # byte extract (fused shift+mask); idx8 holds the bytes in
                # [s0 s2 x0 x2 | s1 s3 x1 x3] order
                vector.tensor_scalar(
                    out=idx8[pr][:, 0:4],
                    in0=v4[pr][:], scalar1=255, scalar2=None,
                    op0=mybir.AluOpType.bitwise_and).then_inc(s_dve, 1)
                vector.tensor_scalar(
                    out=idx8[pr][:, 4:8],
                    in0=v4[pr][:], scalar1=8, scalar2=255,
                    op0=mybir.AluOpType.logical_shift_right,
                    op1=mybir.AluOpType.bitwise_and).then_inc(s_dve, 1)
                n += 2
                vector.wait_ge(s_dve, n)
                vector.tensor_copy(idxf[pr][:], idx8[pr][:]).then_inc(s_dve, 1)
                n += 1
                vector.wait_ge(s_dve, n)
                # encode: 8 single-src is_equal ops against the u8 iota table,
                # one per output byte, per-partition scalar = that byte's value
                perm = [0, 4, 1, 5, 2, 6, 3, 7]
                for e in range(8):
                    ins = vector.tensor_scalar(
                        out=og[jo][:, 2048 * k + 256 * e:
                                   2048 * k + 256 * (e + 1)],
                        in0=tabio_t[:],
                        scalar1=idxf[pr][:, perm[e]:perm[e] + 1],
                        scalar2=None,
                        op0=mybir.AluOpType.is_equal,
                    )
                    if e == 7:
                        ins.then_inc(s_comp, 1)

    return nc


def _make_tables():
    pos = np.arange(P, dtype=np.float64)
    w = np.zeros((NCH, P, 4), np.float64)
    for s in range(8):
        col = s // 2 if s < 4 else 2 + (s - 4) // 2
        mul = 1.0 if (s % 2 == 0) else 256.0
        for h in range(2):
            c = 2 * s + h
            w[c, :, col] = (pos + 128.0 * h) * mul
    tabw = w.transpose(1, 0, 2).reshape(P, NCH * 4).astype(ml_dtypes.bfloat16)
    tabio = np.tile(np.arange(256, dtype=np.uint8)[None, :], (P, 1))
    tabsh = np.tile(np.array([0, 8] * 4, np.int32)[None, :], (P, 1))
    tabid = np.eye(4, dtype=np.float32)
    return tabw, tabio, tabsh, tabid


def _pack_inputs(a: np.ndarray, b: np.ndarray) -> np.ndarray:
    """[B,4,256] f32 x2 -> [NCH, P, B] fp8 slab-major transposed chunks."""
    at = np.ascontiguousarray(a.reshape(B, 4, 256).transpose(1, 2, 0))
    bt = np.ascontiguousarray(b.reshape(B, 4, 256).transpose(1, 2, 0))
    ab = np.concatenate([at, bt], axis=0)            # [8, 256, B]
    return ab.reshape(NCH, P, B).astype(ml_dtypes.float8_e4m3)


_NC_CACHE = {}


def _get_nc(variant: str = "main"):
    if variant not in _NC_CACHE:
        _NC_CACHE[variant] = _build_nc()
    return _NC_CACHE[variant]


def _run(a: np.ndarray, b: np.ndarray, **spmd_kwargs):
    assert a.shape == (B, 4, 256) and b.shape == (B, 4, 256)
    abt = _pack_inputs(np.asarray(a, np.float32), np.asarray(b, np.float32))
    tabw, tabio, tabsh, tabid = _make_tables()
    in_maps = [
        {
            "abt": np.ascontiguousarray(abt[:, :, i * B_LOC:(i + 1) * B_LOC]),
            "tabw": tabw,
            "tabio": tabio,
            "tabsh": tabsh,
            "tabid": tabid,
        }
        for i in range(N_CORES)
    ]
    nc = _get_nc()
    kr = run_bass_kernel_spmd(nc, in_maps, list(range(N_CORES)), **spmd_kwargs)
    shards = [kr.results[i]["out"] for i in range(N_CORES)]
    out = np.concatenate(shards, axis=1).astype(np.float32)
    return out.reshape(2, B, 4, 256), kr


def kernel(a: np.ndarray, b: np.ndarray) -> np.ndarray:
    out, _ = _run(a, b)
    return out


def run_sim():
    """CoreSim one core vs numpy oracle (invoked by test.py --sim)."""
    from concourse.bass_interp import CoreSim

    rng = np.random.default_rng(1)
    Bl = B_LOC
    ai = rng.integers(0, 256, (Bl, 4))
    bi = rng.integers(0, 256, (Bl, 4))
    ai[0] = [255] * 4
    bi[0] = [255] * 4
    ai[1] = [255, 255, 255, 255]
    bi[1] = [1, 0, 0, 0]
    a = np.zeros((Bl, 4, 256), np.float32)
    b = np.zeros((Bl, 4, 256), np.float32)
    r = np.arange(Bl)[:, None]
    j = np.arange(4)[None, :]
    a[r, j, ai] = 1.0
    b[r, j, bi] = 1.0

    at = np.ascontiguousarray(a.transpose(1, 2, 0))
    bt = np.ascontiguousarray(b.transpose(1, 2, 0))
    abt = np.concatenate([at, bt], 0).reshape(NCH, P, Bl).astype(
        ml_dtypes.float8_e4m3)
    tabw, tabio, tabsh, tabid = _make_tables()

    nc = _get_nc()
    sim = CoreSim(nc)
    sim.tensor("abt")[:] = abt
    sim.tensor("tabw")[:] = tabw
    sim.tensor("tabio")[:] = tabio
    sim.tensor("tabsh")[:] = tabsh
    sim.tensor("tabid")[:] = tabid
    sim.simulate()
    out = np.array(sim.tensor("out")).astype(np.float32).reshape(2, Bl, 4, 256)

    # numpy oracle
    pw = (256 ** np.arange(4)).astype(np.int64)
    a32 = (ai * pw).sum(-1)
    b32 = (bi * pw).sum(-1)
    s32 = (a32 + b32) % (2 ** 32)
    x32 = a32 ^ b32
    sb_ = np.stack([(s32 >> (8 * i)) & 255 for i in range(4)], -1)
    xb_ = np.stack([(x32 >> (8 * i)) & 255 for i in range(4)], -1)
    exp = np.zeros((2, Bl, 4, 256), np.float32)
    exp[0, r, j, sb_] = 1.0
    exp[1, r, j, xb_] = 1.0
    err = np.abs(out - exp).max()
    print(f"SIM max abs err: {err}")
    assert err == 0.0, "sim mismatch"
    print("SIM PASS")


# revision 12
# speedup vs baseline: 1.7475x; 1.1515x over previous
"""MoE-ALU (add with carry + xor over one-hot byte encodings) on 8 NeuronCores.

Semantics (validated against the jax reference bit-exactly): inputs a, b are
exact one-hot byte encodings [B, 4, 256] (little-endian bytes of 32-bit ints);
with SCALE=100 every softmax in the reference collapses to an exact one-hot, so

    out[0] = one_hot bytes of (a_int + b_int) mod 2^32
    out[1] = one_hot bytes of (a_int ^ b_int)

V3 layout: the host stores the one-hot inputs slab-major+transposed as fp8
([slab, position, batch], values 0.0/1.0 are exact in fp8e4) and the outputs
as uint8 one-hots (exact 0/1), so the device moves 8 MiB in + 8 MiB out per
core instead of 32+32 for f32 batch-major. All compute still happens on
device; the host only reorders/recodes losslessly.

Device pipeline per 512-row batch group (8 groups per core):
  decode  TensorE: 16 accumulating matmuls (K=128 chunk each) of the fp8
          one-hot slabs against bf16 iota/256*iota weight columns produce
          PSUM [4, 512] = (a_lo16, a_hi16, b_lo16, b_hi16) 16-bit halves,
          exact in f32.
  stage   ScalarE copies PSUM -> SBUF f32 (frees the bank for group g+2).
  flip    TensorE transposes [4, 128] -> PSUM [128, 4] per 128-row tile.
  alu     VectorE per tile: int32 cast, halves add / xor, carry fold,
          shift/mask -> 8 byte indices; ops are [128, <=8] wide.
  encode  8x tensor_scalar(is_equal) with per-partition scalar pointer
          against a u8 iota table -> uint8 one-hot [128, 2048] (single-src
          op: runs in the DVE 2x_2p perf mode).
  store   ScalarE issues the two output DMAs (add half / xor half).

Raw Bass (one sync wait per instruction); rotating per-slot semaphores gate
buffer reuse; DVE same-engine RAW steps wait on a monotonically counted
semaphore (DVE ops do not self-interlock).
"""
from contextlib import ExitStack

import numpy as np
import ml_dtypes

import concourse.bass as bass
from concourse import mybir
from concourse.bass_utils import run_bass_kernel_spmd

F32 = mybir.dt.float32
I32 = mybir.dt.int32
U8 = mybir.dt.uint8
BF16 = mybir.dt.bfloat16
FP8 = mybir.dt.float8e4

P = 128
N_CORES = 8
B = 32768
B_LOC = B // N_CORES          # 4096 rows per core
ROW = 4 * 256                 # 1024 per row per tensor
NG = 512                      # batch rows per matmul group (one PSUM bank)
G = B_LOC // NG               # 8 groups
N_TILES = B_LOC // P          # 32 tiles of 128 rows
NCH = 16                      # K-chunks: 8 slabs (a0..a3,b0..b3) x 2 halves

NBUF = 4                      # input group-buffer slots
OBUF = 3                      # output group-buffer slots
NSUB = 4                      # input sub-DMAs per group (4 chunks each)

DVE_OPS = 7                   # s_dve increments per tile (chain ops)
GP_BYTES = ()                 # encode bytes handled by GPSIMD
DVE_BYTES = tuple(e for e in range(8) if e not in GP_BYTES)
PERM = [0, 4, 1, 5, 2, 6, 3, 7]  # output byte e -> idxf column


def _build_nc() -> bass.Bass:
    nc = bass.Bass(trn_type="TRN2")
    ab_d = nc.dram_tensor("abt", [NCH, P, B_LOC], FP8, kind="ExternalInput")
    tabw_d = nc.dram_tensor("tabw", [P, NCH * 4], BF16, kind="ExternalInput")
    tabio_d = nc.dram_tensor("tabio", [P, 256], BF16, kind="ExternalInput")
    tabid_d = nc.dram_tensor("tabid", [4, 4], F32, kind="ExternalInput")
    out_d = nc.dram_tensor("out", [2, B_LOC, ROW], BF16, kind="ExternalOutput")

    with ExitStack() as ctx:
        sb = lambda name, shape, dt: ctx.enter_context(
            nc.sbuf_tensor(name, shape, dt))
        tabw_t = sb("tabw_t", [P, NCH * 4], BF16)
        tabio_t = sb("tabio_t", [P, 256], BF16)
        tabid_t = sb("tabid_t", [4, 4], F32)
        in_t = [sb(f"in_t{k}", [P, NCH * NG], FP8) for k in range(NBUF)]
        sval = [sb(f"sval{k}", [4, NG], F32) for k in range(2)]
        og = [sb(f"og{k}", [P, 4 * 2 * ROW], BF16) for k in range(OBUF)]
        # parity-double-buffered per-tile temporaries
        iv = [sb(f"iv_{p}", [P, 6], I32) for p in range(2)]   # halves + s_raw
        v4 = [sb(f"v4_{p}", [P, 4], I32) for p in range(2)]   # slo shi xlo xhi
        idx8 = [sb(f"idx8_{p}", [P, 8], I32) for p in range(2)]
        idxf = [sb(f"idxf_{p}", [P, 8], F32) for p in range(2)]

        pv = [ctx.enter_context(nc.psum_tensor(f"pv{k}", [4, NG], F32))
              for k in range(2)]
        pt = [ctx.enter_context(nc.psum_tensor(f"pt{k}", [P, 16], F32))
              for k in range(2)]

        s_tabw = ctx.enter_context(nc.semaphore("s_tabw"))
        s_tabid = ctx.enter_context(nc.semaphore("s_tabid"))
        s_tabio = ctx.enter_context(nc.semaphore("s_tabio"))
        s_sub = [ctx.enter_context(nc.semaphore(f"s_sub{u}"))
                 for u in range(NSUB)]
        s_store = [ctx.enter_context(nc.semaphore(f"s_store{j}"))
                   for j in range(OBUF)]
        s_mm = ctx.enter_context(nc.semaphore("s_mm"))      # matmul groups done
        s_sv = ctx.enter_context(nc.semaphore("s_sv"))      # psum->sbuf copies
        s_T = ctx.enter_context(nc.semaphore("s_T"))        # transposes done
        s_comp = ctx.enter_context(nc.semaphore("s_comp"))  # encode tiles done
        s_dve = ctx.enter_context(nc.semaphore("s_dve"))    # chain ops done
        s_gp = ctx.enter_context(nc.semaphore("s_gp"))      # gpsimd tiles done

        block = ctx.enter_context(nc.Block())

        @block.sync
        def _(sync: bass.BassEngine):
            CS = NCH // NSUB   # chunks per sub-DMA
            def load_group(g):
                j = g % NBUF
                for u in range(NSUB):
                    c0 = u * CS
                    sync.dma_start(
                        out=in_t[j][:, NG * c0:NG * (c0 + CS)].rearrange(
                            "p (c n) -> p c n", c=CS),
                        in_=ab_d[c0:c0 + CS, :,
                                 NG * g:NG * (g + 1)].rearrange(
                            "c p n -> p c n"),
                    ).then_inc(s_sub[u], 16)
            load_group(0)
            sync.dma_start(out=tabw_t[:], in_=tabw_d[:]).then_inc(s_tabw, 16)
            sync.dma_start(out=tabid_t[:], in_=tabid_d[:]).then_inc(
                s_tabid, 16)
            sync.dma_start(out=tabio_t[:], in_=tabio_d[:]).then_inc(
                s_tabio, 16)
            for g in range(1, G):
                # serialize groups so the oldest outstanding load finishes
                # first (prefetch must not starve the critical group)
                for u in range(NSUB):
                    sync.wait_ge(s_sub[u], 16 * g)
                if g >= NBUF:
                    # slot reuse: matmuls of group g-NBUF consumed it
                    sync.wait_ge(s_mm, g - NBUF + 1)
                load_group(g)

        @block.tensor
        def _(tensor: bass.BassEngine):
            CS = NCH // NSUB
            tensor.wait_ge(s_tabw, 16)   # weights
            for g in range(G + 1):
                if g < G:
                    j = g % NBUF
                    if g >= 2:
                        # pv[g%2] freed once ScalarE copied group g-2
                        tensor.wait_ge(s_sv, g - 1)
                    for c in range(NCH):
                        if c % CS == 0:
                            tensor.wait_ge(s_sub[c // CS], 16 * (g + 1))
                        ins = tensor.matmul(
                            out=pv[g % 2][:, :],
                            lhsT=tabw_t[:, 4 * c:4 * (c + 1)],
                            rhs=in_t[j][:, NG * c:NG * (c + 1)],
                            start=(c == 0),
                            stop=(c == NCH - 1),
                        )
                        if c == NCH - 1:
                            ins.then_inc(s_mm, 1)
                q = g - 1
                if q >= 0:
                    if q == 0:
                        tensor.wait_ge(s_tabid, 16)  # identity
                    tensor.wait_ge(s_sv, q + 1)
                    if q >= 2:
                        # pt[q%2] freed once DVE's iv-copy of the last tile
                        # of group q-2 retired (copy is op 1 of DVE_OPS)
                        need_tile = 4 * (q - 2) + 3
                        tensor.wait_ge(s_dve, DVE_OPS * need_tile + 1)
                    for k in range(4):
                        tensor.transpose(
                            out=pt[q % 2][:, 4 * k:4 * (k + 1)],
                            in_=sval[q % 2][:, P * k:P * (k + 1)],
                            identity=tabid_t[:],
                        ).then_inc(s_T, 1)

        @block.scalar
        def _(scalar: bass.BassEngine):
            for g in range(G + 1):
                if g < G:
                    scalar.wait_ge(s_mm, g + 1)
                    if g >= 2:
                        # sval[g%2] freed once transposes of group g-2 done
                        scalar.wait_ge(s_T, 4 * (g - 1))
                    scalar.activation(
                        out=sval[g % 2][:, :], in_=pv[g % 2][:, :],
                        func=mybir.ActivationFunctionType.Copy,
                    ).then_inc(s_sv, 1)
                q = g - 1
                if q >= 0:
                    jo = q % OBUF
                    r0 = q * NG
                    src = og[jo][:].rearrange("p (t two r) -> p t two r",
                                              t=4, two=2)
                    dst0 = out_d[0, r0:r0 + NG, :].rearrange(
                        "(t p) r -> p t r", p=P)
                    dst1 = out_d[1, r0:r0 + NG, :].rearrange(
                        "(t p) r -> p t r", p=P)
                    scalar.wait_ge(s_comp, 4 * (q + 1))
                    if GP_BYTES:
                        scalar.wait_ge(s_gp, 4 * (q + 1))
                    scalar.dma_start(
                        out=dst0, in_=src[:, :, 0, :]
                    ).then_inc(s_store[jo], 16)
                    scalar.dma_start(
                        out=dst1, in_=src[:, :, 1, :]
                    ).then_inc(s_store[jo], 16)

        @block.vector
        def _(vector: bass.BassEngine):
            n = 0  # statically tracked s_dve count
            for t in range(N_TILES):
                q = t // 4
                k = t % 4
                pr = t % 2
                jo = q % OBUF
                if t == 0:
                    vector.wait_ge(s_tabio, 16)  # iota table
                vector.wait_ge(s_T, t + 1)
                if t >= 2:
                    # tile t-2 (same parity) fully retired before its
                    # temporaries are reused
                    vector.wait_ge(s_comp, t - 1)
                    if GP_BYTES:
                        vector.wait_ge(s_gp, t - 1)
                if k == 0 and q >= OBUF:
                    vector.wait_ge(s_store[jo], 32 * (q // OBUF))
                # int cast of the four 16-bit halves [a_lo a_hi b_lo b_hi]
                vector.tensor_copy(
                    iv[pr][:, 0:4], pt[q % 2][:, 4 * k:4 * (k + 1)]
                ).then_inc(s_dve, 1)
                n += 1
                vector.wait_ge(s_dve, n)
                # v4 = [s_lo_raw(17b), s_hi_raw, x_lo, x_hi]
                vector.tensor_tensor(
                    out=v4[pr][:, 0:2], in0=iv[pr][:, 0:2],
                    in1=iv[pr][:, 2:4],
                    op=mybir.AluOpType.add).then_inc(s_dve, 1)
                vector.tensor_tensor(
                    out=v4[pr][:, 2:4], in0=iv[pr][:, 0:2],
                    in1=iv[pr][:, 2:4],
                    op=mybir.AluOpType.bitwise_xor).then_inc(s_dve, 1)
                n += 2
                vector.wait_ge(s_dve, n)
                # fold the 2^16 carry into s_hi (s_lo_raw keeps bit 16; the
                # &255 byte masks strip it later)
                vector.scalar_tensor_tensor(
                    out=v4[pr][:, 1:2], in0=v4[pr][:, 0:1], scalar=65536,
                    in1=v4[pr][:, 1:2],
                    op0=mybir.AluOpType.is_ge,
                    op1=mybir.AluOpType.add).then_inc(s_dve, 1)
                n += 1
                vector.wait_ge(s_dve, n)
                # byte extract (fused shift+mask); idx8 holds the bytes in
                # [s0 s2 x0 x2 | s1 s3 x1 x3] order
                vector.tensor_scalar(
                    out=idx8[pr][:, 0:4],
                    in0=v4[pr][:], scalar1=255, scalar2=None,
                    op0=mybir.AluOpType.bitwise_and).then_inc(s_dve, 1)
                vector.tensor_scalar(
                    out=idx8[pr][:, 4:8],
                    in0=v4[pr][:], scalar1=8, scalar2=255,
                    op0=mybir.AluOpType.logical_shift_right,
                    op1=mybir.AluOpType.bitwise_and).then_inc(s_dve, 1)
                n += 2
                vector.wait_ge(s_dve, n)
                vector.tensor_copy(idxf[pr][:], idx8[pr][:]).then_inc(s_dve, 1)
                n += 1
                vector.wait_ge(s_dve, n)
                # encode: single-src is_equal against the iota table, one op
                # per output byte, per-partition scalar = that byte's value
                for i, e in enumerate(DVE_BYTES):
                    ins = vector.tensor_scalar(
                        out=og[jo][:, 2048 * k + 256 * e:
                                   2048 * k + 256 * (e + 1)],
                        in0=tabio_t[:],
                        scalar1=idxf[pr][:, PERM[e]:PERM[e] + 1],
                        scalar2=None,
                        op0=mybir.AluOpType.is_equal,
                    )
                    if i == len(DVE_BYTES) - 1:
                        ins.then_inc(s_comp, 1)

        @block.gpsimd
        def _(gp: bass.BassEngine):
            if not GP_BYTES:
                return
            for t in range(N_TILES):
                q = t // 4
                k = t % 4
                pr = t % 2
                jo = q % OBUF
                if t == 0:
                    gp.wait_ge(s_tabio, 16)  # iota table
                gp.wait_ge(s_dve, DVE_OPS * (t + 1))
                if k == 0 and q >= OBUF:
                    gp.wait_ge(s_store[jo], 32 * (q // OBUF))
                for i, e in enumerate(GP_BYTES):
                    ins = gp.tensor_scalar(
                        out=og[jo][:, 2048 * k + 256 * e:
                                   2048 * k + 256 * (e + 1)],
                        in0=tabio_t[:],
                        scalar1=idxf[pr][:, PERM[e]:PERM[e] + 1],
                        scalar2=None,
                        op0=mybir.AluOpType.is_equal,
                    )
                    if i == len(GP_BYTES) - 1:
                        ins.then_inc(s_gp, 1)

    return nc


def _make_tables():
    pos = np.arange(P, dtype=np.float64)
    w = np.zeros((NCH, P, 4), np.float64)
    for s in range(8):
        col = s // 2 if s < 4 else 2 + (s - 4) // 2
        mul = 1.0 if (s % 2 == 0) else 256.0
        for h in range(2):
            c = 2 * s + h
            w[c, :, col] = (pos + 128.0 * h) * mul
    tabw = w.transpose(1, 0, 2).reshape(P, NCH * 4).astype(ml_dtypes.bfloat16)
    tabio = np.tile(np.arange(256).astype(ml_dtypes.bfloat16)[None, :],
                    (P, 1))
    tabid = np.eye(4, dtype=np.float32)
    return tabw, tabio, tabid


def _pack_inputs(a: np.ndarray, b: np.ndarray) -> np.ndarray:
    """[B,4,256] f32 x2 -> [NCH, P, B] fp8 slab-major transposed chunks."""
    at = np.ascontiguousarray(a.reshape(B, 4, 256).transpose(1, 2, 0))
    bt = np.ascontiguousarray(b.reshape(B, 4, 256).transpose(1, 2, 0))
    ab = np.concatenate([at, bt], axis=0)            # [8, 256, B]
    return ab.reshape(NCH, P, B).astype(ml_dtypes.float8_e4m3)


_NC_CACHE = {}


def _get_nc(variant: str = "main"):
    if variant not in _NC_CACHE:
        _NC_CACHE[variant] = _build_nc()
    return _NC_CACHE[variant]


def _run(a: np.ndarray, b: np.ndarray, **spmd_kwargs):
    assert a.shape == (B, 4, 256) and b.shape == (B, 4, 256)
    abt = _pack_inputs(np.asarray(a, np.float32), np.asarray(b, np.float32))
    tabw, tabio, tabid = _make_tables()
    in_maps = [
        {
            "abt": np.ascontiguousarray(abt[:, :, i * B_LOC:(i + 1) * B_LOC]),
            "tabw": tabw,
            "tabio": tabio,
            "tabid": tabid,
        }
        for i in range(N_CORES)
    ]
    nc = _get_nc()
    kr = run_bass_kernel_spmd(nc, in_maps, list(range(N_CORES)), **spmd_kwargs)
    shards = [kr.results[i]["out"] for i in range(N_CORES)]
    out = np.concatenate(shards, axis=1).astype(np.float32)
    return out.reshape(2, B, 4, 256), kr


def kernel(a: np.ndarray, b: np.ndarray) -> np.ndarray:
    out, _ = _run(a, b)
    return out


def run_sim():
    """CoreSim one core vs numpy oracle (invoked by test.py --sim)."""
    from concourse.bass_interp import CoreSim

    rng = np.random.default_rng(1)
    Bl = B_LOC
    ai = rng.integers(0, 256, (Bl, 4))
    bi = rng.integers(0, 256, (Bl, 4))
    ai[0] = [255] * 4
    bi[0] = [255] * 4
    ai[1] = [255, 255, 255, 255]
    bi[1] = [1, 0, 0, 0]
    a = np.zeros((Bl, 4, 256), np.float32)
    b = np.zeros((Bl, 4, 256), np.float32)
    r = np.arange(Bl)[:, None]
    j = np.arange(4)[None, :]
    a[r, j, ai] = 1.0
    b[r, j, bi] = 1.0

    at = np.ascontiguousarray(a.transpose(1, 2, 0))
    bt = np.ascontiguousarray(b.transpose(1, 2, 0))
    abt = np.concatenate([at, bt], 0).reshape(NCH, P, Bl).astype(
        ml_dtypes.float8_e4m3)
    tabw, tabio, tabid = _make_tables()

    nc = _get_nc()
    sim = CoreSim(nc)
    sim.tensor("abt")[:] = abt
    sim.tensor("tabw")[:] = tabw
    sim.tensor("tabio")[:] = tabio
    sim.tensor("tabid")[:] = tabid
    sim.simulate()
    out = np.array(sim.tensor("out")).astype(np.float32).reshape(2, Bl, 4, 256)

    # numpy oracle
    pw = (256 ** np.arange(4)).astype(np.int64)
    a32 = (ai * pw).sum(-1)
    b32 = (bi * pw).sum(-1)
    s32 = (a32 + b32) % (2 ** 32)
    x32 = a32 ^ b32
    sb_ = np.stack([(s32 >> (8 * i)) & 255 for i in range(4)], -1)
    xb_ = np.stack([(x32 >> (8 * i)) & 255 for i in range(4)], -1)
    exp = np.zeros((2, Bl, 4, 256), np.float32)
    exp[0, r, j, sb_] = 1.0
    exp[1, r, j, xb_] = 1.0
    err = np.abs(out - exp).max()
    print(f"SIM max abs err: {err}")
    assert err == 0.0, "sim mismatch"
    print("SIM PASS")


# revision 13
# speedup vs baseline: 1.7629x; 1.0088x over previous
"""MoE-ALU (add with carry + xor over one-hot byte encodings) on 8 NeuronCores.

Semantics (validated against the jax reference bit-exactly): inputs a, b are
exact one-hot byte encodings [B, 4, 256] (little-endian bytes of 32-bit ints);
with SCALE=100 every softmax in the reference collapses to an exact one-hot, so

    out[0] = one_hot bytes of (a_int + b_int) mod 2^32
    out[1] = one_hot bytes of (a_int ^ b_int)

V3 layout: the host stores the one-hot inputs slab-major+transposed as fp8
([slab, position, batch], values 0.0/1.0 are exact in fp8e4) and the outputs
as uint8 one-hots (exact 0/1), so the device moves 8 MiB in + 8 MiB out per
core instead of 32+32 for f32 batch-major. All compute still happens on
device; the host only reorders/recodes losslessly.

Device pipeline per 512-row batch group (8 groups per core):
  decode  TensorE: 16 accumulating matmuls (K=128 chunk each) of the fp8
          one-hot slabs against bf16 iota/256*iota weight columns produce
          PSUM [4, 512] = (a_lo16, a_hi16, b_lo16, b_hi16) 16-bit halves,
          exact in f32.
  stage   ScalarE copies PSUM -> SBUF f32 (frees the bank for group g+2).
  flip    TensorE transposes [4, 128] -> PSUM [128, 4] per 128-row tile.
  alu     VectorE per tile: int32 cast, halves add / xor, carry fold,
          shift/mask -> 8 byte indices; ops are [128, <=8] wide.
  encode  8x tensor_scalar(is_equal) with per-partition scalar pointer
          against a u8 iota table -> uint8 one-hot [128, 2048] (single-src
          op: runs in the DVE 2x_2p perf mode).
  store   ScalarE issues the two output DMAs (add half / xor half).

Raw Bass (one sync wait per instruction); rotating per-slot semaphores gate
buffer reuse; DVE same-engine RAW steps wait on a monotonically counted
semaphore (DVE ops do not self-interlock).
"""
from contextlib import ExitStack

import numpy as np
import ml_dtypes

import concourse.bass as bass
from concourse import mybir
from concourse.bass_utils import run_bass_kernel_spmd

F32 = mybir.dt.float32
I32 = mybir.dt.int32
U8 = mybir.dt.uint8
BF16 = mybir.dt.bfloat16
FP8 = mybir.dt.float8e4

P = 128
N_CORES = 8
B = 32768
B_LOC = B // N_CORES          # 4096 rows per core
ROW = 4 * 256                 # 1024 per row per tensor
NG = 512                      # batch rows per matmul group (one PSUM bank)
G = B_LOC // NG               # 8 groups
N_TILES = B_LOC // P          # 32 tiles of 128 rows
NCH = 16                      # K-chunks: 8 slabs (a0..a3,b0..b3) x 2 halves

NBUF = 4                      # input group-buffer slots
OBUF = 4                      # output group-buffer slots
NSUB = 2                      # input sub-DMAs per group (8 chunks each)
INFLIGHT = 2                  # concurrent group loads

DVE_OPS = 7                   # s_dve increments per tile (chain ops)
GP_BYTES = ()                 # encode bytes handled by GPSIMD
DVE_BYTES = tuple(e for e in range(8) if e not in GP_BYTES)
PERM = [0, 4, 1, 5, 2, 6, 3, 7]  # output byte e -> idxf column


def _build_nc() -> bass.Bass:
    nc = bass.Bass(trn_type="TRN2")
    ab_d = nc.dram_tensor("abt", [NCH, P, B_LOC], FP8, kind="ExternalInput")
    tabw_d = nc.dram_tensor("tabw", [P, NCH * 4], BF16, kind="ExternalInput")
    tabio_d = nc.dram_tensor("tabio", [P, 256], BF16, kind="ExternalInput")
    tabid_d = nc.dram_tensor("tabid", [4, 4], F32, kind="ExternalInput")
    out_d = nc.dram_tensor("out", [2, B_LOC, ROW], BF16, kind="ExternalOutput")

    with ExitStack() as ctx:
        sb = lambda name, shape, dt: ctx.enter_context(
            nc.sbuf_tensor(name, shape, dt))
        tabw_t = sb("tabw_t", [P, NCH * 4], BF16)
        tabio_t = sb("tabio_t", [P, 256], BF16)
        tabid_t = sb("tabid_t", [4, 4], F32)
        in_t = [sb(f"in_t{k}", [P, NCH * NG], FP8) for k in range(NBUF)]
        sval = [sb(f"sval{k}", [4, NG], F32) for k in range(2)]
        og = [sb(f"og{k}", [P, 4 * 2 * ROW], BF16) for k in range(OBUF)]
        # parity-double-buffered per-tile temporaries
        iv = [sb(f"iv_{p}", [P, 6], I32) for p in range(2)]   # halves + s_raw
        v4 = [sb(f"v4_{p}", [P, 4], I32) for p in range(2)]   # slo shi xlo xhi
        idx8 = [sb(f"idx8_{p}", [P, 8], I32) for p in range(2)]
        idxf = [sb(f"idxf_{p}", [P, 8], F32) for p in range(2)]

        pv = [ctx.enter_context(nc.psum_tensor(f"pv{k}", [4, NG], F32))
              for k in range(2)]
        pt = [ctx.enter_context(nc.psum_tensor(f"pt{k}", [P, 16], F32))
              for k in range(2)]

        s_tabw = ctx.enter_context(nc.semaphore("s_tabw"))
        s_tabid = ctx.enter_context(nc.semaphore("s_tabid"))
        s_tabio = ctx.enter_context(nc.semaphore("s_tabio"))
        s_sub = [[ctx.enter_context(nc.semaphore(f"s_sub{j}_{u}"))
                  for u in range(NSUB)] for j in range(NBUF)]
        s_store = [ctx.enter_context(nc.semaphore(f"s_store{j}"))
                   for j in range(OBUF)]
        s_mm = ctx.enter_context(nc.semaphore("s_mm"))      # matmul groups done
        s_sv = ctx.enter_context(nc.semaphore("s_sv"))      # psum->sbuf copies
        s_T = ctx.enter_context(nc.semaphore("s_T"))        # transposes done
        s_comp = ctx.enter_context(nc.semaphore("s_comp"))  # encode tiles done
        s_dve = ctx.enter_context(nc.semaphore("s_dve"))    # chain ops done
        s_gp = ctx.enter_context(nc.semaphore("s_gp"))      # gpsimd tiles done

        block = ctx.enter_context(nc.Block())

        @block.sync
        def _(sync: bass.BassEngine):
            CS = NCH // NSUB   # chunks per sub-DMA
            def load_group(g):
                j = g % NBUF
                for u in range(NSUB):
                    c0 = u * CS
                    sync.dma_start(
                        out=in_t[j][:, NG * c0:NG * (c0 + CS)].rearrange(
                            "p (c n) -> p c n", c=CS),
                        in_=ab_d[c0:c0 + CS, :,
                                 NG * g:NG * (g + 1)].rearrange(
                            "c p n -> p c n"),
                    ).then_inc(s_sub[j][u], 16)
            sync.dma_start(out=tabw_t[:], in_=tabw_d[:]).then_inc(s_tabw, 16)
            load_group(0)
            sync.dma_start(out=tabid_t[:], in_=tabid_d[:]).then_inc(
                s_tabid, 16)
            sync.dma_start(out=tabio_t[:], in_=tabio_d[:]).then_inc(
                s_tabio, 16)
            for g in range(1, G):
                if g >= INFLIGHT:
                    # bounded prefetch: group g-INFLIGHT fully landed first
                    gp_ = g - INFLIGHT
                    for u in range(NSUB):
                        sync.wait_ge(s_sub[gp_ % NBUF][u],
                                     16 * (gp_ // NBUF + 1))
                if g >= NBUF:
                    # slot reuse: matmuls of group g-NBUF consumed it
                    sync.wait_ge(s_mm, g - NBUF + 1)
                load_group(g)

        @block.tensor
        def _(tensor: bass.BassEngine):
            CS = NCH // NSUB
            tensor.wait_ge(s_tabw, 16)   # weights
            for g in range(G + 1):
                if g < G:
                    j = g % NBUF
                    if g >= 2:
                        # pv[g%2] freed once ScalarE copied group g-2
                        tensor.wait_ge(s_sv, g - 1)
                    for c in range(NCH):
                        if c % CS == 0:
                            tensor.wait_ge(s_sub[j][c // CS],
                                           16 * (g // NBUF + 1))
                        ins = tensor.matmul(
                            out=pv[g % 2][:, :],
                            lhsT=tabw_t[:, 4 * c:4 * (c + 1)],
                            rhs=in_t[j][:, NG * c:NG * (c + 1)],
                            start=(c == 0),
                            stop=(c == NCH - 1),
                        )
                        if c == NCH - 1:
                            ins.then_inc(s_mm, 1)
                q = g - 1
                if q >= 0:
                    if q == 0:
                        tensor.wait_ge(s_tabid, 16)  # identity
                    tensor.wait_ge(s_sv, q + 1)
                    if q >= 2:
                        # pt[q%2] freed once DVE's iv-copy of the last tile
                        # of group q-2 retired (copy is op 1 of DVE_OPS)
                        need_tile = 4 * (q - 2) + 3
                        tensor.wait_ge(s_dve, DVE_OPS * need_tile + 1)
                    for k in range(4):
                        tensor.transpose(
                            out=pt[q % 2][:, 4 * k:4 * (k + 1)],
                            in_=sval[q % 2][:, P * k:P * (k + 1)],
                            identity=tabid_t[:],
                        ).then_inc(s_T, 1)

        @block.scalar
        def _(scalar: bass.BassEngine):
            for g in range(G + 1):
                if g < G:
                    scalar.wait_ge(s_mm, g + 1)
                    if g >= 2:
                        # sval[g%2] freed once transposes of group g-2 done
                        scalar.wait_ge(s_T, 4 * (g - 1))
                    scalar.activation(
                        out=sval[g % 2][:, :], in_=pv[g % 2][:, :],
                        func=mybir.ActivationFunctionType.Copy,
                    ).then_inc(s_sv, 1)
                q = g - 1
                if q >= 0:
                    jo = q % OBUF
                    r0 = q * NG
                    src = og[jo][:].rearrange("p (t two r) -> p t two r",
                                              t=4, two=2)
                    dst0 = out_d[0, r0:r0 + NG, :].rearrange(
                        "(t p) r -> p t r", p=P)
                    dst1 = out_d[1, r0:r0 + NG, :].rearrange(
                        "(t p) r -> p t r", p=P)
                    scalar.wait_ge(s_comp, 4 * (q + 1))
                    if GP_BYTES:
                        scalar.wait_ge(s_gp, 4 * (q + 1))
                    scalar.dma_start(
                        out=dst0, in_=src[:, :, 0, :]
                    ).then_inc(s_store[jo], 16)
                    scalar.dma_start(
                        out=dst1, in_=src[:, :, 1, :]
                    ).then_inc(s_store[jo], 16)

        @block.vector
        def _(vector: bass.BassEngine):
            n = 0  # statically tracked s_dve count
            for t in range(N_TILES):
                q = t // 4
                k = t % 4
                pr = t % 2
                jo = q % OBUF
                if t == 0:
                    vector.wait_ge(s_tabio, 16)  # iota table
                vector.wait_ge(s_T, t + 1)
                if t >= 2:
                    # tile t-2 (same parity) fully retired before its
                    # temporaries are reused
                    vector.wait_ge(s_comp, t - 1)
                    if GP_BYTES:
                        vector.wait_ge(s_gp, t - 1)
                if k == 0 and q >= OBUF:
                    vector.wait_ge(s_store[jo], 32 * (q // OBUF))
                # int cast of the four 16-bit halves [a_lo a_hi b_lo b_hi]
                vector.tensor_copy(
                    iv[pr][:, 0:4], pt[q % 2][:, 4 * k:4 * (k + 1)]
                ).then_inc(s_dve, 1)
                n += 1
                vector.wait_ge(s_dve, n)
                # v4 = [s_lo_raw(17b), s_hi_raw, x_lo, x_hi]
                vector.tensor_tensor(
                    out=v4[pr][:, 0:2], in0=iv[pr][:, 0:2],
                    in1=iv[pr][:, 2:4],
                    op=mybir.AluOpType.add).then_inc(s_dve, 1)
                vector.tensor_tensor(
                    out=v4[pr][:, 2:4], in0=iv[pr][:, 0:2],
                    in1=iv[pr][:, 2:4],
                    op=mybir.AluOpType.bitwise_xor).then_inc(s_dve, 1)
                n += 2
                vector.wait_ge(s_dve, n)
                # fold the 2^16 carry into s_hi (s_lo_raw keeps bit 16; the
                # &255 byte masks strip it later)
                vector.scalar_tensor_tensor(
                    out=v4[pr][:, 1:2], in0=v4[pr][:, 0:1], scalar=65536,
                    in1=v4[pr][:, 1:2],
                    op0=mybir.AluOpType.is_ge,
                    op1=mybir.AluOpType.add).then_inc(s_dve, 1)
                n += 1
                vector.wait_ge(s_dve, n)
                # byte extract (fused shift+mask); idx8 holds the bytes in
                # [s0 s2 x0 x2 | s1 s3 x1 x3] order
                vector.tensor_scalar(
                    out=idx8[pr][:, 0:4],
                    in0=v4[pr][:], scalar1=255, scalar2=None,
                    op0=mybir.AluOpType.bitwise_and).then_inc(s_dve, 1)
                vector.tensor_scalar(
                    out=idx8[pr][:, 4:8],
                    in0=v4[pr][:], scalar1=8, scalar2=255,
                    op0=mybir.AluOpType.logical_shift_right,
                    op1=mybir.AluOpType.bitwise_and).then_inc(s_dve, 1)
                n += 2
                vector.wait_ge(s_dve, n)
                vector.tensor_copy(idxf[pr][:], idx8[pr][:]).then_inc(s_dve, 1)
                n += 1
                vector.wait_ge(s_dve, n)
                # encode: single-src is_equal against the iota table, one op
                # per output byte, per-partition scalar = that byte's value
                for i, e in enumerate(DVE_BYTES):
                    ins = vector.tensor_scalar(
                        out=og[jo][:, 2048 * k + 256 * e:
                                   2048 * k + 256 * (e + 1)],
                        in0=tabio_t[:],
                        scalar1=idxf[pr][:, PERM[e]:PERM[e] + 1],
                        scalar2=None,
                        op0=mybir.AluOpType.is_equal,
                    )
                    if i == len(DVE_BYTES) - 1:
                        ins.then_inc(s_comp, 1)

        @block.gpsimd
        def _(gp: bass.BassEngine):
            if not GP_BYTES:
                return
            for t in range(N_TILES):
                q = t // 4
                k = t % 4
                pr = t % 2
                jo = q % OBUF
                if t == 0:
                    gp.wait_ge(s_tabio, 16)  # iota table
                gp.wait_ge(s_dve, DVE_OPS * (t + 1))
                if k == 0 and q >= OBUF:
                    gp.wait_ge(s_store[jo], 32 * (q // OBUF))
                for i, e in enumerate(GP_BYTES):
                    ins = gp.tensor_scalar(
                        out=og[jo][:, 2048 * k + 256 * e:
                                   2048 * k + 256 * (e + 1)],
                        in0=tabio_t[:],
                        scalar1=idxf[pr][:, PERM[e]:PERM[e] + 1],
                        scalar2=None,
                        op0=mybir.AluOpType.is_equal,
                    )
                    if i == len(GP_BYTES) - 1:
                        ins.then_inc(s_gp, 1)

    return nc


def _make_tables():
    pos = np.arange(P, dtype=np.float64)
    w = np.zeros((NCH, P, 4), np.float64)
    for s in range(8):
        col = s // 2 if s < 4 else 2 + (s - 4) // 2
        mul = 1.0 if (s % 2 == 0) else 256.0
        for h in range(2):
            c = 2 * s + h
            w[c, :, col] = (pos + 128.0 * h) * mul
    tabw = w.transpose(1, 0, 2).reshape(P, NCH * 4).astype(ml_dtypes.bfloat16)
    tabio = np.tile(np.arange(256).astype(ml_dtypes.bfloat16)[None, :],
                    (P, 1))
    tabid = np.eye(4, dtype=np.float32)
    return tabw, tabio, tabid


def _pack_inputs(a: np.ndarray, b: np.ndarray) -> np.ndarray:
    """[B,4,256] f32 x2 -> [NCH, P, B] fp8 slab-major transposed chunks."""
    at = np.ascontiguousarray(a.reshape(B, 4, 256).transpose(1, 2, 0))
    bt = np.ascontiguousarray(b.reshape(B, 4, 256).transpose(1, 2, 0))
    ab = np.concatenate([at, bt], axis=0)            # [8, 256, B]
    return ab.reshape(NCH, P, B).astype(ml_dtypes.float8_e4m3)


_NC_CACHE = {}


def _get_nc(variant: str = "main"):
    if variant not in _NC_CACHE:
        _NC_CACHE[variant] = _build_nc()
    return _NC_CACHE[variant]


def _run(a: np.ndarray, b: np.ndarray, **spmd_kwargs):
    assert a.shape == (B, 4, 256) and b.shape == (B, 4, 256)
    abt = _pack_inputs(np.asarray(a, np.float32), np.asarray(b, np.float32))
    tabw, tabio, tabid = _make_tables()
    in_maps = [
        {
            "abt": np.ascontiguousarray(abt[:, :, i * B_LOC:(i + 1) * B_LOC]),
            "tabw": tabw,
            "tabio": tabio,
            "tabid": tabid,
        }
        for i in range(N_CORES)
    ]
    nc = _get_nc()
    kr = run_bass_kernel_spmd(nc, in_maps, list(range(N_CORES)), **spmd_kwargs)
    shards = [kr.results[i]["out"] for i in range(N_CORES)]
    out = np.concatenate(shards, axis=1).astype(np.float32)
    return out.reshape(2, B, 4, 256), kr


def kernel(a: np.ndarray, b: np.ndarray) -> np.ndarray:
    out, _ = _run(a, b)
    return out


def run_sim():
    """CoreSim one core vs numpy oracle (invoked by test.py --sim)."""
    from concourse.bass_interp import CoreSim

    rng = np.random.default_rng(1)
    Bl = B_LOC
    ai = rng.integers(0, 256, (Bl, 4))
    bi = rng.integers(0, 256, (Bl, 4))
    ai[0] = [255] * 4
    bi[0] = [255] * 4
    ai[1] = [255, 255, 255, 255]
    bi[1] = [1, 0, 0, 0]
    a = np.zeros((Bl, 4, 256), np.float32)
    b = np.zeros((Bl, 4, 256), np.float32)
    r = np.arange(Bl)[:, None]
    j = np.arange(4)[None, :]
    a[r, j, ai] = 1.0
    b[r, j, bi] = 1.0

    at = np.ascontiguousarray(a.transpose(1, 2, 0))
    bt = np.ascontiguousarray(b.transpose(1, 2, 0))
    abt = np.concatenate([at, bt], 0).reshape(NCH, P, Bl).astype(
        ml_dtypes.float8_e4m3)
    tabw, tabio, tabid = _make_tables()

    nc = _get_nc()
    sim = CoreSim(nc)
    sim.tensor("abt")[:] = abt
    sim.tensor("tabw")[:] = tabw
    sim.tensor("tabio")[:] = tabio
    sim.tensor("tabid")[:] = tabid
    sim.simulate()
    out = np.array(sim.tensor("out")).astype(np.float32).reshape(2, Bl, 4, 256)

    # numpy oracle
    pw = (256 ** np.arange(4)).astype(np.int64)
    a32 = (ai * pw).sum(-1)
    b32 = (bi * pw).sum(-1)
    s32 = (a32 + b32) % (2 ** 32)
    x32 = a32 ^ b32
    sb_ = np.stack([(s32 >> (8 * i)) & 255 for i in range(4)], -1)
    xb_ = np.stack([(x32 >> (8 * i)) & 255 for i in range(4)], -1)
    exp = np.zeros((2, Bl, 4, 256), np.float32)
    exp[0, r, j, sb_] = 1.0
    exp[1, r, j, xb_] = 1.0
    err = np.abs(out - exp).max()
    print(f"SIM max abs err: {err}")
    assert err == 0.0, "sim mismatch"
    print("SIM PASS")


# revision 19
# speedup vs baseline: 1.8847x; 1.0691x over previous
"""MoE-ALU (add with carry + xor over one-hot byte encodings) on 8 NeuronCores.

Semantics (validated against the jax reference bit-exactly): inputs a, b are
exact one-hot byte encodings [B, 4, 256] (little-endian bytes of 32-bit ints);
with SCALE=100 every softmax in the reference collapses to an exact one-hot, so

    out[0] = one_hot bytes of (a_int + b_int) mod 2^32
    out[1] = one_hot bytes of (a_int ^ b_int)

Layout: the host stores the one-hot inputs group/partition-major as fp8
([group, partition, chunk*column]; 0.0/1.0 are exact in fp8e4) so every DMA
descriptor is one contiguous 4 KiB run per partition, and the outputs as bf16
one-hots (exact 0/1). The device moves 8 MiB in + 16 MiB out per core instead
of 32+32 for f32 batch-major. All compute happens on device; the host only
reorders/recodes losslessly.

Device pipeline per 512-row batch group (8 groups per core):
  decode  TensorE: 16 accumulating matmuls (K=128 chunk each) of the fp8
          one-hot slabs against bf16 iota/256*iota weight columns produce
          PSUM [4, 512] = (a_lo16, a_hi16, b_lo16, b_hi16), exact in f32.
  stage   ScalarE copies PSUM -> SBUF f32 (frees the bank for group g+2).
  flip    TensorE transposes [4, 128] -> PSUM [128, 4] per 128-row tile.
  alu     VectorE per tile: int32 cast, halves add / xor, carry fold,
          fused shift+mask byte extract; two tiles' chains are interleaved
          so every RAW wait's producer is >=2 ops back (the DVE pipe does
          not self-interlock; adjacent RAW stalls ~230ns).
  encode  per output byte, one-hot = is_equal against an iota table with a
          per-partition scalar: 5 bytes as DVE tensor_scalar (bf16 4x perf
          mode), 2 bytes as GPSIMD tensor_tensor (broadcast scalar), 1 byte
          as ScalarE Square/Relu pair -- relu(1-(iota-idx)^2).
  store   ScalarE issues two 1 MiB output DMAs per group.

Raw Bass (one sync wait per instruction); rotating per-slot semaphores gate
buffer reuse; DVE same-engine RAW steps wait on a monotonically counted
semaphore.
"""
from contextlib import ExitStack

import numpy as np
import ml_dtypes

import concourse.bass as bass
from concourse import mybir
from concourse.bass_utils import run_bass_kernel_spmd

F32 = mybir.dt.float32
I32 = mybir.dt.int32
BF16 = mybir.dt.bfloat16
FP8 = mybir.dt.float8e4

P = 128
N_CORES = 8
B = 32768
B_LOC = B // N_CORES          # 4096 rows per core
ROW = 4 * 256                 # 1024 per row per tensor
NG = 512                      # batch rows per matmul group (one PSUM bank)
G = B_LOC // NG               # 8 groups
N_TILES = B_LOC // P          # 32 tiles of 128 rows
NCH = 16                      # K-chunks: 8 slabs (a0..a3,b0..b3) x 2 halves

NBUF = 4                      # input group-buffer slots
OBUF = 4                      # output group-buffer slots
NSUB = 2                      # input sub-DMAs per group
INFLIGHT = 2                  # concurrent group loads
WARMUP_MM = 4                 # dummy matmuls to ramp the PE clock

DVE_OPS = 7                   # s_dve increments per tile (chain ops)
GP_BYTES = ()                 # (Pool has no compare ops; GPSIMD issues stores)
ACT_BYTES = ()                # encode bytes on ScalarE (square+relu)
DVE_BYTES = tuple(e for e in range(8)
                  if e not in GP_BYTES and e not in ACT_BYTES)
PERM = [0, 4, 1, 5, 2, 6, 3, 7]  # output byte e -> idxf column


def _op1_count(t):
    """s_dve value once tile t's pt->iv copy has retired (pair interleave)."""
    return 2 * DVE_OPS * (t // 2) + 1 + (t % 2)


def _chain_count(t):
    """s_dve value once tile t's full chain (incl. idxf) has retired."""
    return 2 * DVE_OPS * (t // 2) + 2 * (DVE_OPS - 1) + 1 + (t % 2)


def _build_nc() -> bass.Bass:
    nc = bass.Bass(trn_type="TRN2")
    ab_d = nc.dram_tensor("abt", [G, P, NCH * NG], FP8, kind="ExternalInput")
    tabw_d = nc.dram_tensor("tabw", [P, NCH * 4], BF16, kind="ExternalInput")
    tabio_d = nc.dram_tensor("tabio", [P, 256], BF16, kind="ExternalInput")
    tabid_d = nc.dram_tensor("tabid", [4, 4], F32, kind="ExternalInput")
    out_d = nc.dram_tensor("out", [2, B_LOC, ROW], BF16, kind="ExternalOutput")

    with ExitStack() as ctx:
        sb = lambda name, shape, dt: ctx.enter_context(
            nc.sbuf_tensor(name, shape, dt))
        tabw_t = sb("tabw_t", [P, NCH * 4], BF16)
        tabio_t = sb("tabio_t", [P, 256], BF16)
        tabid_t = sb("tabid_t", [4, 4], F32)
        in_t = [sb(f"in_t{k}", [P, NCH * NG], FP8) for k in range(NBUF)]
        sval = [sb(f"sval{k}", [4, NG], F32) for k in range(2)]
        og = [sb(f"og{k}", [P, 4 * 2 * ROW], BF16) for k in range(OBUF)]
        # parity-double-buffered per-tile temporaries
        iv = [sb(f"iv_{p}", [P, 4], I32) for p in range(2)]
        v4 = [sb(f"v4_{p}", [P, 4], I32) for p in range(2)]
        idx8 = [sb(f"idx8_{p}", [P, 8], I32) for p in range(2)]
        idxf = [sb(f"idxf_{p}", [P, 8], F32) for p in range(2)]
        tmpa = [[sb(f"tmpa_{p}_{i}", [P, 256], F32)
                 for i in range(len(ACT_BYTES))] for p in range(2)]

        pv = [ctx.enter_context(nc.psum_tensor(f"pv{k}", [4, NG], F32))
              for k in range(2)]
        pt = [ctx.enter_context(nc.psum_tensor(f"pt{k}", [P, 16], F32))
              for k in range(2)]

        s_tabw = ctx.enter_context(nc.semaphore("s_tabw"))
        s_tabid = ctx.enter_context(nc.semaphore("s_tabid"))
        s_tabio = ctx.enter_context(nc.semaphore("s_tabio"))
        s_sub = [[ctx.enter_context(nc.semaphore(f"s_sub{j}_{u}"))
                  for u in range(NSUB)] for j in range(NBUF)]
        s_store = [ctx.enter_context(nc.semaphore(f"s_store{j}"))
                   for j in range(OBUF)]
        s_mm = ctx.enter_context(nc.semaphore("s_mm"))      # matmul groups
        s_sv = ctx.enter_context(nc.semaphore("s_sv"))      # psum->sbuf copies
        s_T = ctx.enter_context(nc.semaphore("s_T"))        # transposes done
        s_comp = ctx.enter_context(nc.semaphore("s_comp"))  # DVE-encoded tiles
        s_dve = ctx.enter_context(nc.semaphore("s_dve"))    # chain ops done
        s_gp = ctx.enter_context(nc.semaphore("s_gp"))      # GP-encoded tiles
        s_ac = ctx.enter_context(nc.semaphore("s_ac"))      # ACT-encoded tiles
        s_acq = ctx.enter_context(nc.semaphore("s_acq"))    # ACT square ops

        block = ctx.enter_context(nc.Block())

        @block.sync
        def _(sync: bass.BassEngine):
            CW = NCH * NG // NSUB   # columns per sub-DMA

            def load_group(g):
                j = g % NBUF
                for u in range(NSUB):
                    sync.dma_start(
                        out=in_t[j][:, CW * u:CW * (u + 1)],
                        in_=ab_d[g, :, CW * u:CW * (u + 1)],
                    ).then_inc(s_sub[j][u], 16)

            sync.dma_start(out=tabw_t[:], in_=tabw_d[:]).then_inc(s_tabw, 16)
            load_group(0)
            sync.dma_start(out=tabid_t[:], in_=tabid_d[:]).then_inc(
                s_tabid, 16)
            sync.dma_start(out=tabio_t[:], in_=tabio_d[:]).then_inc(
                s_tabio, 16)
            for g in range(1, G):
                if g >= INFLIGHT:
                    # bounded prefetch: group g-INFLIGHT fully landed first
                    gp_ = g - INFLIGHT
                    for u in range(NSUB):
                        sync.wait_ge(s_sub[gp_ % NBUF][u],
                                     16 * (gp_ // NBUF + 1))
                if g >= NBUF:
                    # slot reuse: matmuls of group g-NBUF consumed it
                    sync.wait_ge(s_mm, g - NBUF + 1)
                load_group(g)

        @block.tensor
        def _(tensor: bass.BassEngine):
            CS = NCH // NSUB
            tensor.wait_ge(s_tabw, 16)
            # clock-ramp warmup while the first input group is in flight
            warm_rhs = tabw_t[:, None, :].to_broadcast((P, 8, NCH * 4))
            for _w in range(WARMUP_MM):
                tensor.matmul(out=pv[1][:, :], lhsT=tabw_t[:, 0:4],
                              rhs=warm_rhs, start=True, stop=True)
            for g in range(G + 1):
                if g < G:
                    j = g % NBUF
                    if g >= 2:
                        # pv[g%2] freed once ScalarE copied group g-2
                        tensor.wait_ge(s_sv, g - 1)
                    for c in range(NCH):
                        if c % CS == 0:
                            tensor.wait_ge(s_sub[j][c // CS],
                                           16 * (g // NBUF + 1))
                        ins = tensor.matmul(
                            out=pv[g % 2][:, :],
                            lhsT=tabw_t[:, 4 * c:4 * (c + 1)],
                            rhs=in_t[j][:, NG * c:NG * (c + 1)],
                            start=(c == 0),
                            stop=(c == NCH - 1),
                        )
                        if c == NCH - 1:
                            ins.then_inc(s_mm, 1)
                q = g - 1
                if q >= 0:
                    if q == 0:
                        tensor.wait_ge(s_tabid, 16)
                    tensor.wait_ge(s_sv, q + 1)
                    if q >= 2:
                        # pt[q%2] freed once the pt->iv copy of the last
                        # tile of group q-2 retired
                        tensor.wait_ge(s_dve, _op1_count(4 * (q - 2) + 3))
                    for k in range(4):
                        tensor.transpose(
                            out=pt[q % 2][:, 4 * k:4 * (k + 1)],
                            in_=sval[q % 2][:, P * k:P * (k + 1)],
                            identity=tabid_t[:],
                        ).then_inc(s_T, 1)

        @block.scalar
        def _(scalar: bass.BassEngine):
            acq = 0
            for g in range(G + 1):
                if g < G:
                    scalar.wait_ge(s_mm, g + 1)
                    if g >= 2:
                        # sval[g%2] freed once transposes of group g-2 done
                        scalar.wait_ge(s_T, 4 * (g - 1))
                    scalar.activation(
                        out=sval[g % 2][:, :], in_=pv[g % 2][:, :],
                        func=mybir.ActivationFunctionType.Copy,
                    ).then_inc(s_sv, 1)
                # ScalarE-encoded bytes for the tiles of group g-1, two
                # tiles interleaved (ACT ops need sems for same-engine RAW)
                qe = g - 1
                if 0 <= qe < G and ACT_BYTES:
                    if qe == 0:
                        scalar.wait_ge(s_tabio, 16)
                    joq = qe % OBUF
                    if qe >= OBUF:
                        scalar.wait_ge(s_store[joq], 32 * (qe // OBUF))
                    for kp in range(2):
                        t0 = 4 * qe + 2 * kp
                        m = t0 // 2
                        if m >= 1:
                            # tmpa WAR: previous pair's Relus retired
                            scalar.wait_ge(
                                s_ac, 2 * len(ACT_BYTES) * m)
                        scalar.wait_ge(s_dve, _chain_count(t0 + 1))
                        for pr in (0, 1):
                            for i, e in enumerate(ACT_BYTES):
                                scalar.activation(
                                    out=tmpa[pr][i][:], in_=tabio_t[:],
                                    func=mybir.ActivationFunctionType.Square,
                                    bias=idxf[pr][:, PERM[e]:PERM[e] + 1],
                                    scale=-1.0,
                                ).then_inc(s_acq, 1)
                                acq += 1
                        scalar.wait_ge(s_acq, acq)
                        for pr in (0, 1):
                            k = 2 * kp + pr
                            for i, e in enumerate(ACT_BYTES):
                                scalar.activation(
                                    out=og[joq][:, 2048 * k + 256 * e:
                                                2048 * k + 256 * (e + 1)],
                                    in_=tmpa[pr][i][:],
                                    func=mybir.ActivationFunctionType.Relu,
                                    bias=1.0, scale=-1.0,
                                ).then_inc(s_ac, 1)

        @block.vector
        def _(vector: bass.BassEngine):
            n = 0  # statically tracked s_dve count

            def chain_op(ins):
                nonlocal n
                ins.then_inc(s_dve, 1)
                n += 1

            for m in range(N_TILES // 2):
                t0 = 2 * m
                q = t0 // 4
                jo = q % OBUF
                ks = (t0 % 4, t0 % 4 + 1)
                if m == 0:
                    vector.wait_ge(s_tabio, 16)
                vector.wait_ge(s_T, t0 + 2)
                if ACT_BYTES and m >= 1:
                    # idxf[pr] reuse: ScalarE read pair m-1 (squares done)
                    vector.wait_ge(s_acq, 2 * len(ACT_BYTES) * m)
                if ks[0] == 0 and q >= OBUF:
                    vector.wait_ge(s_store[jo], 32 * (q // OBUF))
                # interleaved chains: each wait's producers are >=2 ops back
                for pr in (0, 1):
                    chain_op(vector.tensor_copy(
                        iv[pr][:], pt[q % 2][:, 4 * ks[pr]:4 * ks[pr] + 4]))
                vector.wait_ge(s_dve, n)
                for pr in (0, 1):
                    # v4 = [s_lo_raw(17b), s_hi_raw, x_lo, x_hi]
                    chain_op(vector.tensor_tensor(
                        out=v4[pr][:, 0:2], in0=iv[pr][:, 0:2],
                        in1=iv[pr][:, 2:4], op=mybir.AluOpType.add))
                    chain_op(vector.tensor_tensor(
                        out=v4[pr][:, 2:4], in0=iv[pr][:, 0:2],
                        in1=iv[pr][:, 2:4], op=mybir.AluOpType.bitwise_xor))
                vector.wait_ge(s_dve, n)
                for pr in (0, 1):
                    # fold the 2^16 carry into s_hi (s_lo_raw keeps bit 16;
                    # the &255 byte masks strip it later)
                    chain_op(vector.scalar_tensor_tensor(
                        out=v4[pr][:, 1:2], in0=v4[pr][:, 0:1], scalar=65536,
                        in1=v4[pr][:, 1:2],
                        op0=mybir.AluOpType.is_ge, op1=mybir.AluOpType.add))
                vector.wait_ge(s_dve, n)
                for pr in (0, 1):
                    # byte extract (fused shift+mask); idx8 holds the bytes
                    # in [s0 s2 x0 x2 | s1 s3 x1 x3] order
                    chain_op(vector.tensor_scalar(
                        out=idx8[pr][:, 0:4], in0=v4[pr][:], scalar1=255,
                        scalar2=None, op0=mybir.AluOpType.bitwise_and))
                    chain_op(vector.tensor_scalar(
                        out=idx8[pr][:, 4:8], in0=v4[pr][:], scalar1=8,
                        scalar2=255,
                        op0=mybir.AluOpType.logical_shift_right,
                        op1=mybir.AluOpType.bitwise_and))
                vector.wait_ge(s_dve, n)
                for pr in (0, 1):
                    chain_op(vector.tensor_copy(idxf[pr][:], idx8[pr][:]))
                vector.wait_ge(s_dve, n)
                # encode: single-src is_equal against the iota table, one op
                # per output byte, per-partition scalar = that byte's value
                for pr in (0, 1):
                    for i, e in enumerate(DVE_BYTES):
                        ins = vector.tensor_scalar(
                            out=og[jo][:, 2048 * ks[pr] + 256 * e:
                                       2048 * ks[pr] + 256 * (e + 1)],
                            in0=tabio_t[:],
                            scalar1=idxf[pr][:, PERM[e]:PERM[e] + 1],
                            scalar2=None,
                            op0=mybir.AluOpType.is_equal,
                        )
                        if i == len(DVE_BYTES) - 1:
                            ins.then_inc(s_comp, 1)

        @block.gpsimd
        def _(gp: bass.BassEngine):
            for qs in range(G):
                jo = qs % OBUF
                r0 = qs * NG
                src = og[jo][:].rearrange("p (t two r) -> p t two r",
                                          t=4, two=2)
                dst0 = out_d[0, r0:r0 + NG, :].rearrange(
                    "(t p) r -> p t r", p=P)
                dst1 = out_d[1, r0:r0 + NG, :].rearrange(
                    "(t p) r -> p t r", p=P)
                gp.wait_ge(s_comp, 4 * (qs + 1))
                if ACT_BYTES:
                    gp.wait_ge(s_ac, 8 * len(ACT_BYTES) * (qs + 1) // 2)
                gp.dma_start(out=dst0, in_=src[:, :, 0, :]).then_inc(
                    s_store[jo], 16)
                gp.dma_start(out=dst1, in_=src[:, :, 1, :]).then_inc(
                    s_store[jo], 16)

    return nc


def _make_tables():
    pos = np.arange(P, dtype=np.float64)
    w = np.zeros((NCH, P, 4), np.float64)
    for s in range(8):
        col = s // 2 if s < 4 else 2 + (s - 4) // 2
        mul = 1.0 if (s % 2 == 0) else 256.0
        for h in range(2):
            c = 2 * s + h
            w[c, :, col] = (pos + 128.0 * h) * mul
    tabw = w.transpose(1, 0, 2).reshape(P, NCH * 4).astype(ml_dtypes.bfloat16)
    tabio = np.tile(np.arange(256).astype(ml_dtypes.bfloat16)[None, :],
                    (P, 1))
    tabid = np.eye(4, dtype=np.float32)
    return tabw, tabio, tabid


def _pack_core(abt, lo):
    """[NCH, P, B] fp8 slab-chunks -> core block [G, P, NCH*NG]."""
    blk = abt[:, :, lo:lo + B_LOC].reshape(NCH, P, G, NG)
    return np.ascontiguousarray(
        blk.transpose(2, 1, 0, 3).reshape(G, P, NCH * NG))


_NC_CACHE = {}


def _get_nc(variant: str = "main"):
    if variant not in _NC_CACHE:
        _NC_CACHE[variant] = _build_nc()
    return _NC_CACHE[variant]


def _run(a: np.ndarray, b: np.ndarray, **spmd_kwargs):
    assert a.shape == (B, 4, 256) and b.shape == (B, 4, 256)
    a_t = np.ascontiguousarray(
        np.asarray(a, np.float32).reshape(B, 4, 256).transpose(1, 2, 0)
    ).astype(ml_dtypes.float8_e4m3)
    b_t = np.ascontiguousarray(
        np.asarray(b, np.float32).reshape(B, 4, 256).transpose(1, 2, 0)
    ).astype(ml_dtypes.float8_e4m3)
    abt = np.concatenate([a_t.reshape(NCH // 2, P, B),
                          b_t.reshape(NCH // 2, P, B)], axis=0)
    tabw, tabio, tabid = _make_tables()
    in_maps = [
        {
            "abt": _pack_core(abt, i * B_LOC),
            "tabw": tabw,
            "tabio": tabio,
            "tabid": tabid,
        }
        for i in range(N_CORES)
    ]
    nc = _get_nc()
    kr = run_bass_kernel_spmd(nc, in_maps, list(range(N_CORES)), **spmd_kwargs)
    shards = [kr.results[i]["out"] for i in range(N_CORES)]
    out = np.concatenate(shards, axis=1).astype(np.float32)
    return out.reshape(2, B, 4, 256), kr


def kernel(a: np.ndarray, b: np.ndarray) -> np.ndarray:
    out, _ = _run(a, b)
    return out


def run_sim():
    """CoreSim one core vs numpy oracle (invoked by test.py --sim)."""
    from concourse.bass_interp import CoreSim

    rng = np.random.default_rng(1)
    Bl = B_LOC
    ai = rng.integers(0, 256, (Bl, 4))
    bi = rng.integers(0, 256, (Bl, 4))
    ai[0] = [255] * 4
    bi[0] = [255] * 4
    ai[1] = [255, 255, 255, 255]
    bi[1] = [1, 0, 0, 0]
    a = np.zeros((Bl, 4, 256), np.float32)
    b = np.zeros((Bl, 4, 256), np.float32)
    r = np.arange(Bl)[:, None]
    j = np.arange(4)[None, :]
    a[r, j, ai] = 1.0
    b[r, j, bi] = 1.0

    a_t = np.ascontiguousarray(a.transpose(1, 2, 0)).astype(
        ml_dtypes.float8_e4m3)
    b_t = np.ascontiguousarray(b.transpose(1, 2, 0)).astype(
        ml_dtypes.float8_e4m3)
    abt = np.concatenate([a_t.reshape(NCH // 2, P, Bl),
                          b_t.reshape(NCH // 2, P, Bl)], axis=0)
    tabw, tabio, tabid = _make_tables()

    nc = _get_nc()
    sim = CoreSim(nc)
    sim.tensor("abt")[:] = _pack_core(abt, 0)
    sim.tensor("tabw")[:] = tabw
    sim.tensor("tabio")[:] = tabio
    sim.tensor("tabid")[:] = tabid
    sim.simulate()
    out = np.array(sim.tensor("out")).astype(np.float32).reshape(2, Bl, 4, 256)

    # numpy oracle
    pw = (256 ** np.arange(4)).astype(np.int64)
    a32 = (ai * pw).sum(-1)
    b32 = (bi * pw).sum(-1)
    s32 = (a32 + b32) % (2 ** 32)
    x32 = a32 ^ b32
    sb_ = np.stack([(s32 >> (8 * i)) & 255 for i in range(4)], -1)
    xb_ = np.stack([(x32 >> (8 * i)) & 255 for i in range(4)], -1)
    exp = np.zeros((2, Bl, 4, 256), np.float32)
    exp[0, r, j, sb_] = 1.0
    exp[1, r, j, xb_] = 1.0
    err = np.abs(out - exp).max()
    print(f"SIM max abs err: {err}")
    assert err == 0.0, "sim mismatch"
    print("SIM PASS")


# revision 20
# speedup vs baseline: 1.9400x; 1.0293x over previous
"""MoE-ALU (add with carry + xor over one-hot byte encodings) on 8 NeuronCores.

Semantics (validated against the jax reference bit-exactly): inputs a, b are
exact one-hot byte encodings [B, 4, 256] (little-endian bytes of 32-bit ints);
with SCALE=100 every softmax in the reference collapses to an exact one-hot, so

    out[0] = one_hot bytes of (a_int + b_int) mod 2^32
    out[1] = one_hot bytes of (a_int ^ b_int)

Layout: the host stores the one-hot inputs group/partition-major as fp8
([group, partition, chunk*column]; 0.0/1.0 are exact in fp8e4) so every DMA
descriptor is one contiguous 4 KiB run per partition, and the outputs as bf16
one-hots (exact 0/1). The device moves 8 MiB in + 16 MiB out per core instead
of 32+32 for f32 batch-major. All compute happens on device; the host only
reorders/recodes losslessly.

Device pipeline per 512-row batch group (8 groups per core):
  decode  TensorE: 16 accumulating matmuls (K=128 chunk each) of the fp8
          one-hot slabs against bf16 iota/256*iota weight columns produce
          PSUM [4, 512] = (a_lo16, a_hi16, b_lo16, b_hi16), exact in f32.
  stage   ScalarE copies PSUM -> SBUF f32 (frees the bank for group g+2).
  flip    TensorE transposes [4, 128] -> PSUM [128, 4] per 128-row tile.
  alu     VectorE per tile: int32 cast, halves add / xor, carry fold,
          fused shift+mask byte extract; two tiles' chains are interleaved
          so every RAW wait's producer is >=2 ops back (the DVE pipe does
          not self-interlock; adjacent RAW stalls ~230ns).
  encode  per output byte, one-hot = is_equal against an iota table with a
          per-partition scalar: 5 bytes as DVE tensor_scalar (bf16 4x perf
          mode), 2 bytes as GPSIMD tensor_tensor (broadcast scalar), 1 byte
          as ScalarE Square/Relu pair -- relu(1-(iota-idx)^2).
  store   ScalarE issues two 1 MiB output DMAs per group.

Raw Bass (one sync wait per instruction); rotating per-slot semaphores gate
buffer reuse; DVE same-engine RAW steps wait on a monotonically counted
semaphore.
"""
from contextlib import ExitStack

import numpy as np
import ml_dtypes

import concourse.bass as bass
from concourse import mybir
from concourse.bass_utils import run_bass_kernel_spmd

F32 = mybir.dt.float32
I32 = mybir.dt.int32
BF16 = mybir.dt.bfloat16
FP8 = mybir.dt.float8e4

P = 128
N_CORES = 8
B = 32768
B_LOC = B // N_CORES          # 4096 rows per core
ROW = 4 * 256                 # 1024 per row per tensor
NG = 512                      # batch rows per matmul group (one PSUM bank)
G = B_LOC // NG               # 8 groups
N_TILES = B_LOC // P          # 32 tiles of 128 rows
NCH = 16                      # K-chunks: 8 slabs (a0..a3,b0..b3) x 2 halves

NBUF = 4                      # input group-buffer slots
OBUF = 4                      # output group-buffer slots
NSUB = 4                      # input sub-DMAs per group
INFLIGHT = 2                  # concurrent group loads
WARMUP_MM = 8                 # dummy matmuls to ramp the PE clock

DVE_OPS = 7                   # s_dve increments per tile (chain ops)
GP_BYTES = ()                 # (Pool has no compare ops; GPSIMD issues stores)
ACT_BYTES = ()                # encode bytes on ScalarE (square+relu)
DVE_BYTES = tuple(e for e in range(8)
                  if e not in GP_BYTES and e not in ACT_BYTES)
PERM = [0, 4, 1, 5, 2, 6, 3, 7]  # output byte e -> idxf column


def _op1_count(t):
    """s_dve value once tile t's pt->iv copy has retired (pair interleave)."""
    return 2 * DVE_OPS * (t // 2) + 1 + (t % 2)


def _chain_count(t):
    """s_dve value once tile t's full chain (incl. idxf) has retired."""
    return 2 * DVE_OPS * (t // 2) + 2 * (DVE_OPS - 1) + 1 + (t % 2)


def _build_nc() -> bass.Bass:
    nc = bass.Bass(trn_type="TRN2")
    ab_d = nc.dram_tensor("abt", [G, P, NCH * NG], FP8, kind="ExternalInput")
    tabw_d = nc.dram_tensor("tabw", [P, NCH * 4], BF16, kind="ExternalInput")
    tabio_d = nc.dram_tensor("tabio", [P, 256], BF16, kind="ExternalInput")
    tabid_d = nc.dram_tensor("tabid", [4, 4], F32, kind="ExternalInput")
    out_d = nc.dram_tensor("out", [2, B_LOC, ROW], BF16, kind="ExternalOutput")

    with ExitStack() as ctx:
        sb = lambda name, shape, dt: ctx.enter_context(
            nc.sbuf_tensor(name, shape, dt))
        tabw_t = sb("tabw_t", [P, NCH * 4], BF16)
        tabio_t = sb("tabio_t", [P, 256], BF16)
        tabid_t = sb("tabid_t", [4, 4], F32)
        in_t = [sb(f"in_t{k}", [P, NCH * NG], FP8) for k in range(NBUF)]
        sval = [sb(f"sval{k}", [4, NG], F32) for k in range(2)]
        og = [sb(f"og{k}", [P, 4 * 2 * ROW], BF16) for k in range(OBUF)]
        # parity-double-buffered per-tile temporaries
        iv = [sb(f"iv_{p}", [P, 4], I32) for p in range(2)]
        v4 = [sb(f"v4_{p}", [P, 4], I32) for p in range(2)]
        idx8 = [sb(f"idx8_{p}", [P, 8], I32) for p in range(2)]
        idxf = [sb(f"idxf_{p}", [P, 8], F32) for p in range(2)]
        tmpa = [[sb(f"tmpa_{p}_{i}", [P, 256], F32)
                 for i in range(len(ACT_BYTES))] for p in range(2)]

        pv = [ctx.enter_context(nc.psum_tensor(f"pv{k}", [4, NG], F32))
              for k in range(2)]
        pt = [ctx.enter_context(nc.psum_tensor(f"pt{k}", [P, 16], F32))
              for k in range(2)]

        s_tabw = ctx.enter_context(nc.semaphore("s_tabw"))
        s_tabid = ctx.enter_context(nc.semaphore("s_tabid"))
        s_tabio = ctx.enter_context(nc.semaphore("s_tabio"))
        s_sub = [[ctx.enter_context(nc.semaphore(f"s_sub{j}_{u}"))
                  for u in range(NSUB)] for j in range(NBUF)]
        s_store = [ctx.enter_context(nc.semaphore(f"s_store{j}"))
                   for j in range(OBUF)]
        s_mm = ctx.enter_context(nc.semaphore("s_mm"))      # matmul groups
        s_sv = ctx.enter_context(nc.semaphore("s_sv"))      # psum->sbuf copies
        s_T = ctx.enter_context(nc.semaphore("s_T"))        # transposes done
        s_comp = ctx.enter_context(nc.semaphore("s_comp"))  # DVE-encoded tiles
        s_dve = ctx.enter_context(nc.semaphore("s_dve"))    # chain ops done
        s_ac = ctx.enter_context(nc.semaphore("s_ac"))      # ACT-encoded tiles
        s_acq = ctx.enter_context(nc.semaphore("s_acq"))    # ACT square ops

        block = ctx.enter_context(nc.Block())

        @block.sync
        def _(sync: bass.BassEngine):
            CW = NCH * NG // NSUB   # columns per sub-DMA

            def load_group(g):
                j = g % NBUF
                for u in range(NSUB):
                    sync.dma_start(
                        out=in_t[j][:, CW * u:CW * (u + 1)],
                        in_=ab_d[g, :, CW * u:CW * (u + 1)],
                    ).then_inc(s_sub[j][u], 16)

            sync.dma_start(out=tabw_t[:], in_=tabw_d[:]).then_inc(s_tabw, 16)
            load_group(0)
            sync.dma_start(out=tabid_t[:], in_=tabid_d[:]).then_inc(
                s_tabid, 16)
            sync.dma_start(out=tabio_t[:], in_=tabio_d[:]).then_inc(
                s_tabio, 16)
            for g in range(1, G):
                if g >= INFLIGHT:
                    # bounded prefetch: group g-INFLIGHT fully landed first
                    gp_ = g - INFLIGHT
                    for u in range(NSUB):
                        sync.wait_ge(s_sub[gp_ % NBUF][u],
                                     16 * (gp_ // NBUF + 1))
                if g >= NBUF:
                    # slot reuse: matmuls of group g-NBUF consumed it
                    sync.wait_ge(s_mm, g - NBUF + 1)
                load_group(g)

        @block.tensor
        def _(tensor: bass.BassEngine):
            CS = NCH // NSUB
            tensor.wait_ge(s_tabw, 16)
            # clock-ramp warmup while the first input group is in flight
            warm_rhs = tabw_t[:, None, :].to_broadcast((P, 8, NCH * 4))
            for _w in range(WARMUP_MM):
                tensor.matmul(out=pv[1][:, :], lhsT=tabw_t[:, 0:4],
                              rhs=warm_rhs, start=True, stop=True)
            for g in range(G + 1):
                if g < G:
                    j = g % NBUF
                    if g >= 2:
                        # pv[g%2] freed once ScalarE copied group g-2
                        tensor.wait_ge(s_sv, g - 1)
                    for c in range(NCH):
                        if c % CS == 0:
                            tensor.wait_ge(s_sub[j][c // CS],
                                           16 * (g // NBUF + 1))
                        ins = tensor.matmul(
                            out=pv[g % 2][:, :],
                            lhsT=tabw_t[:, 4 * c:4 * (c + 1)],
                            rhs=in_t[j][:, NG * c:NG * (c + 1)],
                            start=(c == 0),
                            stop=(c == NCH - 1),
                        )
                        if c == NCH - 1:
                            ins.then_inc(s_mm, 1)
                q = g - 1
                if q >= 0:
                    if q == 0:
                        tensor.wait_ge(s_tabid, 16)
                    tensor.wait_ge(s_sv, q + 1)
                    if q >= 2:
                        # pt[q%2] freed once the pt->iv copy of the last
                        # tile of group q-2 retired
                        tensor.wait_ge(s_dve, _op1_count(4 * (q - 2) + 3))
                    for k in range(4):
                        tensor.transpose(
                            out=pt[q % 2][:, 4 * k:4 * (k + 1)],
                            in_=sval[q % 2][:, P * k:P * (k + 1)],
                            identity=tabid_t[:],
                        ).then_inc(s_T, 1)

        @block.scalar
        def _(scalar: bass.BassEngine):
            acq = 0
            for g in range(G + 1):
                if g < G:
                    scalar.wait_ge(s_mm, g + 1)
                    if g >= 2:
                        # sval[g%2] freed once transposes of group g-2 done
                        scalar.wait_ge(s_T, 4 * (g - 1))
                    scalar.activation(
                        out=sval[g % 2][:, :], in_=pv[g % 2][:, :],
                        func=mybir.ActivationFunctionType.Copy,
                    ).then_inc(s_sv, 1)
                # ScalarE-encoded bytes for the tiles of group g-1, two
                # tiles interleaved (ACT ops need sems for same-engine RAW)
                qe = g - 1
                if 0 <= qe < G and ACT_BYTES:
                    if qe == 0:
                        scalar.wait_ge(s_tabio, 16)
                    joq = qe % OBUF
                    if qe >= OBUF:
                        scalar.wait_ge(s_store[joq], 32 * (qe // OBUF))
                    for kp in range(2):
                        t0 = 4 * qe + 2 * kp
                        m = t0 // 2
                        if m >= 1:
                            # tmpa WAR: previous pair's Relus retired
                            scalar.wait_ge(
                                s_ac, 2 * len(ACT_BYTES) * m)
                        scalar.wait_ge(s_dve, _chain_count(t0 + 1))
                        for pr in (0, 1):
                            for i, e in enumerate(ACT_BYTES):
                                scalar.activation(
                                    out=tmpa[pr][i][:], in_=tabio_t[:],
                                    func=mybir.ActivationFunctionType.Square,
                                    bias=idxf[pr][:, PERM[e]:PERM[e] + 1],
                                    scale=-1.0,
                                ).then_inc(s_acq, 1)
                                acq += 1
                        scalar.wait_ge(s_acq, acq)
                        for pr in (0, 1):
                            k = 2 * kp + pr
                            for i, e in enumerate(ACT_BYTES):
                                scalar.activation(
                                    out=og[joq][:, 2048 * k + 256 * e:
                                                2048 * k + 256 * (e + 1)],
                                    in_=tmpa[pr][i][:],
                                    func=mybir.ActivationFunctionType.Relu,
                                    bias=1.0, scale=-1.0,
                                ).then_inc(s_ac, 1)

        @block.vector
        def _(vector: bass.BassEngine):
            n = 0  # statically tracked s_dve count

            def chain_op(ins):
                nonlocal n
                ins.then_inc(s_dve, 1)
                n += 1

            for m in range(N_TILES // 2):
                t0 = 2 * m
                q = t0 // 4
                jo = q % OBUF
                ks = (t0 % 4, t0 % 4 + 1)
                if m == 0:
                    vector.wait_ge(s_tabio, 16)
                vector.wait_ge(s_T, t0 + 2)
                if ACT_BYTES and m >= 1:
                    # idxf[pr] reuse: ScalarE read pair m-1 (squares done)
                    vector.wait_ge(s_acq, 2 * len(ACT_BYTES) * m)
                if ks[0] == 0 and q >= OBUF:
                    vector.wait_ge(s_store[jo], 32 * (q // OBUF))
                # interleaved chains: each wait's producers are >=2 ops back
                for pr in (0, 1):
                    chain_op(vector.tensor_copy(
                        iv[pr][:], pt[q % 2][:, 4 * ks[pr]:4 * ks[pr] + 4]))
                vector.wait_ge(s_dve, n)
                for pr in (0, 1):
                    # v4 = [s_lo_raw(17b), s_hi_raw, x_lo, x_hi]
                    chain_op(vector.tensor_tensor(
                        out=v4[pr][:, 0:2], in0=iv[pr][:, 0:2],
                        in1=iv[pr][:, 2:4], op=mybir.AluOpType.add))
                    chain_op(vector.tensor_tensor(
                        out=v4[pr][:, 2:4], in0=iv[pr][:, 0:2],
                        in1=iv[pr][:, 2:4], op=mybir.AluOpType.bitwise_xor))
                vector.wait_ge(s_dve, n)
                for pr in (0, 1):
                    # fold the 2^16 carry into s_hi (s_lo_raw keeps bit 16;
                    # the &255 byte masks strip it later)
                    chain_op(vector.scalar_tensor_tensor(
                        out=v4[pr][:, 1:2], in0=v4[pr][:, 0:1], scalar=65536,
                        in1=v4[pr][:, 1:2],
                        op0=mybir.AluOpType.is_ge, op1=mybir.AluOpType.add))
                vector.wait_ge(s_dve, n)
                for pr in (0, 1):
                    # byte extract (fused shift+mask); idx8 holds the bytes
                    # in [s0 s2 x0 x2 | s1 s3 x1 x3] order
                    chain_op(vector.tensor_scalar(
                        out=idx8[pr][:, 0:4], in0=v4[pr][:], scalar1=255,
                        scalar2=None, op0=mybir.AluOpType.bitwise_and))
                    chain_op(vector.tensor_scalar(
                        out=idx8[pr][:, 4:8], in0=v4[pr][:], scalar1=8,
                        scalar2=255,
                        op0=mybir.AluOpType.logical_shift_right,
                        op1=mybir.AluOpType.bitwise_and))
                vector.wait_ge(s_dve, n)
                for pr in (0, 1):
                    chain_op(vector.tensor_copy(idxf[pr][:], idx8[pr][:]))
                vector.wait_ge(s_dve, n)
                # encode: single-src is_equal against the iota table, one op
                # per output byte, per-partition scalar = that byte's value
                for pr in (0, 1):
                    for i, e in enumerate(DVE_BYTES):
                        ins = vector.tensor_scalar(
                            out=og[jo][:, 2048 * ks[pr] + 256 * e:
                                       2048 * ks[pr] + 256 * (e + 1)],
                            in0=tabio_t[:],
                            scalar1=idxf[pr][:, PERM[e]:PERM[e] + 1],
                            scalar2=None,
                            op0=mybir.AluOpType.is_equal,
                        )
                        if i == len(DVE_BYTES) - 1:
                            ins.then_inc(s_comp, 1)

        @block.gpsimd
        def _(gp: bass.BassEngine):
            for qs in range(G):
                jo = qs % OBUF
                r0 = qs * NG
                src = og[jo][:].rearrange("p (t two r) -> p t two r",
                                          t=4, two=2)
                dst0 = out_d[0, r0:r0 + NG, :].rearrange(
                    "(t p) r -> p t r", p=P)
                dst1 = out_d[1, r0:r0 + NG, :].rearrange(
                    "(t p) r -> p t r", p=P)
                gp.wait_ge(s_comp, 4 * (qs + 1))
                if ACT_BYTES:
                    gp.wait_ge(s_ac, 8 * len(ACT_BYTES) * (qs + 1) // 2)
                gp.dma_start(out=dst0, in_=src[:, :, 0, :]).then_inc(
                    s_store[jo], 16)
                gp.dma_start(out=dst1, in_=src[:, :, 1, :]).then_inc(
                    s_store[jo], 16)

    return nc


def _make_tables():
    pos = np.arange(P, dtype=np.float64)
    w = np.zeros((NCH, P, 4), np.float64)
    for s in range(8):
        col = s // 2 if s < 4 else 2 + (s - 4) // 2
        mul = 1.0 if (s % 2 == 0) else 256.0
        for h in range(2):
            c = 2 * s + h
            w[c, :, col] = (pos + 128.0 * h) * mul
    tabw = w.transpose(1, 0, 2).reshape(P, NCH * 4).astype(ml_dtypes.bfloat16)
    tabio = np.tile(np.arange(256).astype(ml_dtypes.bfloat16)[None, :],
                    (P, 1))
    tabid = np.eye(4, dtype=np.float32)
    return tabw, tabio, tabid


def _pack_core(abt, lo):
    """[NCH, P, B] fp8 slab-chunks -> core block [G, P, NCH*NG]."""
    blk = abt[:, :, lo:lo + B_LOC].reshape(NCH, P, G, NG)
    return np.ascontiguousarray(
        blk.transpose(2, 1, 0, 3).reshape(G, P, NCH * NG))


_NC_CACHE = {}


def _get_nc(variant: str = "main"):
    if variant not in _NC_CACHE:
        _NC_CACHE[variant] = _build_nc()
    return _NC_CACHE[variant]


def _run(a: np.ndarray, b: np.ndarray, **spmd_kwargs):
    assert a.shape == (B, 4, 256) and b.shape == (B, 4, 256)
    a_t = np.ascontiguousarray(
        np.asarray(a, np.float32).reshape(B, 4, 256).transpose(1, 2, 0)
    ).astype(ml_dtypes.float8_e4m3)
    b_t = np.ascontiguousarray(
        np.asarray(b, np.float32).reshape(B, 4, 256).transpose(1, 2, 0)
    ).astype(ml_dtypes.float8_e4m3)
    abt = np.concatenate([a_t.reshape(NCH // 2, P, B),
                          b_t.reshape(NCH // 2, P, B)], axis=0)
    tabw, tabio, tabid = _make_tables()
    in_maps = [
        {
            "abt": _pack_core(abt, i * B_LOC),
            "tabw": tabw,
            "tabio": tabio,
            "tabid": tabid,
        }
        for i in range(N_CORES)
    ]
    nc = _get_nc()
    kr = run_bass_kernel_spmd(nc, in_maps, list(range(N_CORES)), **spmd_kwargs)
    shards = [kr.results[i]["out"] for i in range(N_CORES)]
    out = np.concatenate(shards, axis=1).astype(np.float32)
    return out.reshape(2, B, 4, 256), kr


def kernel(a: np.ndarray, b: np.ndarray) -> np.ndarray:
    out, _ = _run(a, b)
    return out


def run_sim():
    """CoreSim one core vs numpy oracle (invoked by test.py --sim)."""
    from concourse.bass_interp import CoreSim

    rng = np.random.default_rng(1)
    Bl = B_LOC
    ai = rng.integers(0, 256, (Bl, 4))
    bi = rng.integers(0, 256, (Bl, 4))
    ai[0] = [255] * 4
    bi[0] = [255] * 4
    ai[1] = [255, 255, 255, 255]
    bi[1] = [1, 0, 0, 0]
    a = np.zeros((Bl, 4, 256), np.float32)
    b = np.zeros((Bl, 4, 256), np.float32)
    r = np.arange(Bl)[:, None]
    j = np.arange(4)[None, :]
    a[r, j, ai] = 1.0
    b[r, j, bi] = 1.0

    a_t = np.ascontiguousarray(a.transpose(1, 2, 0)).astype(
        ml_dtypes.float8_e4m3)
    b_t = np.ascontiguousarray(b.transpose(1, 2, 0)).astype(
        ml_dtypes.float8_e4m3)
    abt = np.concatenate([a_t.reshape(NCH // 2, P, Bl),
                          b_t.reshape(NCH // 2, P, Bl)], axis=0)
    tabw, tabio, tabid = _make_tables()

    nc = _get_nc()
    sim = CoreSim(nc)
    sim.tensor("abt")[:] = _pack_core(abt, 0)
    sim.tensor("tabw")[:] = tabw
    sim.tensor("tabio")[:] = tabio
    sim.tensor("tabid")[:] = tabid
    sim.simulate()
    out = np.array(sim.tensor("out")).astype(np.float32).reshape(2, Bl, 4, 256)

    # numpy oracle
    pw = (256 ** np.arange(4)).astype(np.int64)
    a32 = (ai * pw).sum(-1)
    b32 = (bi * pw).sum(-1)
    s32 = (a32 + b32) % (2 ** 32)
    x32 = a32 ^ b32
    sb_ = np.stack([(s32 >> (8 * i)) & 255 for i in range(4)], -1)
    xb_ = np.stack([(x32 >> (8 * i)) & 255 for i in range(4)], -1)
    exp = np.zeros((2, Bl, 4, 256), np.float32)
    exp[0, r, j, sb_] = 1.0
    exp[1, r, j, xb_] = 1.0
    err = np.abs(out - exp).max()
    print(f"SIM max abs err: {err}")
    assert err == 0.0, "sim mismatch"
    print("SIM PASS")


# revision 21
# speedup vs baseline: 2.1586x; 1.1127x over previous
"""MoE-ALU (add with carry + xor over one-hot byte encodings) on 8 NeuronCores.

Semantics (validated against the jax reference bit-exactly): inputs a, b are
exact one-hot byte encodings [B, 4, 256] (little-endian bytes of 32-bit ints);
with SCALE=100 every softmax in the reference collapses to an exact one-hot, so

    out[0] = one_hot bytes of (a_int + b_int) mod 2^32
    out[1] = one_hot bytes of (a_int ^ b_int)

Layout: the host stores the one-hot inputs group/partition-major as fp8
([group, partition, chunk*column]; 0.0/1.0 are exact in fp8e4) so every DMA
descriptor is one contiguous 4 KiB run per partition, and the outputs as bf16
one-hots (exact 0/1). The device moves 8 MiB in + 16 MiB out per core instead
of 32+32 for f32 batch-major. All compute happens on device; the host only
reorders/recodes losslessly.

Device pipeline per 512-row batch group (8 groups per core):
  decode  TensorE: 16 accumulating matmuls (K=128 chunk each) of the fp8
          one-hot slabs against bf16 iota/256*iota weight columns produce
          PSUM [4, 512] = (a_lo16, a_hi16, b_lo16, b_hi16), exact in f32.
  stage   ScalarE copies PSUM -> SBUF f32 (frees the bank for group g+2).
  flip    TensorE transposes [4, 128] -> PSUM [128, 4] per 128-row tile.
  alu     VectorE per tile: int32 cast, halves add / xor, carry fold,
          fused shift+mask byte extract; two tiles' chains are interleaved
          so every RAW wait's producer is >=2 ops back (the DVE pipe does
          not self-interlock; adjacent RAW stalls ~230ns).
  encode  per output byte, one-hot = is_equal against an iota table with a
          per-partition scalar: 5 bytes as DVE tensor_scalar (bf16 4x perf
          mode), 2 bytes as GPSIMD tensor_tensor (broadcast scalar), 1 byte
          as ScalarE Square/Relu pair -- relu(1-(iota-idx)^2).
  store   ScalarE issues two 1 MiB output DMAs per group.

Raw Bass (one sync wait per instruction); rotating per-slot semaphores gate
buffer reuse; DVE same-engine RAW steps wait on a monotonically counted
semaphore.
"""
from contextlib import ExitStack

import numpy as np
import ml_dtypes

import concourse.bass as bass
from concourse import mybir
from concourse.bass_utils import run_bass_kernel_spmd

F32 = mybir.dt.float32
I32 = mybir.dt.int32
BF16 = mybir.dt.bfloat16
FP8 = mybir.dt.float8e4

P = 128
N_CORES = 8
B = 32768
B_LOC = B // N_CORES          # 4096 rows per core
ROW = 4 * 256                 # 1024 per row per tensor
NG = 512                      # batch rows per matmul group (one PSUM bank)
G = B_LOC // NG               # 8 groups
N_TILES = B_LOC // P          # 32 tiles of 128 rows
NCH = 16                      # K-chunks: 8 slabs (a0..a3,b0..b3) x 2 halves

NBUF = 4                      # input group-buffer slots
OBUF = 4                      # output group-buffer slots
NSUB = 4                      # input sub-DMAs per group
INFLIGHT = 2                  # concurrent group loads
WARMUP_MM = 8                 # dummy matmuls to ramp the PE clock

DVE_OPS = 7                   # s_dve increments per tile (chain ops)
GP_BYTES = ()                 # (Pool has no compare ops; GPSIMD issues stores)
ACT_BYTES = ()                # encode bytes on ScalarE (square+relu)
DVE_BYTES = tuple(e for e in range(8)
                  if e not in GP_BYTES and e not in ACT_BYTES)
PERM = [0, 4, 1, 5, 2, 6, 3, 7]  # output byte e -> idxf column


def _op1_count(t):
    """s_dve value once tile t's pt->iv copy has retired (quad interleave)."""
    return 4 * DVE_OPS * (t // 4) + 1 + (t % 4)


def _chain_count(t):
    """s_dve value once tile t's full chain (incl. idxf) has retired."""
    return 4 * DVE_OPS * (t // 4) + 4 * (DVE_OPS - 1) + 1 + (t % 4)


def _build_nc() -> bass.Bass:
    nc = bass.Bass(trn_type="TRN2")
    ab_d = nc.dram_tensor("abt", [G, P, NCH * NG], FP8, kind="ExternalInput")
    tabw_d = nc.dram_tensor("tabw", [P, NCH * 4], BF16, kind="ExternalInput")
    tabio_d = nc.dram_tensor("tabio", [P, 256], BF16, kind="ExternalInput")
    tabid_d = nc.dram_tensor("tabid", [4, 4], F32, kind="ExternalInput")
    out_d = nc.dram_tensor("out", [2, B_LOC, ROW], BF16, kind="ExternalOutput")

    with ExitStack() as ctx:
        sb = lambda name, shape, dt: ctx.enter_context(
            nc.sbuf_tensor(name, shape, dt))
        tabw_t = sb("tabw_t", [P, NCH * 4], BF16)
        tabio_t = sb("tabio_t", [P, 256], BF16)
        tabid_t = sb("tabid_t", [4, 4], F32)
        in_t = [sb(f"in_t{k}", [P, NCH * NG], FP8) for k in range(NBUF)]
        sval = [sb(f"sval{k}", [4, NG], F32) for k in range(2)]
        og = [sb(f"og{k}", [P, 4 * 2 * ROW], BF16) for k in range(OBUF)]
        # parity-double-buffered per-tile temporaries
        iv = [sb(f"iv_{p}", [P, 4], I32) for p in range(4)]
        v4 = [sb(f"v4_{p}", [P, 4], I32) for p in range(4)]
        idx8 = [sb(f"idx8_{p}", [P, 8], I32) for p in range(4)]
        idxf = [sb(f"idxf_{p}", [P, 8], F32) for p in range(4)]
        tmpa = [[sb(f"tmpa_{p}_{i}", [P, 256], F32)
                 for i in range(max(1, len(ACT_BYTES)))] for p in range(4)]

        pv = [ctx.enter_context(nc.psum_tensor(f"pv{k}", [4, NG], F32))
              for k in range(2)]
        pt = [ctx.enter_context(nc.psum_tensor(f"pt{k}", [P, 16], F32))
              for k in range(2)]

        s_tabw = ctx.enter_context(nc.semaphore("s_tabw"))
        s_tabid = ctx.enter_context(nc.semaphore("s_tabid"))
        s_tabio = ctx.enter_context(nc.semaphore("s_tabio"))
        s_sub = [[ctx.enter_context(nc.semaphore(f"s_sub{j}_{u}"))
                  for u in range(NSUB)] for j in range(NBUF)]
        s_store = [ctx.enter_context(nc.semaphore(f"s_store{j}"))
                   for j in range(OBUF)]
        s_mm = ctx.enter_context(nc.semaphore("s_mm"))      # matmul groups
        s_sv = ctx.enter_context(nc.semaphore("s_sv"))      # psum->sbuf copies
        s_T = ctx.enter_context(nc.semaphore("s_T"))        # transposes done
        s_comp = ctx.enter_context(nc.semaphore("s_comp"))  # DVE-encoded tiles
        s_dve = ctx.enter_context(nc.semaphore("s_dve"))    # chain ops done
        s_ac = ctx.enter_context(nc.semaphore("s_ac"))      # ACT-encoded tiles
        s_acq = ctx.enter_context(nc.semaphore("s_acq"))    # ACT square ops

        block = ctx.enter_context(nc.Block())

        @block.sync
        def _(sync: bass.BassEngine):
            CW = NCH * NG // NSUB   # columns per sub-DMA

            def load_group(g):
                j = g % NBUF
                for u in range(NSUB):
                    sync.dma_start(
                        out=in_t[j][:, CW * u:CW * (u + 1)],
                        in_=ab_d[g, :, CW * u:CW * (u + 1)],
                    ).then_inc(s_sub[j][u], 16)

            sync.dma_start(out=tabw_t[:], in_=tabw_d[:]).then_inc(s_tabw, 16)
            load_group(0)
            sync.dma_start(out=tabid_t[:], in_=tabid_d[:]).then_inc(
                s_tabid, 16)
            sync.dma_start(out=tabio_t[:], in_=tabio_d[:]).then_inc(
                s_tabio, 16)
            for g in range(1, G):
                if g >= INFLIGHT:
                    # bounded prefetch: group g-INFLIGHT fully landed first
                    gp_ = g - INFLIGHT
                    for u in range(NSUB):
                        sync.wait_ge(s_sub[gp_ % NBUF][u],
                                     16 * (gp_ // NBUF + 1))
                if g >= NBUF:
                    # slot reuse: matmuls of group g-NBUF consumed it
                    sync.wait_ge(s_mm, g - NBUF + 1)
                load_group(g)

        @block.tensor
        def _(tensor: bass.BassEngine):
            CS = NCH // NSUB
            tensor.wait_ge(s_tabw, 16)
            # clock-ramp warmup while the first input group is in flight
            warm_rhs = tabw_t[:, None, :].to_broadcast((P, 8, NCH * 4))
            for _w in range(WARMUP_MM):
                tensor.matmul(out=pv[1][:, :], lhsT=tabw_t[:, 0:4],
                              rhs=warm_rhs, start=True, stop=True)
            for g in range(G + 1):
                if g < G:
                    j = g % NBUF
                    if g >= 2:
                        # pv[g%2] freed once ScalarE copied group g-2
                        tensor.wait_ge(s_sv, g - 1)
                    for c in range(NCH):
                        if c % CS == 0:
                            tensor.wait_ge(s_sub[j][c // CS],
                                           16 * (g // NBUF + 1))
                        ins = tensor.matmul(
                            out=pv[g % 2][:, :],
                            lhsT=tabw_t[:, 4 * c:4 * (c + 1)],
                            rhs=in_t[j][:, NG * c:NG * (c + 1)],
                            start=(c == 0),
                            stop=(c == NCH - 1),
                        )
                        if c == NCH - 1:
                            ins.then_inc(s_mm, 1)
                q = g - 1
                if q >= 0:
                    if q == 0:
                        tensor.wait_ge(s_tabid, 16)
                    tensor.wait_ge(s_sv, q + 1)
                    if q >= 2:
                        # pt[q%2] freed once the pt->iv copy of the last
                        # tile of group q-2 retired
                        tensor.wait_ge(s_dve, _op1_count(4 * (q - 2) + 3))
                    for k in range(4):
                        tensor.transpose(
                            out=pt[q % 2][:, 4 * k:4 * (k + 1)],
                            in_=sval[q % 2][:, P * k:P * (k + 1)],
                            identity=tabid_t[:],
                        ).then_inc(s_T, 1)

        @block.scalar
        def _(scalar: bass.BassEngine):
            acq = 0
            for g in range(G + 1):
                if g < G:
                    scalar.wait_ge(s_mm, g + 1)
                    if g >= 2:
                        # sval[g%2] freed once transposes of group g-2 done
                        scalar.wait_ge(s_T, 4 * (g - 1))
                    scalar.activation(
                        out=sval[g % 2][:, :], in_=pv[g % 2][:, :],
                        func=mybir.ActivationFunctionType.Copy,
                    ).then_inc(s_sv, 1)
                # ScalarE-encoded bytes for the tiles of group g-1, two
                # tiles interleaved (ACT ops need sems for same-engine RAW)
                qe = g - 1
                if 0 <= qe < G and ACT_BYTES:
                    if qe == 0:
                        scalar.wait_ge(s_tabio, 16)
                    joq = qe % OBUF
                    if qe >= OBUF:
                        scalar.wait_ge(s_store[joq], 32 * (qe // OBUF))
                    if qe >= 1:
                        # tmpa WAR: previous group's Relus retired
                        scalar.wait_ge(s_ac, 4 * len(ACT_BYTES) * qe)
                    scalar.wait_ge(s_dve, _chain_count(4 * qe + 3))
                    for pr in range(4):
                        for i, e in enumerate(ACT_BYTES):
                            scalar.activation(
                                out=tmpa[pr][i][:], in_=tabio_t[:],
                                func=mybir.ActivationFunctionType.Square,
                                bias=idxf[pr][:, PERM[e]:PERM[e] + 1],
                                scale=-1.0,
                            ).then_inc(s_acq, 1)
                            acq += 1
                    scalar.wait_ge(s_acq, acq)
                    for pr in range(4):
                        for i, e in enumerate(ACT_BYTES):
                            scalar.activation(
                                out=og[joq][:, 2048 * pr + 256 * e:
                                            2048 * pr + 256 * (e + 1)],
                                in_=tmpa[pr][i][:],
                                func=mybir.ActivationFunctionType.Relu,
                                bias=1.0, scale=-1.0,
                            ).then_inc(s_ac, 1)

        @block.vector
        def _(vector: bass.BassEngine):
            n = 0  # statically tracked s_dve count

            def chain_op(ins):
                nonlocal n
                ins.then_inc(s_dve, 1)
                n += 1

            PRS = (0, 1, 2, 3)
            for q in range(G):
                jo = q % OBUF
                if q == 0:
                    vector.wait_ge(s_tabio, 16)
                vector.wait_ge(s_T, 4 * (q + 1))
                if ACT_BYTES and q >= 1:
                    # idxf reuse: ScalarE read group q-1 (squares done)
                    vector.wait_ge(s_acq, 4 * len(ACT_BYTES) * q)
                if q >= OBUF:
                    vector.wait_ge(s_store[jo], 64 * (q // OBUF))
                # interleaved chains: each wait's producers are >=4 ops back
                for pr in PRS:
                    chain_op(vector.tensor_copy(
                        iv[pr][:], pt[q % 2][:, 4 * pr:4 * pr + 4]))
                vector.wait_ge(s_dve, n)
                for pr in PRS:
                    # v4 = [s_lo_raw(17b), s_hi_raw, x_lo, x_hi]
                    chain_op(vector.tensor_tensor(
                        out=v4[pr][:, 0:2], in0=iv[pr][:, 0:2],
                        in1=iv[pr][:, 2:4], op=mybir.AluOpType.add))
                    chain_op(vector.tensor_tensor(
                        out=v4[pr][:, 2:4], in0=iv[pr][:, 0:2],
                        in1=iv[pr][:, 2:4], op=mybir.AluOpType.bitwise_xor))
                vector.wait_ge(s_dve, n)
                for pr in PRS:
                    # fold the 2^16 carry into s_hi (s_lo_raw keeps bit 16;
                    # the &255 byte masks strip it later)
                    chain_op(vector.scalar_tensor_tensor(
                        out=v4[pr][:, 1:2], in0=v4[pr][:, 0:1], scalar=65536,
                        in1=v4[pr][:, 1:2],
                        op0=mybir.AluOpType.is_ge, op1=mybir.AluOpType.add))
                vector.wait_ge(s_dve, n)
                for pr in PRS:
                    # byte extract (fused shift+mask); idx8 holds the bytes
                    # in [s0 s2 x0 x2 | s1 s3 x1 x3] order
                    chain_op(vector.tensor_scalar(
                        out=idx8[pr][:, 0:4], in0=v4[pr][:], scalar1=255,
                        scalar2=None, op0=mybir.AluOpType.bitwise_and))
                    chain_op(vector.tensor_scalar(
                        out=idx8[pr][:, 4:8], in0=v4[pr][:], scalar1=8,
                        scalar2=255,
                        op0=mybir.AluOpType.logical_shift_right,
                        op1=mybir.AluOpType.bitwise_and))
                vector.wait_ge(s_dve, n)
                for pr in PRS:
                    chain_op(vector.tensor_copy(idxf[pr][:], idx8[pr][:]))
                vector.wait_ge(s_dve, n)
                # encode: single-src is_equal against the iota table, one op
                # per output byte, per-partition scalar = that byte's value
                for pr in PRS:
                    for i, e in enumerate(DVE_BYTES):
                        ins = vector.tensor_scalar(
                            out=og[jo][:, 2048 * pr + 256 * e:
                                       2048 * pr + 256 * (e + 1)],
                            in0=tabio_t[:],
                            scalar1=idxf[pr][:, PERM[e]:PERM[e] + 1],
                            scalar2=None,
                            op0=mybir.AluOpType.is_equal,
                        )
                        if i == len(DVE_BYTES) - 1:
                            ins.then_inc(s_comp, 1)

        @block.gpsimd
        def _(gp: bass.BassEngine):
            for qs in range(G):
                jo = qs % OBUF
                for h in range(2):
                    r0 = qs * NG + 256 * h
                    src = og[jo][:, 4096 * h:4096 * (h + 1)].rearrange(
                        "p (t two r) -> p t two r", t=2, two=2)
                    dst0 = out_d[0, r0:r0 + 256, :].rearrange(
                        "(t p) r -> p t r", p=P)
                    dst1 = out_d[1, r0:r0 + 256, :].rearrange(
                        "(t p) r -> p t r", p=P)
                    gp.wait_ge(s_comp, 4 * qs + 2 * (h + 1))
                    if ACT_BYTES:
                        gp.wait_ge(s_ac, len(ACT_BYTES) *
                                   (4 * qs + 2 * (h + 1)))
                    gp.dma_start(out=dst0, in_=src[:, :, 0, :]).then_inc(
                        s_store[jo], 16)
                    gp.dma_start(out=dst1, in_=src[:, :, 1, :]).then_inc(
                        s_store[jo], 16)

    return nc


def _make_tables():
    pos = np.arange(P, dtype=np.float64)
    w = np.zeros((NCH, P, 4), np.float64)
    for s in range(8):
        col = s // 2 if s < 4 else 2 + (s - 4) // 2
        mul = 1.0 if (s % 2 == 0) else 256.0
        for h in range(2):
            c = 2 * s + h
            w[c, :, col] = (pos + 128.0 * h) * mul
    tabw = w.transpose(1, 0, 2).reshape(P, NCH * 4).astype(ml_dtypes.bfloat16)
    tabio = np.tile(np.arange(256).astype(ml_dtypes.bfloat16)[None, :],
                    (P, 1))
    tabid = np.eye(4, dtype=np.float32)
    return tabw, tabio, tabid


def _pack_core(abt, lo):
    """[NCH, P, B] fp8 slab-chunks -> core block [G, P, NCH*NG]."""
    blk = abt[:, :, lo:lo + B_LOC].reshape(NCH, P, G, NG)
    return np.ascontiguousarray(
        blk.transpose(2, 1, 0, 3).reshape(G, P, NCH * NG))


_NC_CACHE = {}


def _get_nc(variant: str = "main"):
    if variant not in _NC_CACHE:
        _NC_CACHE[variant] = _build_nc()
    return _NC_CACHE[variant]


def _run(a: np.ndarray, b: np.ndarray, **spmd_kwargs):
    assert a.shape == (B, 4, 256) and b.shape == (B, 4, 256)
    a_t = np.ascontiguousarray(
        np.asarray(a, np.float32).reshape(B, 4, 256).transpose(1, 2, 0)
    ).astype(ml_dtypes.float8_e4m3)
    b_t = np.ascontiguousarray(
        np.asarray(b, np.float32).reshape(B, 4, 256).transpose(1, 2, 0)
    ).astype(ml_dtypes.float8_e4m3)
    abt = np.concatenate([a_t.reshape(NCH // 2, P, B),
                          b_t.reshape(NCH // 2, P, B)], axis=0)
    tabw, tabio, tabid = _make_tables()
    in_maps = [
        {
            "abt": _pack_core(abt, i * B_LOC),
            "tabw": tabw,
            "tabio": tabio,
            "tabid": tabid,
        }
        for i in range(N_CORES)
    ]
    nc = _get_nc()
    kr = run_bass_kernel_spmd(nc, in_maps, list(range(N_CORES)), **spmd_kwargs)
    shards = [kr.results[i]["out"] for i in range(N_CORES)]
    out = np.concatenate(shards, axis=1).astype(np.float32)
    return out.reshape(2, B, 4, 256), kr


def kernel(a: np.ndarray, b: np.ndarray) -> np.ndarray:
    out, _ = _run(a, b)
    return out


def run_sim():
    """CoreSim one core vs numpy oracle (invoked by test.py --sim)."""
    from concourse.bass_interp import CoreSim

    rng = np.random.default_rng(1)
    Bl = B_LOC
    ai = rng.integers(0, 256, (Bl, 4))
    bi = rng.integers(0, 256, (Bl, 4))
    ai[0] = [255] * 4
    bi[0] = [255] * 4
    ai[1] = [255, 255, 255, 255]
    bi[1] = [1, 0, 0, 0]
    a = np.zeros((Bl, 4, 256), np.float32)
    b = np.zeros((Bl, 4, 256), np.float32)
    r = np.arange(Bl)[:, None]
    j = np.arange(4)[None, :]
    a[r, j, ai] = 1.0
    b[r, j, bi] = 1.0

    a_t = np.ascontiguousarray(a.transpose(1, 2, 0)).astype(
        ml_dtypes.float8_e4m3)
    b_t = np.ascontiguousarray(b.transpose(1, 2, 0)).astype(
        ml_dtypes.float8_e4m3)
    abt = np.concatenate([a_t.reshape(NCH // 2, P, Bl),
                          b_t.reshape(NCH // 2, P, Bl)], axis=0)
    tabw, tabio, tabid = _make_tables()

    nc = _get_nc()
    sim = CoreSim(nc)
    sim.tensor("abt")[:] = _pack_core(abt, 0)
    sim.tensor("tabw")[:] = tabw
    sim.tensor("tabio")[:] = tabio
    sim.tensor("tabid")[:] = tabid
    sim.simulate()
    out = np.array(sim.tensor("out")).astype(np.float32).reshape(2, Bl, 4, 256)

    # numpy oracle
    pw = (256 ** np.arange(4)).astype(np.int64)
    a32 = (ai * pw).sum(-1)
    b32 = (bi * pw).sum(-1)
    s32 = (a32 + b32) % (2 ** 32)
    x32 = a32 ^ b32
    sb_ = np.stack([(s32 >> (8 * i)) & 255 for i in range(4)], -1)
    xb_ = np.stack([(x32 >> (8 * i)) & 255 for i in range(4)], -1)
    exp = np.zeros((2, Bl, 4, 256), np.float32)
    exp[0, r, j, sb_] = 1.0
    exp[1, r, j, xb_] = 1.0
    err = np.abs(out - exp).max()
    print(f"SIM max abs err: {err}")
    assert err == 0.0, "sim mismatch"
    print("SIM PASS")


# revision 22
# speedup vs baseline: 2.2905x; 1.0611x over previous
"""MoE-ALU (add with carry + xor over one-hot byte encodings) on 8 NeuronCores.

Semantics (validated against the jax reference bit-exactly): inputs a, b are
exact one-hot byte encodings [B, 4, 256] (little-endian bytes of 32-bit ints);
with SCALE=100 every softmax in the reference collapses to an exact one-hot, so

    out[0] = one_hot bytes of (a_int + b_int) mod 2^32
    out[1] = one_hot bytes of (a_int ^ b_int)

Layout: the host stores the one-hot inputs group/partition-major as fp8
([group, partition, chunk*column]; 0.0/1.0 are exact in fp8e4) so every DMA
descriptor is one contiguous 4 KiB run per partition, and the outputs as bf16
one-hots (exact 0/1). The device moves 8 MiB in + 16 MiB out per core instead
of 32+32 for f32 batch-major. All compute happens on device; the host only
reorders/recodes losslessly.

Device pipeline per 512-row batch group (8 groups per core):
  decode  TensorE: 16 accumulating matmuls (K=128 chunk each) of the fp8
          one-hot slabs against bf16 iota/256*iota weight columns produce
          PSUM [4, 512] = (a_lo16, a_hi16, b_lo16, b_hi16), exact in f32.
  stage   ScalarE copies PSUM -> SBUF f32 (frees the bank for group g+2).
  flip    TensorE transposes [4, 128] -> PSUM [128, 4] per 128-row tile.
  alu     VectorE per tile: int32 cast, halves add / xor, carry fold,
          fused shift+mask byte extract; two tiles' chains are interleaved
          so every RAW wait's producer is >=2 ops back (the DVE pipe does
          not self-interlock; adjacent RAW stalls ~230ns).
  encode  per output byte, one-hot = is_equal against an iota table with a
          per-partition scalar: 5 bytes as DVE tensor_scalar (bf16 4x perf
          mode), 2 bytes as GPSIMD tensor_tensor (broadcast scalar), 1 byte
          as ScalarE Square/Relu pair -- relu(1-(iota-idx)^2).
  store   ScalarE issues two 1 MiB output DMAs per group.

Raw Bass (one sync wait per instruction); rotating per-slot semaphores gate
buffer reuse; DVE same-engine RAW steps wait on a monotonically counted
semaphore.
"""
from contextlib import ExitStack

import numpy as np
import ml_dtypes

import concourse.bass as bass
from concourse import mybir
from concourse.bass_utils import run_bass_kernel_spmd

F32 = mybir.dt.float32
I32 = mybir.dt.int32
BF16 = mybir.dt.bfloat16
FP8 = mybir.dt.float8e4

P = 128
N_CORES = 8
B = 32768
B_LOC = B // N_CORES          # 4096 rows per core
ROW = 4 * 256                 # 1024 per row per tensor
NG = 512                      # batch rows per matmul group (one PSUM bank)
G = B_LOC // NG               # 8 groups
N_TILES = B_LOC // P          # 32 tiles of 128 rows
NCH = 16                      # K-chunks: 8 slabs (a0..a3,b0..b3) x 2 halves

NBUF = 4                      # input group-buffer slots
OBUF = 4                      # output group-buffer slots
NSUB = 4                      # input sub-DMAs per group
INFLIGHT = 2                  # concurrent group loads
WARMUP_MM = 8                 # dummy matmuls to ramp the PE clock

DVE_OPS = 7                   # s_dve increments per tile (chain ops)
GP_BYTES = ()                 # (Pool has no compare ops; GPSIMD issues stores)
ACT_BYTES = (1, 5)            # encode bytes on ScalarE (square+relu)
DVE_BYTES = tuple(e for e in range(8)
                  if e not in GP_BYTES and e not in ACT_BYTES)
PERM = [0, 4, 1, 5, 2, 6, 3, 7]  # output byte e -> idxf column


def _op1_count(t):
    """s_dve value once tile t's pt->iv copy has retired (quad interleave)."""
    return 4 * DVE_OPS * (t // 4) + 1 + (t % 4)


def _chain_count(t):
    """s_dve value once tile t's full chain (incl. idxf) has retired."""
    return 4 * DVE_OPS * (t // 4) + 4 * (DVE_OPS - 1) + 1 + (t % 4)


def _build_nc() -> bass.Bass:
    nc = bass.Bass(trn_type="TRN2")
    ab_d = nc.dram_tensor("abt", [G, P, NCH * NG], FP8, kind="ExternalInput")
    tabw_d = nc.dram_tensor("tabw", [P, NCH * 4], BF16, kind="ExternalInput")
    tabio_d = nc.dram_tensor("tabio", [P, 256], BF16, kind="ExternalInput")
    tabid_d = nc.dram_tensor("tabid", [4, 4], F32, kind="ExternalInput")
    out_d = nc.dram_tensor("out", [2, B_LOC, ROW], BF16, kind="ExternalOutput")

    with ExitStack() as ctx:
        sb = lambda name, shape, dt: ctx.enter_context(
            nc.sbuf_tensor(name, shape, dt))
        tabw_t = sb("tabw_t", [P, NCH * 4], BF16)
        tabio_t = sb("tabio_t", [P, 256], BF16)
        tabid_t = sb("tabid_t", [4, 4], F32)
        in_t = [sb(f"in_t{k}", [P, NCH * NG], FP8) for k in range(NBUF)]
        sval = [sb(f"sval{k}", [4, NG], F32) for k in range(2)]
        og = [sb(f"og{k}", [P, 4 * 2 * ROW], BF16) for k in range(OBUF)]
        # parity-double-buffered per-tile temporaries
        iv = [sb(f"iv_{p}", [P, 4], I32) for p in range(4)]
        v4 = [sb(f"v4_{p}", [P, 4], I32) for p in range(4)]
        idx8 = [sb(f"idx8_{p}", [P, 8], I32) for p in range(4)]
        idxf = [sb(f"idxf_{p}", [P, 8], F32) for p in range(4)]
        tmpa = [[sb(f"tmpa_{p}_{i}", [P, 256], F32)
                 for i in range(max(1, len(ACT_BYTES)))] for p in range(4)]

        pv = [ctx.enter_context(nc.psum_tensor(f"pv{k}", [4, NG], F32))
              for k in range(2)]
        pt = [ctx.enter_context(nc.psum_tensor(f"pt{k}", [P, 16], F32))
              for k in range(2)]

        s_tabw = ctx.enter_context(nc.semaphore("s_tabw"))
        s_tabid = ctx.enter_context(nc.semaphore("s_tabid"))
        s_tabio = ctx.enter_context(nc.semaphore("s_tabio"))
        s_sub = [[ctx.enter_context(nc.semaphore(f"s_sub{j}_{u}"))
                  for u in range(NSUB)] for j in range(NBUF)]
        s_store = [ctx.enter_context(nc.semaphore(f"s_store{j}"))
                   for j in range(OBUF)]
        s_mm = ctx.enter_context(nc.semaphore("s_mm"))      # matmul groups
        s_sv = ctx.enter_context(nc.semaphore("s_sv"))      # psum->sbuf copies
        s_T = ctx.enter_context(nc.semaphore("s_T"))        # transposes done
        s_comp = ctx.enter_context(nc.semaphore("s_comp"))  # DVE-encoded tiles
        s_dve = ctx.enter_context(nc.semaphore("s_dve"))    # chain ops done
        s_ac = ctx.enter_context(nc.semaphore("s_ac"))      # ACT-encoded tiles
        s_acq = ctx.enter_context(nc.semaphore("s_acq"))    # ACT square ops

        block = ctx.enter_context(nc.Block())

        @block.sync
        def _(sync: bass.BassEngine):
            CW = NCH * NG // NSUB   # columns per sub-DMA

            def load_group(g):
                j = g % NBUF
                for u in range(NSUB):
                    sync.dma_start(
                        out=in_t[j][:, CW * u:CW * (u + 1)],
                        in_=ab_d[g, :, CW * u:CW * (u + 1)],
                    ).then_inc(s_sub[j][u], 16)

            sync.dma_start(out=tabw_t[:], in_=tabw_d[:]).then_inc(s_tabw, 16)
            load_group(0)
            sync.dma_start(out=tabid_t[:], in_=tabid_d[:]).then_inc(
                s_tabid, 16)
            sync.dma_start(out=tabio_t[:], in_=tabio_d[:]).then_inc(
                s_tabio, 16)
            for g in range(1, G):
                if g >= INFLIGHT:
                    # bounded prefetch: group g-INFLIGHT fully landed first
                    gp_ = g - INFLIGHT
                    for u in range(NSUB):
                        sync.wait_ge(s_sub[gp_ % NBUF][u],
                                     16 * (gp_ // NBUF + 1))
                if g >= NBUF:
                    # slot reuse: matmuls of group g-NBUF consumed it
                    sync.wait_ge(s_mm, g - NBUF + 1)
                load_group(g)

        @block.tensor
        def _(tensor: bass.BassEngine):
            CS = NCH // NSUB
            tensor.wait_ge(s_tabw, 16)
            # clock-ramp warmup while the first input group is in flight
            warm_rhs = tabw_t[:, None, :].to_broadcast((P, 8, NCH * 4))
            for _w in range(WARMUP_MM):
                tensor.matmul(out=pv[1][:, :], lhsT=tabw_t[:, 0:4],
                              rhs=warm_rhs, start=True, stop=True)
            for g in range(G + 1):
                if g < G:
                    j = g % NBUF
                    if g >= 2:
                        # pv[g%2] freed once ScalarE copied group g-2
                        tensor.wait_ge(s_sv, g - 1)
                    for c in range(NCH):
                        if c % CS == 0:
                            tensor.wait_ge(s_sub[j][c // CS],
                                           16 * (g // NBUF + 1))
                        ins = tensor.matmul(
                            out=pv[g % 2][:, :],
                            lhsT=tabw_t[:, 4 * c:4 * (c + 1)],
                            rhs=in_t[j][:, NG * c:NG * (c + 1)],
                            start=(c == 0),
                            stop=(c == NCH - 1),
                        )
                        if c == NCH - 1:
                            ins.then_inc(s_mm, 1)
                q = g - 1
                if q >= 0:
                    if q == 0:
                        tensor.wait_ge(s_tabid, 16)
                    tensor.wait_ge(s_sv, q + 1)
                    if q >= 2:
                        # pt[q%2] freed once the pt->iv copy of the last
                        # tile of group q-2 retired
                        tensor.wait_ge(s_dve, _op1_count(4 * (q - 2) + 3))
                    for k in range(4):
                        tensor.transpose(
                            out=pt[q % 2][:, 4 * k:4 * (k + 1)],
                            in_=sval[q % 2][:, P * k:P * (k + 1)],
                            identity=tabid_t[:],
                        ).then_inc(s_T, 1)

        @block.scalar
        def _(scalar: bass.BassEngine):
            acq = 0
            for g in range(G + 1):
                if g < G:
                    scalar.wait_ge(s_mm, g + 1)
                    if g >= 2:
                        # sval[g%2] freed once transposes of group g-2 done
                        scalar.wait_ge(s_T, 4 * (g - 1))
                    scalar.activation(
                        out=sval[g % 2][:, :], in_=pv[g % 2][:, :],
                        func=mybir.ActivationFunctionType.Copy,
                    ).then_inc(s_sv, 1)
                # ScalarE-encoded bytes for the tiles of group g-1, two
                # tiles interleaved (ACT ops need sems for same-engine RAW)
                qe = g - 1
                if 0 <= qe < G and ACT_BYTES:
                    if qe == 0:
                        scalar.wait_ge(s_tabio, 16)
                    joq = qe % OBUF
                    if qe >= OBUF:
                        scalar.wait_ge(s_store[joq], 32 * (qe // OBUF))
                    if qe >= 1:
                        # tmpa WAR: previous group's Relus retired
                        scalar.wait_ge(s_ac, 4 * len(ACT_BYTES) * qe)
                    scalar.wait_ge(s_dve, _chain_count(4 * qe + 3))
                    for pr in range(4):
                        for i, e in enumerate(ACT_BYTES):
                            scalar.activation(
                                out=tmpa[pr][i][:], in_=tabio_t[:],
                                func=mybir.ActivationFunctionType.Square,
                                bias=idxf[pr][:, PERM[e]:PERM[e] + 1],
                                scale=-1.0,
                            ).then_inc(s_acq, 1)
                            acq += 1
                    scalar.wait_ge(s_acq, acq)
                    for pr in range(4):
                        for i, e in enumerate(ACT_BYTES):
                            scalar.activation(
                                out=og[joq][:, 2048 * pr + 256 * e:
                                            2048 * pr + 256 * (e + 1)],
                                in_=tmpa[pr][i][:],
                                func=mybir.ActivationFunctionType.Relu,
                                bias=1.0, scale=-1.0,
                            ).then_inc(s_ac, 1)

        @block.vector
        def _(vector: bass.BassEngine):
            n = 0  # statically tracked s_dve count

            def chain_op(ins):
                nonlocal n
                ins.then_inc(s_dve, 1)
                n += 1

            PRS = (0, 1, 2, 3)
            for q in range(G):
                jo = q % OBUF
                if q == 0:
                    vector.wait_ge(s_tabio, 16)
                vector.wait_ge(s_T, 4 * (q + 1))
                if ACT_BYTES and q >= 1:
                    # idxf reuse: ScalarE read group q-1 (squares done)
                    vector.wait_ge(s_acq, 4 * len(ACT_BYTES) * q)
                if q >= OBUF:
                    vector.wait_ge(s_store[jo], 64 * (q // OBUF))
                # interleaved chains: each wait's producers are >=4 ops back
                for pr in PRS:
                    chain_op(vector.tensor_copy(
                        iv[pr][:], pt[q % 2][:, 4 * pr:4 * pr + 4]))
                vector.wait_ge(s_dve, n)
                for pr in PRS:
                    # v4 = [s_lo_raw(17b), s_hi_raw, x_lo, x_hi]
                    chain_op(vector.tensor_tensor(
                        out=v4[pr][:, 0:2], in0=iv[pr][:, 0:2],
                        in1=iv[pr][:, 2:4], op=mybir.AluOpType.add))
                    chain_op(vector.tensor_tensor(
                        out=v4[pr][:, 2:4], in0=iv[pr][:, 0:2],
                        in1=iv[pr][:, 2:4], op=mybir.AluOpType.bitwise_xor))
                vector.wait_ge(s_dve, n)
                for pr in PRS:
                    # fold the 2^16 carry into s_hi (s_lo_raw keeps bit 16;
                    # the &255 byte masks strip it later)
                    chain_op(vector.scalar_tensor_tensor(
                        out=v4[pr][:, 1:2], in0=v4[pr][:, 0:1], scalar=65536,
                        in1=v4[pr][:, 1:2],
                        op0=mybir.AluOpType.is_ge, op1=mybir.AluOpType.add))
                vector.wait_ge(s_dve, n)
                for pr in PRS:
                    # byte extract (fused shift+mask); idx8 holds the bytes
                    # in [s0 s2 x0 x2 | s1 s3 x1 x3] order
                    chain_op(vector.tensor_scalar(
                        out=idx8[pr][:, 0:4], in0=v4[pr][:], scalar1=255,
                        scalar2=None, op0=mybir.AluOpType.bitwise_and))
                    chain_op(vector.tensor_scalar(
                        out=idx8[pr][:, 4:8], in0=v4[pr][:], scalar1=8,
                        scalar2=255,
                        op0=mybir.AluOpType.logical_shift_right,
                        op1=mybir.AluOpType.bitwise_and))
                vector.wait_ge(s_dve, n)
                for pr in PRS:
                    chain_op(vector.tensor_copy(idxf[pr][:], idx8[pr][:]))
                vector.wait_ge(s_dve, n)
                # encode: single-src is_equal against the iota table, one op
                # per output byte, per-partition scalar = that byte's value
                for pr in PRS:
                    for i, e in enumerate(DVE_BYTES):
                        ins = vector.tensor_scalar(
                            out=og[jo][:, 2048 * pr + 256 * e:
                                       2048 * pr + 256 * (e + 1)],
                            in0=tabio_t[:],
                            scalar1=idxf[pr][:, PERM[e]:PERM[e] + 1],
                            scalar2=None,
                            op0=mybir.AluOpType.is_equal,
                        )
                        if i == len(DVE_BYTES) - 1:
                            ins.then_inc(s_comp, 1)

        @block.gpsimd
        def _(gp: bass.BassEngine):
            for qs in range(G):
                jo = qs % OBUF
                for h in range(2):
                    r0 = qs * NG + 256 * h
                    src = og[jo][:, 4096 * h:4096 * (h + 1)].rearrange(
                        "p (t two r) -> p t two r", t=2, two=2)
                    dst0 = out_d[0, r0:r0 + 256, :].rearrange(
                        "(t p) r -> p t r", p=P)
                    dst1 = out_d[1, r0:r0 + 256, :].rearrange(
                        "(t p) r -> p t r", p=P)
                    gp.wait_ge(s_comp, 4 * qs + 2 * (h + 1))
                    if ACT_BYTES:
                        gp.wait_ge(s_ac, len(ACT_BYTES) *
                                   (4 * qs + 2 * (h + 1)))
                    gp.dma_start(out=dst0, in_=src[:, :, 0, :]).then_inc(
                        s_store[jo], 16)
                    gp.dma_start(out=dst1, in_=src[:, :, 1, :]).then_inc(
                        s_store[jo], 16)

    return nc


def _make_tables():
    pos = np.arange(P, dtype=np.float64)
    w = np.zeros((NCH, P, 4), np.float64)
    for s in range(8):
        col = s // 2 if s < 4 else 2 + (s - 4) // 2
        mul = 1.0 if (s % 2 == 0) else 256.0
        for h in range(2):
            c = 2 * s + h
            w[c, :, col] = (pos + 128.0 * h) * mul
    tabw = w.transpose(1, 0, 2).reshape(P, NCH * 4).astype(ml_dtypes.bfloat16)
    tabio = np.tile(np.arange(256).astype(ml_dtypes.bfloat16)[None, :],
                    (P, 1))
    tabid = np.eye(4, dtype=np.float32)
    return tabw, tabio, tabid


def _pack_core(abt, lo):
    """[NCH, P, B] fp8 slab-chunks -> core block [G, P, NCH*NG]."""
    blk = abt[:, :, lo:lo + B_LOC].reshape(NCH, P, G, NG)
    return np.ascontiguousarray(
        blk.transpose(2, 1, 0, 3).reshape(G, P, NCH * NG))


_NC_CACHE = {}


def _get_nc(variant: str = "main"):
    if variant not in _NC_CACHE:
        _NC_CACHE[variant] = _build_nc()
    return _NC_CACHE[variant]


def _run(a: np.ndarray, b: np.ndarray, **spmd_kwargs):
    assert a.shape == (B, 4, 256) and b.shape == (B, 4, 256)
    a_t = np.ascontiguousarray(
        np.asarray(a, np.float32).reshape(B, 4, 256).transpose(1, 2, 0)
    ).astype(ml_dtypes.float8_e4m3)
    b_t = np.ascontiguousarray(
        np.asarray(b, np.float32).reshape(B, 4, 256).transpose(1, 2, 0)
    ).astype(ml_dtypes.float8_e4m3)
    abt = np.concatenate([a_t.reshape(NCH // 2, P, B),
                          b_t.reshape(NCH // 2, P, B)], axis=0)
    tabw, tabio, tabid = _make_tables()
    in_maps = [
        {
            "abt": _pack_core(abt, i * B_LOC),
            "tabw": tabw,
            "tabio": tabio,
            "tabid": tabid,
        }
        for i in range(N_CORES)
    ]
    nc = _get_nc()
    kr = run_bass_kernel_spmd(nc, in_maps, list(range(N_CORES)), **spmd_kwargs)
    shards = [kr.results[i]["out"] for i in range(N_CORES)]
    out = np.concatenate(shards, axis=1).astype(np.float32)
    return out.reshape(2, B, 4, 256), kr


def kernel(a: np.ndarray, b: np.ndarray) -> np.ndarray:
    out, _ = _run(a, b)
    return out


def run_sim():
    """CoreSim one core vs numpy oracle (invoked by test.py --sim)."""
    from concourse.bass_interp import CoreSim

    rng = np.random.default_rng(1)
    Bl = B_LOC
    ai = rng.integers(0, 256, (Bl, 4))
    bi = rng.integers(0, 256, (Bl, 4))
    ai[0] = [255] * 4
    bi[0] = [255] * 4
    ai[1] = [255, 255, 255, 255]
    bi[1] = [1, 0, 0, 0]
    a = np.zeros((Bl, 4, 256), np.float32)
    b = np.zeros((Bl, 4, 256), np.float32)
    r = np.arange(Bl)[:, None]
    j = np.arange(4)[None, :]
    a[r, j, ai] = 1.0
    b[r, j, bi] = 1.0

    a_t = np.ascontiguousarray(a.transpose(1, 2, 0)).astype(
        ml_dtypes.float8_e4m3)
    b_t = np.ascontiguousarray(b.transpose(1, 2, 0)).astype(
        ml_dtypes.float8_e4m3)
    abt = np.concatenate([a_t.reshape(NCH // 2, P, Bl),
                          b_t.reshape(NCH // 2, P, Bl)], axis=0)
    tabw, tabio, tabid = _make_tables()

    nc = _get_nc()
    sim = CoreSim(nc)
    sim.tensor("abt")[:] = _pack_core(abt, 0)
    sim.tensor("tabw")[:] = tabw
    sim.tensor("tabio")[:] = tabio
    sim.tensor("tabid")[:] = tabid
    sim.simulate()
    out = np.array(sim.tensor("out")).astype(np.float32).reshape(2, Bl, 4, 256)

    # numpy oracle
    pw = (256 ** np.arange(4)).astype(np.int64)
    a32 = (ai * pw).sum(-1)
    b32 = (bi * pw).sum(-1)
    s32 = (a32 + b32) % (2 ** 32)
    x32 = a32 ^ b32
    sb_ = np.stack([(s32 >> (8 * i)) & 255 for i in range(4)], -1)
    xb_ = np.stack([(x32 >> (8 * i)) & 255 for i in range(4)], -1)
    exp = np.zeros((2, Bl, 4, 256), np.float32)
    exp[0, r, j, sb_] = 1.0
    exp[1, r, j, xb_] = 1.0
    err = np.abs(out - exp).max()
    print(f"SIM max abs err: {err}")
    assert err == 0.0, "sim mismatch"
    print("SIM PASS")


# revision 23
# speedup vs baseline: 2.3127x; 1.0097x over previous
"""MoE-ALU (add with carry + xor over one-hot byte encodings) on 8 NeuronCores.

Semantics (validated against the jax reference bit-exactly): inputs a, b are
exact one-hot byte encodings [B, 4, 256] (little-endian bytes of 32-bit ints);
with SCALE=100 every softmax in the reference collapses to an exact one-hot, so

    out[0] = one_hot bytes of (a_int + b_int) mod 2^32
    out[1] = one_hot bytes of (a_int ^ b_int)

Layout: the host stores the one-hot inputs group/partition-major as fp8
([group, partition, chunk*column]; 0.0/1.0 are exact in fp8e4) so every DMA
descriptor is one contiguous 4 KiB run per partition, and the outputs as bf16
one-hots (exact 0/1). The device moves 8 MiB in + 16 MiB out per core instead
of 32+32 for f32 batch-major. All compute happens on device; the host only
reorders/recodes losslessly.

Device pipeline per 512-row batch group (8 groups per core):
  decode  TensorE: 16 accumulating matmuls (K=128 chunk each) of the fp8
          one-hot slabs against bf16 iota/256*iota weight columns produce
          PSUM [4, 512] = (a_lo16, a_hi16, b_lo16, b_hi16), exact in f32.
  stage   ScalarE copies PSUM -> SBUF f32 (frees the bank for group g+2).
  flip    TensorE transposes [4, 128] -> PSUM [128, 4] per 128-row tile.
  alu     VectorE per tile: int32 cast, halves add / xor, carry fold,
          fused shift+mask byte extract; two tiles' chains are interleaved
          so every RAW wait's producer is >=2 ops back (the DVE pipe does
          not self-interlock; adjacent RAW stalls ~230ns).
  encode  per output byte, one-hot = is_equal against an iota table with a
          per-partition scalar: 5 bytes as DVE tensor_scalar (bf16 4x perf
          mode), 2 bytes as GPSIMD tensor_tensor (broadcast scalar), 1 byte
          as ScalarE Square/Relu pair -- relu(1-(iota-idx)^2).
  store   ScalarE issues two 1 MiB output DMAs per group.

Raw Bass (one sync wait per instruction); rotating per-slot semaphores gate
buffer reuse; DVE same-engine RAW steps wait on a monotonically counted
semaphore.
"""
from contextlib import ExitStack

import numpy as np
import ml_dtypes

import concourse.bass as bass
from concourse import mybir
from concourse.bass_utils import run_bass_kernel_spmd

F32 = mybir.dt.float32
I32 = mybir.dt.int32
BF16 = mybir.dt.bfloat16
FP8 = mybir.dt.float8e4

P = 128
N_CORES = 8
B = 32768
B_LOC = B // N_CORES          # 4096 rows per core
ROW = 4 * 256                 # 1024 per row per tensor
NG = 512                      # batch rows per matmul group (one PSUM bank)
G = B_LOC // NG               # 8 groups
N_TILES = B_LOC // P          # 32 tiles of 128 rows
NCH = 16                      # K-chunks: 8 slabs (a0..a3,b0..b3) x 2 halves

NBUF = 4                      # input group-buffer slots
OBUF = 4                      # output group-buffer slots
NSUB = 4                      # input sub-DMAs per group
INFLIGHT = 2                  # concurrent group loads
WARMUP_MM = 8                 # dummy matmuls to ramp the PE clock

DVE_OPS = 7                   # s_dve increments per tile (chain ops)
GP_BYTES = ()                 # (Pool has no compare ops; GPSIMD issues stores)
ACT_BYTES = (1, 5)            # encode bytes on ScalarE (square+relu)
DVE_BYTES = tuple(e for e in range(8)
                  if e not in GP_BYTES and e not in ACT_BYTES)
PERM = [0, 4, 1, 5, 2, 6, 3, 7]  # output byte e -> idxf column


def _op1_count(t):
    """s_dve value once tile t's pt->iv copy has retired (quad interleave)."""
    return 4 * DVE_OPS * (t // 4) + 1 + (t % 4)


def _chain_count(t):
    """s_dve value once tile t's full chain (incl. idxf) has retired."""
    return 4 * DVE_OPS * (t // 4) + 4 * (DVE_OPS - 1) + 1 + (t % 4)


def _build_nc() -> bass.Bass:
    nc = bass.Bass(trn_type="TRN2")
    ab_d = nc.dram_tensor("abt", [G, P, NCH * NG], FP8, kind="ExternalInput")
    tabw_d = nc.dram_tensor("tabw", [P, NCH * 4], BF16, kind="ExternalInput")
    tabio_d = nc.dram_tensor("tabio", [P, 256], BF16, kind="ExternalInput")
    tabid_d = nc.dram_tensor("tabid", [4, 4], F32, kind="ExternalInput")
    out_d = nc.dram_tensor("out", [2, B_LOC, ROW], BF16, kind="ExternalOutput")

    with ExitStack() as ctx:
        sb = lambda name, shape, dt: ctx.enter_context(
            nc.sbuf_tensor(name, shape, dt))
        tabw_t = sb("tabw_t", [P, NCH * 4], BF16)
        tabio_t = sb("tabio_t", [P, 256], BF16)
        tabid_t = sb("tabid_t", [4, 4], F32)
        in_t = [sb(f"in_t{k}", [P, NCH * NG], FP8) for k in range(NBUF)]
        sval = [sb(f"sval{k}", [4, NG], F32) for k in range(2)]
        og = [sb(f"og{k}", [P, 4 * 2 * ROW], BF16) for k in range(OBUF)]
        # parity-double-buffered per-tile temporaries
        iv = [sb(f"iv_{p}", [P, 4], I32) for p in range(4)]
        v4 = [sb(f"v4_{p}", [P, 4], I32) for p in range(4)]
        idx8 = [sb(f"idx8_{p}", [P, 8], I32) for p in range(4)]
        idxf = [sb(f"idxf_{p}", [P, 8], F32) for p in range(4)]
        tmpa = [[sb(f"tmpa_{p}_{i}", [P, 256], F32)
                 for i in range(max(1, len(ACT_BYTES)))] for p in range(4)]

        pv = [ctx.enter_context(nc.psum_tensor(f"pv{k}", [4, NG], F32))
              for k in range(2)]
        pt = [ctx.enter_context(nc.psum_tensor(f"pt{k}", [P, 16], F32))
              for k in range(2)]

        s_tabw = ctx.enter_context(nc.semaphore("s_tabw"))
        s_tabid = ctx.enter_context(nc.semaphore("s_tabid"))
        s_tabio = ctx.enter_context(nc.semaphore("s_tabio"))
        s_sub = [[ctx.enter_context(nc.semaphore(f"s_sub{j}_{u}"))
                  for u in range(NSUB)] for j in range(NBUF)]
        s_store = [ctx.enter_context(nc.semaphore(f"s_store{j}"))
                   for j in range(OBUF)]
        s_mm = ctx.enter_context(nc.semaphore("s_mm"))      # matmul groups
        s_sv = ctx.enter_context(nc.semaphore("s_sv"))      # psum->sbuf copies
        s_T = ctx.enter_context(nc.semaphore("s_T"))        # transposes done
        s_comp = ctx.enter_context(nc.semaphore("s_comp"))  # DVE-encoded tiles
        s_dve = ctx.enter_context(nc.semaphore("s_dve"))    # chain ops done
        s_ac = ctx.enter_context(nc.semaphore("s_ac"))      # ACT-encoded tiles
        s_acq = ctx.enter_context(nc.semaphore("s_acq"))    # ACT square ops

        block = ctx.enter_context(nc.Block())

        @block.sync
        def _(sync: bass.BassEngine):
            CW = NCH * NG // NSUB   # columns per sub-DMA

            def load_group(g):
                j = g % NBUF
                for u in range(NSUB):
                    sync.dma_start(
                        out=in_t[j][:, CW * u:CW * (u + 1)],
                        in_=ab_d[g, :, CW * u:CW * (u + 1)],
                    ).then_inc(s_sub[j][u], 16)

            sync.dma_start(out=tabw_t[:], in_=tabw_d[:]).then_inc(s_tabw, 16)
            load_group(0)
            sync.dma_start(out=tabid_t[:], in_=tabid_d[:]).then_inc(
                s_tabid, 16)
            sync.dma_start(out=tabio_t[:], in_=tabio_d[:]).then_inc(
                s_tabio, 16)
            for g in range(1, G):
                if g >= INFLIGHT:
                    # bounded prefetch: group g-INFLIGHT fully landed first
                    gp_ = g - INFLIGHT
                    for u in range(NSUB):
                        sync.wait_ge(s_sub[gp_ % NBUF][u],
                                     16 * (gp_ // NBUF + 1))
                if g >= NBUF:
                    # slot reuse: matmuls of group g-NBUF consumed it
                    sync.wait_ge(s_mm, g - NBUF + 1)
                load_group(g)

        @block.tensor
        def _(tensor: bass.BassEngine):
            CS = NCH // NSUB
            tensor.wait_ge(s_tabw, 16)
            # clock-ramp warmup while the first input group is in flight
            warm_rhs = tabw_t[:, None, :].to_broadcast((P, 8, NCH * 4))
            for _w in range(WARMUP_MM):
                tensor.matmul(out=pv[1][:, :], lhsT=tabw_t[:, 0:4],
                              rhs=warm_rhs, start=True, stop=True)
            for g in range(G + 1):
                if g < G:
                    j = g % NBUF
                    if g >= 2:
                        # pv[g%2] freed once ScalarE copied group g-2
                        tensor.wait_ge(s_sv, g - 1)
                    for c in range(NCH):
                        if c % CS == 0:
                            tensor.wait_ge(s_sub[j][c // CS],
                                           16 * (g // NBUF + 1))
                        ins = tensor.matmul(
                            out=pv[g % 2][:, :],
                            lhsT=tabw_t[:, 4 * c:4 * (c + 1)],
                            rhs=in_t[j][:, NG * c:NG * (c + 1)],
                            start=(c == 0),
                            stop=(c == NCH - 1),
                        )
                        if c == NCH - 1:
                            ins.then_inc(s_mm, 1)
                q = g - 1
                if q >= 0:
                    if q == 0:
                        tensor.wait_ge(s_tabid, 16)
                    tensor.wait_ge(s_sv, q + 1)
                    if q >= 2:
                        # pt[q%2] freed once the pt->iv copy of the last
                        # tile of group q-2 retired
                        tensor.wait_ge(s_dve, _op1_count(4 * (q - 2) + 3))
                    for k in range(4):
                        tensor.transpose(
                            out=pt[q % 2][:, 4 * k:4 * (k + 1)],
                            in_=sval[q % 2][:, P * k:P * (k + 1)],
                            identity=tabid_t[:],
                        ).then_inc(s_T, 1)

        @block.scalar
        def _(scalar: bass.BassEngine):
            acq = 0
            for g in range(G + 1):
                if g < G:
                    scalar.wait_ge(s_mm, g + 1)
                    if g >= 2:
                        # sval[g%2] freed once transposes of group g-2 done
                        scalar.wait_ge(s_T, 4 * (g - 1))
                    scalar.activation(
                        out=sval[g % 2][:, :], in_=pv[g % 2][:, :],
                        func=mybir.ActivationFunctionType.Copy,
                    ).then_inc(s_sv, 1)
                # ScalarE-encoded bytes for the tiles of group g-1, two
                # tiles interleaved (ACT ops need sems for same-engine RAW)
                qe = g - 1
                if 0 <= qe < G and ACT_BYTES:
                    if qe == 0:
                        scalar.wait_ge(s_tabio, 16)
                    joq = qe % OBUF
                    if qe >= OBUF:
                        scalar.wait_ge(s_store[joq], 64 * (qe // OBUF))
                    if qe >= 1:
                        # tmpa WAR: previous group's Relus retired
                        scalar.wait_ge(s_ac, 4 * len(ACT_BYTES) * qe)
                    scalar.wait_ge(s_dve, _chain_count(4 * qe + 3))
                    for pr in range(4):
                        for i, e in enumerate(ACT_BYTES):
                            scalar.activation(
                                out=tmpa[pr][i][:], in_=tabio_t[:],
                                func=mybir.ActivationFunctionType.Square,
                                bias=idxf[pr][:, PERM[e]:PERM[e] + 1],
                                scale=-1.0,
                            ).then_inc(s_acq, 1)
                            acq += 1
                    scalar.wait_ge(s_acq, acq)
                    for pr in range(4):
                        for i, e in enumerate(ACT_BYTES):
                            scalar.activation(
                                out=og[joq][:, 2048 * pr + 256 * e:
                                            2048 * pr + 256 * (e + 1)],
                                in_=tmpa[pr][i][:],
                                func=mybir.ActivationFunctionType.Relu,
                                bias=1.0, scale=-1.0,
                            ).then_inc(s_ac, 1)

        @block.vector
        def _(vector: bass.BassEngine):
            n = 0  # statically tracked s_dve count

            def chain_op(ins):
                nonlocal n
                ins.then_inc(s_dve, 1)
                n += 1

            PRS = (0, 1, 2, 3)
            for q in range(G):
                jo = q % OBUF
                if q == 0:
                    vector.wait_ge(s_tabio, 16)
                vector.wait_ge(s_T, 4 * (q + 1))
                if ACT_BYTES and q >= 1:
                    # idxf reuse: ScalarE read group q-1 (squares done)
                    vector.wait_ge(s_acq, 4 * len(ACT_BYTES) * q)
                if q >= OBUF:
                    vector.wait_ge(s_store[jo], 64 * (q // OBUF))
                # interleaved chains: each wait's producers are >=4 ops back
                for pr in PRS:
                    chain_op(vector.tensor_copy(
                        iv[pr][:], pt[q % 2][:, 4 * pr:4 * pr + 4]))
                vector.wait_ge(s_dve, n)
                for pr in PRS:
                    # v4 = [s_lo_raw(17b), s_hi_raw, x_lo, x_hi]
                    chain_op(vector.tensor_tensor(
                        out=v4[pr][:, 0:2], in0=iv[pr][:, 0:2],
                        in1=iv[pr][:, 2:4], op=mybir.AluOpType.add))
                    chain_op(vector.tensor_tensor(
                        out=v4[pr][:, 2:4], in0=iv[pr][:, 0:2],
                        in1=iv[pr][:, 2:4], op=mybir.AluOpType.bitwise_xor))
                vector.wait_ge(s_dve, n)
                for pr in PRS:
                    # fold the 2^16 carry into s_hi (s_lo_raw keeps bit 16;
                    # the &255 byte masks strip it later)
                    chain_op(vector.scalar_tensor_tensor(
                        out=v4[pr][:, 1:2], in0=v4[pr][:, 0:1], scalar=65536,
                        in1=v4[pr][:, 1:2],
                        op0=mybir.AluOpType.is_ge, op1=mybir.AluOpType.add))
                vector.wait_ge(s_dve, n)
                for pr in PRS:
                    # byte extract (fused shift+mask); idx8 holds the bytes
                    # in [s0 s2 x0 x2 | s1 s3 x1 x3] order
                    chain_op(vector.tensor_scalar(
                        out=idx8[pr][:, 0:4], in0=v4[pr][:], scalar1=255,
                        scalar2=None, op0=mybir.AluOpType.bitwise_and))
                    chain_op(vector.tensor_scalar(
                        out=idx8[pr][:, 4:8], in0=v4[pr][:], scalar1=8,
                        scalar2=255,
                        op0=mybir.AluOpType.logical_shift_right,
                        op1=mybir.AluOpType.bitwise_and))
                vector.wait_ge(s_dve, n)
                for pr in PRS:
                    chain_op(vector.tensor_copy(idxf[pr][:], idx8[pr][:]))
                vector.wait_ge(s_dve, n)
                # encode: single-src is_equal against the iota table, one op
                # per output byte, per-partition scalar = that byte's value
                for pr in PRS:
                    for i, e in enumerate(DVE_BYTES):
                        ins = vector.tensor_scalar(
                            out=og[jo][:, 2048 * pr + 256 * e:
                                       2048 * pr + 256 * (e + 1)],
                            in0=tabio_t[:],
                            scalar1=idxf[pr][:, PERM[e]:PERM[e] + 1],
                            scalar2=None,
                            op0=mybir.AluOpType.is_equal,
                        )
                        if i == len(DVE_BYTES) - 1:
                            ins.then_inc(s_comp, 1)

        @block.gpsimd
        def _(gp: bass.BassEngine):
            for qs in range(G):
                jo = qs % OBUF
                for h in range(2):
                    r0 = qs * NG + 256 * h
                    src = og[jo][:, 4096 * h:4096 * (h + 1)].rearrange(
                        "p (t two r) -> p t two r", t=2, two=2)
                    dst0 = out_d[0, r0:r0 + 256, :].rearrange(
                        "(t p) r -> p t r", p=P)
                    dst1 = out_d[1, r0:r0 + 256, :].rearrange(
                        "(t p) r -> p t r", p=P)
                    gp.wait_ge(s_comp, 4 * qs + 2 * (h + 1))
                    if ACT_BYTES:
                        gp.wait_ge(s_ac, len(ACT_BYTES) *
                                   (4 * qs + 2 * (h + 1)))
                    gp.dma_start(out=dst0, in_=src[:, :, 0, :]).then_inc(
                        s_store[jo], 16)
                    gp.dma_start(out=dst1, in_=src[:, :, 1, :]).then_inc(
                        s_store[jo], 16)

    return nc


def _make_tables():
    pos = np.arange(P, dtype=np.float64)
    w = np.zeros((NCH, P, 4), np.float64)
    for s in range(8):
        col = s // 2 if s < 4 else 2 + (s - 4) // 2
        mul = 1.0 if (s % 2 == 0) else 256.0
        for h in range(2):
            c = 2 * s + h
            w[c, :, col] = (pos + 128.0 * h) * mul
    tabw = w.transpose(1, 0, 2).reshape(P, NCH * 4).astype(ml_dtypes.bfloat16)
    tabio = np.tile(np.arange(256).astype(ml_dtypes.bfloat16)[None, :],
                    (P, 1))
    tabid = np.eye(4, dtype=np.float32)
    return tabw, tabio, tabid


def _pack_core(abt, lo):
    """[NCH, P, B] fp8 slab-chunks -> core block [G, P, NCH*NG]."""
    blk = abt[:, :, lo:lo + B_LOC].reshape(NCH, P, G, NG)
    return np.ascontiguousarray(
        blk.transpose(2, 1, 0, 3).reshape(G, P, NCH * NG))


_NC_CACHE = {}


def _get_nc(variant: str = "main"):
    if variant not in _NC_CACHE:
        _NC_CACHE[variant] = _build_nc()
    return _NC_CACHE[variant]


def _run(a: np.ndarray, b: np.ndarray, **spmd_kwargs):
    assert a.shape == (B, 4, 256) and b.shape == (B, 4, 256)
    a_t = np.ascontiguousarray(
        np.asarray(a, np.float32).reshape(B, 4, 256).transpose(1, 2, 0)
    ).astype(ml_dtypes.float8_e4m3)
    b_t = np.ascontiguousarray(
        np.asarray(b, np.float32).reshape(B, 4, 256).transpose(1, 2, 0)
    ).astype(ml_dtypes.float8_e4m3)
    abt = np.concatenate([a_t.reshape(NCH // 2, P, B),
                          b_t.reshape(NCH // 2, P, B)], axis=0)
    tabw, tabio, tabid = _make_tables()
    in_maps = [
        {
            "abt": _pack_core(abt, i * B_LOC),
            "tabw": tabw,
            "tabio": tabio,
            "tabid": tabid,
        }
        for i in range(N_CORES)
    ]
    nc = _get_nc()
    kr = run_bass_kernel_spmd(nc, in_maps, list(range(N_CORES)), **spmd_kwargs)
    shards = [kr.results[i]["out"] for i in range(N_CORES)]
    out = np.concatenate(shards, axis=1).astype(np.float32)
    return out.reshape(2, B, 4, 256), kr


def kernel(a: np.ndarray, b: np.ndarray) -> np.ndarray:
    out, _ = _run(a, b)
    return out


def run_sim():
    """CoreSim one core vs numpy oracle (invoked by test.py --sim)."""
    from concourse.bass_interp import CoreSim

    rng = np.random.default_rng(1)
    Bl = B_LOC
    ai = rng.integers(0, 256, (Bl, 4))
    bi = rng.integers(0, 256, (Bl, 4))
    ai[0] = [255] * 4
    bi[0] = [255] * 4
    ai[1] = [255, 255, 255, 255]
    bi[1] = [1, 0, 0, 0]
    a = np.zeros((Bl, 4, 256), np.float32)
    b = np.zeros((Bl, 4, 256), np.float32)
    r = np.arange(Bl)[:, None]
    j = np.arange(4)[None, :]
    a[r, j, ai] = 1.0
    b[r, j, bi] = 1.0

    a_t = np.ascontiguousarray(a.transpose(1, 2, 0)).astype(
        ml_dtypes.float8_e4m3)
    b_t = np.ascontiguousarray(b.transpose(1, 2, 0)).astype(
        ml_dtypes.float8_e4m3)
    abt = np.concatenate([a_t.reshape(NCH // 2, P, Bl),
                          b_t.reshape(NCH // 2, P, Bl)], axis=0)
    tabw, tabio, tabid = _make_tables()

    nc = _get_nc()
    sim = CoreSim(nc)
    sim.tensor("abt")[:] = _pack_core(abt, 0)
    sim.tensor("tabw")[:] = tabw
    sim.tensor("tabio")[:] = tabio
    sim.tensor("tabid")[:] = tabid
    sim.simulate()
    out = np.array(sim.tensor("out")).astype(np.float32).reshape(2, Bl, 4, 256)

    # numpy oracle
    pw = (256 ** np.arange(4)).astype(np.int64)
    a32 = (ai * pw).sum(-1)
    b32 = (bi * pw).sum(-1)
    s32 = (a32 + b32) % (2 ** 32)
    x32 = a32 ^ b32
    sb_ = np.stack([(s32 >> (8 * i)) & 255 for i in range(4)], -1)
    xb_ = np.stack([(x32 >> (8 * i)) & 255 for i in range(4)], -1)
    exp = np.zeros((2, Bl, 4, 256), np.float32)
    exp[0, r, j, sb_] = 1.0
    exp[1, r, j, xb_] = 1.0
    err = np.abs(out - exp).max()
    print(f"SIM max abs err: {err}")
    assert err == 0.0, "sim mismatch"
    print("SIM PASS")


# revision 24
# speedup vs baseline: 2.3153x; 1.0011x over previous
"""MoE-ALU (add with carry + xor over one-hot byte encodings) on 8 NeuronCores.

Semantics (validated against the jax reference bit-exactly): inputs a, b are
exact one-hot byte encodings [B, 4, 256] (little-endian bytes of 32-bit ints);
with SCALE=100 every softmax in the reference collapses to an exact one-hot, so

    out[0] = one_hot bytes of (a_int + b_int) mod 2^32
    out[1] = one_hot bytes of (a_int ^ b_int)

Layout: the host stores the one-hot inputs group/partition-major as fp8
([group, partition, chunk*column]; 0.0/1.0 are exact in fp8e4) so every DMA
descriptor is one contiguous 4 KiB run per partition, and the outputs as bf16
one-hots (exact 0/1). The device moves 8 MiB in + 16 MiB out per core instead
of 32+32 for f32 batch-major. All compute happens on device; the host only
reorders/recodes losslessly.

Device pipeline per 512-row batch group (8 groups per core):
  decode  TensorE: 16 accumulating matmuls (K=128 chunk each) of the fp8
          one-hot slabs against bf16 iota/256*iota weight columns produce
          PSUM [4, 512] = (a_lo16, a_hi16, b_lo16, b_hi16), exact in f32.
  stage   ScalarE copies PSUM -> SBUF f32 (frees the bank for group g+2).
  flip    TensorE transposes [4, 128] -> PSUM [128, 4] per 128-row tile.
  alu     VectorE per tile: int32 cast, halves add / xor, carry fold,
          fused shift+mask byte extract; two tiles' chains are interleaved
          so every RAW wait's producer is >=2 ops back (the DVE pipe does
          not self-interlock; adjacent RAW stalls ~230ns).
  encode  per output byte, one-hot = is_equal against an iota table with a
          per-partition scalar: 5 bytes as DVE tensor_scalar (bf16 4x perf
          mode), 2 bytes as GPSIMD tensor_tensor (broadcast scalar), 1 byte
          as ScalarE Square/Relu pair -- relu(1-(iota-idx)^2).
  store   ScalarE issues two 1 MiB output DMAs per group.

Raw Bass (one sync wait per instruction); rotating per-slot semaphores gate
buffer reuse; DVE same-engine RAW steps wait on a monotonically counted
semaphore.
"""
from contextlib import ExitStack

import numpy as np
import ml_dtypes

import concourse.bass as bass
from concourse import mybir
from concourse.bass_utils import run_bass_kernel_spmd

F32 = mybir.dt.float32
I32 = mybir.dt.int32
BF16 = mybir.dt.bfloat16
FP8 = mybir.dt.float8e4

P = 128
N_CORES = 8
B = 32768
B_LOC = B // N_CORES          # 4096 rows per core
ROW = 4 * 256                 # 1024 per row per tensor
NG = 512                      # batch rows per matmul group (one PSUM bank)
G = B_LOC // NG               # 8 groups
N_TILES = B_LOC // P          # 32 tiles of 128 rows
NCH = 16                      # K-chunks: 8 slabs (a0..a3,b0..b3) x 2 halves

NBUF = 4                      # input group-buffer slots
OBUF = 4                      # output group-buffer slots
NSUB = 4                      # input sub-DMAs per group
INFLIGHT = 2                  # concurrent group loads
WARMUP_MM = 8                 # dummy matmuls to ramp the PE clock

DVE_OPS = 7                   # s_dve increments per tile (chain ops)
GP_BYTES = ()                 # (Pool has no compare ops; GPSIMD issues stores)
ACT_BYTES = (1, 5)            # encode bytes on ScalarE (square+relu)
DVE_BYTES = tuple(e for e in range(8)
                  if e not in GP_BYTES and e not in ACT_BYTES)
PERM = [0, 4, 1, 5, 2, 6, 3, 7]  # output byte e -> idxf column


def _op1_count(t):
    """s_dve value once tile t's pt->iv copy has retired (quad interleave)."""
    return 4 * DVE_OPS * (t // 4) + 1 + (t % 4)


def _chain_count(t):
    """s_dve value once tile t's full chain (incl. idxf) has retired."""
    return 4 * DVE_OPS * (t // 4) + 4 * (DVE_OPS - 1) + 1 + (t % 4)


def _build_nc() -> bass.Bass:
    nc = bass.Bass(trn_type="TRN2")
    ab_d = nc.dram_tensor("abt", [G, P, NCH * NG], FP8, kind="ExternalInput")
    tabw_d = nc.dram_tensor("tabw", [P, NCH * 4], BF16, kind="ExternalInput")
    tabio_d = nc.dram_tensor("tabio", [P, 256], BF16, kind="ExternalInput")
    tabid_d = nc.dram_tensor("tabid", [4, 4], F32, kind="ExternalInput")
    out_d = nc.dram_tensor("out", [2, B_LOC, ROW], BF16, kind="ExternalOutput")

    with ExitStack() as ctx:
        sb = lambda name, shape, dt: ctx.enter_context(
            nc.sbuf_tensor(name, shape, dt))
        tabw_t = sb("tabw_t", [P, NCH * 4], BF16)
        tabio_t = sb("tabio_t", [P, 256], BF16)
        tabid_t = sb("tabid_t", [4, 4], F32)
        in_t = [sb(f"in_t{k}", [P, NCH * NG], FP8) for k in range(NBUF)]
        sval = [sb(f"sval{k}", [4, NG], F32) for k in range(2)]
        og = [sb(f"og{k}", [P, 4 * 2 * ROW], BF16) for k in range(OBUF)]
        # parity-double-buffered per-tile temporaries
        iv = [sb(f"iv_{p}", [P, 4], I32) for p in range(4)]
        v4 = [sb(f"v4_{p}", [P, 4], I32) for p in range(4)]
        idx8 = [sb(f"idx8_{p}", [P, 8], I32) for p in range(4)]
        idxf = [sb(f"idxf_{p}", [P, 8], F32) for p in range(4)]
        tmpa = [[sb(f"tmpa_{p}_{i}", [P, 256], F32)
                 for i in range(max(1, len(ACT_BYTES)))] for p in range(4)]

        pv = [ctx.enter_context(nc.psum_tensor(f"pv{k}", [4, NG], F32))
              for k in range(2)]
        pt = [ctx.enter_context(nc.psum_tensor(f"pt{k}", [P, 16], F32))
              for k in range(2)]

        s_tabw = ctx.enter_context(nc.semaphore("s_tabw"))
        s_tabid = ctx.enter_context(nc.semaphore("s_tabid"))
        s_tabio = ctx.enter_context(nc.semaphore("s_tabio"))
        s_sub = [[ctx.enter_context(nc.semaphore(f"s_sub{j}_{u}"))
                  for u in range(NSUB)] for j in range(NBUF)]
        s_store = [ctx.enter_context(nc.semaphore(f"s_store{j}"))
                   for j in range(OBUF)]
        s_mm = ctx.enter_context(nc.semaphore("s_mm"))      # matmul groups
        s_sv = ctx.enter_context(nc.semaphore("s_sv"))      # psum->sbuf copies
        s_T = ctx.enter_context(nc.semaphore("s_T"))        # transposes done
        s_comp = ctx.enter_context(nc.semaphore("s_comp"))  # DVE-encoded tiles
        s_dve = ctx.enter_context(nc.semaphore("s_dve"))    # chain ops done
        s_ac = ctx.enter_context(nc.semaphore("s_ac"))      # ACT-encoded tiles
        s_acq = ctx.enter_context(nc.semaphore("s_acq"))    # ACT square ops

        block = ctx.enter_context(nc.Block())

        @block.sync
        def _(sync: bass.BassEngine):
            CW = NCH * NG // NSUB   # columns per sub-DMA

            def load_group(g):
                j = g % NBUF
                for u in range(NSUB):
                    sync.dma_start(
                        out=in_t[j][:, CW * u:CW * (u + 1)],
                        in_=ab_d[g, :, CW * u:CW * (u + 1)],
                    ).then_inc(s_sub[j][u], 16)

            sync.dma_start(out=tabw_t[:], in_=tabw_d[:]).then_inc(s_tabw, 16)
            load_group(0)
            sync.dma_start(out=tabid_t[:], in_=tabid_d[:]).then_inc(
                s_tabid, 16)
            sync.dma_start(out=tabio_t[:], in_=tabio_d[:]).then_inc(
                s_tabio, 16)
            for g in range(1, G):
                if g >= INFLIGHT:
                    # bounded prefetch: group g-INFLIGHT fully landed first
                    gp_ = g - INFLIGHT
                    for u in range(NSUB):
                        sync.wait_ge(s_sub[gp_ % NBUF][u],
                                     16 * (gp_ // NBUF + 1))
                if g >= NBUF:
                    # slot reuse: matmuls of group g-NBUF consumed it
                    sync.wait_ge(s_mm, g - NBUF + 1)
                load_group(g)

        @block.tensor
        def _(tensor: bass.BassEngine):
            CS = NCH // NSUB
            tensor.wait_ge(s_tabw, 16)
            # clock-ramp warmup while the first input group is in flight
            warm_rhs = tabw_t[:, None, :].to_broadcast((P, 8, NCH * 4))
            for _w in range(WARMUP_MM):
                tensor.matmul(out=pv[1][:, :], lhsT=tabw_t[:, 0:4],
                              rhs=warm_rhs, start=True, stop=True)
            for g in range(G + 1):
                if g < G:
                    j = g % NBUF
                    if g >= 2:
                        # pv[g%2] freed once ScalarE copied group g-2
                        tensor.wait_ge(s_sv, g - 1)
                    for c in range(NCH):
                        if c % CS == 0:
                            tensor.wait_ge(s_sub[j][c // CS],
                                           16 * (g // NBUF + 1))
                        ins = tensor.matmul(
                            out=pv[g % 2][:, :],
                            lhsT=tabw_t[:, 4 * c:4 * (c + 1)],
                            rhs=in_t[j][:, NG * c:NG * (c + 1)],
                            start=(c == 0),
                            stop=(c == NCH - 1),
                        )
                        if c == NCH - 1:
                            ins.then_inc(s_mm, 1)
                q = g - 1
                if q >= 0:
                    if q == 0:
                        tensor.wait_ge(s_tabid, 16)
                    tensor.wait_ge(s_sv, q + 1)
                    if q >= 2:
                        # pt[q%2] freed once the pt->iv copy of the last
                        # tile of group q-2 retired
                        tensor.wait_ge(s_dve, _op1_count(4 * (q - 2) + 3))
                    for k in range(4):
                        tensor.transpose(
                            out=pt[q % 2][:, 4 * k:4 * (k + 1)],
                            in_=sval[q % 2][:, P * k:P * (k + 1)],
                            identity=tabid_t[:],
                        ).then_inc(s_T, 1)

        @block.scalar
        def _(scalar: bass.BassEngine):
            acq = 0
            for g in range(G + 1):
                if g < G:
                    scalar.wait_ge(s_mm, g + 1)
                    if g >= 2:
                        # sval[g%2] freed once transposes of group g-2 done
                        scalar.wait_ge(s_T, 4 * (g - 1))
                    scalar.activation(
                        out=sval[g % 2][:, :], in_=pv[g % 2][:, :],
                        func=mybir.ActivationFunctionType.Copy,
                    ).then_inc(s_sv, 1)
                # ScalarE-encoded bytes for the tiles of group g-1, two
                # tiles interleaved (ACT ops need sems for same-engine RAW)
                qe = g - 1
                if 0 <= qe < G and ACT_BYTES:
                    if qe == 0:
                        scalar.wait_ge(s_tabio, 16)
                    joq = qe % OBUF
                    if qe >= OBUF:
                        scalar.wait_ge(s_store[joq], 128 * (qe // OBUF))
                    if qe >= 1:
                        # tmpa WAR: previous group's Relus retired
                        scalar.wait_ge(s_ac, 4 * len(ACT_BYTES) * qe)
                    scalar.wait_ge(s_dve, _chain_count(4 * qe + 3))
                    for pr in range(4):
                        for i, e in enumerate(ACT_BYTES):
                            scalar.activation(
                                out=tmpa[pr][i][:], in_=tabio_t[:],
                                func=mybir.ActivationFunctionType.Square,
                                bias=idxf[pr][:, PERM[e]:PERM[e] + 1],
                                scale=-1.0,
                            ).then_inc(s_acq, 1)
                            acq += 1
                    scalar.wait_ge(s_acq, acq)
                    for pr in range(4):
                        for i, e in enumerate(ACT_BYTES):
                            scalar.activation(
                                out=og[joq][:, 2048 * pr + 256 * e:
                                            2048 * pr + 256 * (e + 1)],
                                in_=tmpa[pr][i][:],
                                func=mybir.ActivationFunctionType.Relu,
                                bias=1.0, scale=-1.0,
                            ).then_inc(s_ac, 1)

        @block.vector
        def _(vector: bass.BassEngine):
            n = 0  # statically tracked s_dve count

            def chain_op(ins):
                nonlocal n
                ins.then_inc(s_dve, 1)
                n += 1

            PRS = (0, 1, 2, 3)
            for q in range(G):
                jo = q % OBUF
                if q == 0:
                    vector.wait_ge(s_tabio, 16)
                vector.wait_ge(s_T, 4 * (q + 1))
                if ACT_BYTES and q >= 1:
                    # idxf reuse: ScalarE read group q-1 (squares done)
                    vector.wait_ge(s_acq, 4 * len(ACT_BYTES) * q)
                if q >= OBUF:
                    vector.wait_ge(s_store[jo], 128 * (q // OBUF))
                # interleaved chains: each wait's producers are >=4 ops back
                for pr in PRS:
                    chain_op(vector.tensor_copy(
                        iv[pr][:], pt[q % 2][:, 4 * pr:4 * pr + 4]))
                vector.wait_ge(s_dve, n)
                for pr in PRS:
                    # v4 = [s_lo_raw(17b), s_hi_raw, x_lo, x_hi]
                    chain_op(vector.tensor_tensor(
                        out=v4[pr][:, 0:2], in0=iv[pr][:, 0:2],
                        in1=iv[pr][:, 2:4], op=mybir.AluOpType.add))
                    chain_op(vector.tensor_tensor(
                        out=v4[pr][:, 2:4], in0=iv[pr][:, 0:2],
                        in1=iv[pr][:, 2:4], op=mybir.AluOpType.bitwise_xor))
                vector.wait_ge(s_dve, n)
                for pr in PRS:
                    # fold the 2^16 carry into s_hi (s_lo_raw keeps bit 16;
                    # the &255 byte masks strip it later)
                    chain_op(vector.scalar_tensor_tensor(
                        out=v4[pr][:, 1:2], in0=v4[pr][:, 0:1], scalar=65536,
                        in1=v4[pr][:, 1:2],
                        op0=mybir.AluOpType.is_ge, op1=mybir.AluOpType.add))
                vector.wait_ge(s_dve, n)
                for pr in PRS:
                    # byte extract (fused shift+mask); idx8 holds the bytes
                    # in [s0 s2 x0 x2 | s1 s3 x1 x3] order
                    chain_op(vector.tensor_scalar(
                        out=idx8[pr][:, 0:4], in0=v4[pr][:], scalar1=255,
                        scalar2=None, op0=mybir.AluOpType.bitwise_and))
                    chain_op(vector.tensor_scalar(
                        out=idx8[pr][:, 4:8], in0=v4[pr][:], scalar1=8,
                        scalar2=255,
                        op0=mybir.AluOpType.logical_shift_right,
                        op1=mybir.AluOpType.bitwise_and))
                vector.wait_ge(s_dve, n)
                for pr in PRS:
                    chain_op(vector.tensor_copy(idxf[pr][:], idx8[pr][:]))
                vector.wait_ge(s_dve, n)
                # encode: single-src is_equal against the iota table, one op
                # per output byte, per-partition scalar = that byte's value
                for pr in PRS:
                    for i, e in enumerate(DVE_BYTES):
                        ins = vector.tensor_scalar(
                            out=og[jo][:, 2048 * pr + 256 * e:
                                       2048 * pr + 256 * (e + 1)],
                            in0=tabio_t[:],
                            scalar1=idxf[pr][:, PERM[e]:PERM[e] + 1],
                            scalar2=None,
                            op0=mybir.AluOpType.is_equal,
                        )
                        if i == len(DVE_BYTES) - 1:
                            ins.then_inc(s_comp, 1)

        @block.gpsimd
        def _(gp: bass.BassEngine):
            for t in range(N_TILES):
                q = t // 4
                k = t % 4
                jo = q % OBUF
                r0 = t * P
                gp.wait_ge(s_comp, t + 1)
                if ACT_BYTES:
                    gp.wait_ge(s_ac, len(ACT_BYTES) * (4 * q + k + 1))
                gp.dma_start(
                    out=out_d[0, r0:r0 + P, :],
                    in_=og[jo][:, 2048 * k:2048 * k + ROW],
                ).then_inc(s_store[jo], 16)
                gp.dma_start(
                    out=out_d[1, r0:r0 + P, :],
                    in_=og[jo][:, 2048 * k + ROW:2048 * k + 2 * ROW],
                ).then_inc(s_store[jo], 16)

    return nc


def _make_tables():
    pos = np.arange(P, dtype=np.float64)
    w = np.zeros((NCH, P, 4), np.float64)
    for s in range(8):
        col = s // 2 if s < 4 else 2 + (s - 4) // 2
        mul = 1.0 if (s % 2 == 0) else 256.0
        for h in range(2):
            c = 2 * s + h
            w[c, :, col] = (pos + 128.0 * h) * mul
    tabw = w.transpose(1, 0, 2).reshape(P, NCH * 4).astype(ml_dtypes.bfloat16)
    tabio = np.tile(np.arange(256).astype(ml_dtypes.bfloat16)[None, :],
                    (P, 1))
    tabid = np.eye(4, dtype=np.float32)
    return tabw, tabio, tabid


def _pack_core(abt, lo):
    """[NCH, P, B] fp8 slab-chunks -> core block [G, P, NCH*NG]."""
    blk = abt[:, :, lo:lo + B_LOC].reshape(NCH, P, G, NG)
    return np.ascontiguousarray(
        blk.transpose(2, 1, 0, 3).reshape(G, P, NCH * NG))


_NC_CACHE = {}


def _get_nc(variant: str = "main"):
    if variant not in _NC_CACHE:
        _NC_CACHE[variant] = _build_nc()
    return _NC_CACHE[variant]


def _run(a: np.ndarray, b: np.ndarray, **spmd_kwargs):
    assert a.shape == (B, 4, 256) and b.shape == (B, 4, 256)
    a_t = np.ascontiguousarray(
        np.asarray(a, np.float32).reshape(B, 4, 256).transpose(1, 2, 0)
    ).astype(ml_dtypes.float8_e4m3)
    b_t = np.ascontiguousarray(
        np.asarray(b, np.float32).reshape(B, 4, 256).transpose(1, 2, 0)
    ).astype(ml_dtypes.float8_e4m3)
    abt = np.concatenate([a_t.reshape(NCH // 2, P, B),
                          b_t.reshape(NCH // 2, P, B)], axis=0)
    tabw, tabio, tabid = _make_tables()
    in_maps = [
        {
            "abt": _pack_core(abt, i * B_LOC),
            "tabw": tabw,
            "tabio": tabio,
            "tabid": tabid,
        }
        for i in range(N_CORES)
    ]
    nc = _get_nc()
    kr = run_bass_kernel_spmd(nc, in_maps, list(range(N_CORES)), **spmd_kwargs)
    shards = [kr.results[i]["out"] for i in range(N_CORES)]
    out = np.concatenate(shards, axis=1).astype(np.float32)
    return out.reshape(2, B, 4, 256), kr


def kernel(a: np.ndarray, b: np.ndarray) -> np.ndarray:
    out, _ = _run(a, b)
    return out


def run_sim():
    """CoreSim one core vs numpy oracle (invoked by test.py --sim)."""
    from concourse.bass_interp import CoreSim

    rng = np.random.default_rng(1)
    Bl = B_LOC
    ai = rng.integers(0, 256, (Bl, 4))
    bi = rng.integers(0, 256, (Bl, 4))
    ai[0] = [255] * 4
    bi[0] = [255] * 4
    ai[1] = [255, 255, 255, 255]
    bi[1] = [1, 0, 0, 0]
    a = np.zeros((Bl, 4, 256), np.float32)
    b = np.zeros((Bl, 4, 256), np.float32)
    r = np.arange(Bl)[:, None]
    j = np.arange(4)[None, :]
    a[r, j, ai] = 1.0
    b[r, j, bi] = 1.0

    a_t = np.ascontiguousarray(a.transpose(1, 2, 0)).astype(
        ml_dtypes.float8_e4m3)
    b_t = np.ascontiguousarray(b.transpose(1, 2, 0)).astype(
        ml_dtypes.float8_e4m3)
    abt = np.concatenate([a_t.reshape(NCH // 2, P, Bl),
                          b_t.reshape(NCH // 2, P, Bl)], axis=0)
    tabw, tabio, tabid = _make_tables()

    nc = _get_nc()
    sim = CoreSim(nc)
    sim.tensor("abt")[:] = _pack_core(abt, 0)
    sim.tensor("tabw")[:] = tabw
    sim.tensor("tabio")[:] = tabio
    sim.tensor("tabid")[:] = tabid
    sim.simulate()
    out = np.array(sim.tensor("out")).astype(np.float32).reshape(2, Bl, 4, 256)

    # numpy oracle
    pw = (256 ** np.arange(4)).astype(np.int64)
    a32 = (ai * pw).sum(-1)
    b32 = (bi * pw).sum(-1)
    s32 = (a32 + b32) % (2 ** 32)
    x32 = a32 ^ b32
    sb_ = np.stack([(s32 >> (8 * i)) & 255 for i in range(4)], -1)
    xb_ = np.stack([(x32 >> (8 * i)) & 255 for i in range(4)], -1)
    exp = np.zeros((2, Bl, 4, 256), np.float32)
    exp[0, r, j, sb_] = 1.0
    exp[1, r, j, xb_] = 1.0
    err = np.abs(out - exp).max()
    print(f"SIM max abs err: {err}")
    assert err == 0.0, "sim mismatch"
    print("SIM PASS")


# revision 36
# speedup vs baseline: 2.3884x; 1.0316x over previous
"""MoE-ALU (add with carry + xor over one-hot byte encodings) on 8 NeuronCores.

Semantics (validated against the jax reference bit-exactly): inputs a, b are
exact one-hot byte encodings [B, 4, 256] (little-endian bytes of 32-bit ints);
with SCALE=100 every softmax in the reference collapses to an exact one-hot, so

    out[0] = one_hot bytes of (a_int + b_int) mod 2^32
    out[1] = one_hot bytes of (a_int ^ b_int)

Layout: the host stores the one-hot inputs group/partition-major as fp8
([group, partition, chunk*column]; 0.0/1.0 are exact in fp8e4) so every DMA
descriptor is one contiguous 4 KiB run per partition, and the outputs as bf16
one-hots (exact 0/1). The device moves 8 MiB in + 16 MiB out per core instead
of 32+32 for f32 batch-major. All compute happens on device; the host only
reorders/recodes losslessly.

Device pipeline per 512-row batch group (8 groups per core):
  decode  TensorE: 16 accumulating matmuls (K=128 chunk each) of the fp8
          one-hot slabs against bf16 iota/256*iota weight columns produce
          PSUM [6, 512] = (a_lo16, a_hi16, b_lo16, b_hi16, s_lo_raw,
          s_hi_raw) -- the raw half sums come free from the PE (cost is
          N-only), exact in f32.
  stage   ScalarE copies PSUM -> SBUF f32 (frees the bank for group g+2).
  flip    TensorE transposes [6, 128] -> PSUM [128, 6] per 128-row tile.
  alu     VectorE per tile (6 ops): int32 cast, halves xor, carry fold,
          fused shift+mask byte extract, f32 cast; all 4 tiles of a group
          run phase-interleaved so every RAW wait's producer is >=4 ops
          back (the DVE pipe does not self-interlock; adjacent RAW stalls
          ~230ns).
  encode  per output byte, one-hot = is_equal against an iota table with a
          per-partition scalar: 6 bytes as DVE tensor_scalar (bf16 4x perf
          mode), 2 bytes as ScalarE Square/Relu pairs --
          relu(1-(idx-iota)^2), interleaved with their own sems.
  store   GPSIMD issues two 256 KiB output DMAs per tile as soon as that
          tile's bytes are encoded.

Raw Bass (one sync wait per instruction); rotating per-slot semaphores gate
buffer reuse; DVE same-engine RAW steps wait on a monotonically counted
semaphore.
"""
from contextlib import ExitStack

import numpy as np
import ml_dtypes

import concourse.bass as bass
from concourse import mybir
from concourse.bass_utils import run_bass_kernel_spmd

F32 = mybir.dt.float32
I32 = mybir.dt.int32
BF16 = mybir.dt.bfloat16
FP8 = mybir.dt.float8e4

P = 128
N_CORES = 8
B = 32768
B_LOC = B // N_CORES          # 4096 rows per core
ROW = 4 * 256                 # 1024 per row per tensor
NG = 512                      # batch rows per matmul group (one PSUM bank)
G = B_LOC // NG               # 8 groups
N_TILES = B_LOC // P          # 32 tiles of 128 rows
NCH = 16                      # K-chunks: 8 slabs (a0..a3,b0..b3) x 2 halves

NBUF = 4                      # input group-buffer slots
OBUF = 4                      # output group-buffer slots
NSUB = 4                      # input sub-DMAs per group
INFLIGHT = 2                  # concurrent group loads
WARMUP_MM = 0                 # dummy matmuls to ramp the PE clock

DVE_OPS = 6                   # s_dve increments per tile (chain ops)
GP_BYTES = ()                 # (Pool has no compare ops; GPSIMD issues stores)
ACT_BYTES = (1, 5)            # encode bytes on ScalarE (square+relu)
PERM = [0, 4, 1, 5, 2, 6, 3, 7]  # output byte e -> idxf column


def _act_bytes(q):
    """ScalarE-encoded bytes for group q."""
    return ACT_BYTES


def _dve_bytes(q):
    return tuple(e for e in range(8)
                 if e not in GP_BYTES and e not in _act_bytes(q))


# cumulative ScalarE relu count through tile t (2 per ACT-encoded tile)
_CUM_AC = []
_c = 0
for _t in range(B_LOC // P):
    _c += len(_act_bytes(_t // 4))
    _CUM_AC.append(_c)


def _op1_count(t):
    """s_dve value once tile t's pt->iv copy has retired (quad interleave)."""
    return 4 * DVE_OPS * (t // 4) + 1 + (t % 4)


def _chain_count(t):
    """s_dve value once tile t's full chain (incl. idxf) has retired."""
    return 4 * DVE_OPS * (t // 4) + 4 * (DVE_OPS - 1) + 1 + (t % 4)


def _build_nc() -> bass.Bass:
    nc = bass.Bass(trn_type="TRN2")
    ab_d = nc.dram_tensor("abt", [G, P, NCH * NG], FP8, kind="ExternalInput")
    tabw_d = nc.dram_tensor("tabw", [P, NCH * 6], BF16, kind="ExternalInput")
    tabio_d = nc.dram_tensor("tabio", [P, 256], BF16, kind="ExternalInput")
    tabid_d = nc.dram_tensor("tabid", [6, 6], F32, kind="ExternalInput")
    out_d = nc.dram_tensor("out", [2, B_LOC, ROW], BF16, kind="ExternalOutput")

    with ExitStack() as ctx:
        sb = lambda name, shape, dt: ctx.enter_context(
            nc.sbuf_tensor(name, shape, dt))
        tabw_t = sb("tabw_t", [P, NCH * 6], BF16)
        tabio_t = sb("tabio_t", [P, 256], BF16)
        tabid_t = sb("tabid_t", [6, 6], F32)
        in_t = [sb(f"in_t{k}", [P, NCH * NG], FP8) for k in range(NBUF)]
        sval = [sb(f"sval{k}", [6, NG], F32) for k in range(2)]
        og = [sb(f"og{k}", [P, 4 * 2 * ROW], BF16) for k in range(OBUF)]
        actsc = sb("actsc", [P, 1], F32)
        # parity-double-buffered per-tile temporaries
        iv = [sb(f"iv_{p}", [P, 8], I32) for p in range(4)]
        idx8 = [sb(f"idx8_{p}", [P, 8], I32) for p in range(4)]
        idxf = [sb(f"idxf_{p}", [P, 8], F32) for p in range(4)]


        pv = [ctx.enter_context(nc.psum_tensor(f"pv{k}", [6, NG], F32))
              for k in range(2)]
        tmpa = [[ctx.enter_context(
            nc.psum_tensor(f"tmpa_{p}_{i}", [P, 256], F32)) if p < 0
            else sb(f"tmpa_{p}_{i}", [P, 256], F32)
            for i in range(max(1, len(ACT_BYTES)))] for p in range(4)]
        pt = [ctx.enter_context(nc.psum_tensor(f"pt{k}", [P, 24], F32))
              for k in range(2)]

        s_tabw = ctx.enter_context(nc.semaphore("s_tabw"))
        s_tabid = ctx.enter_context(nc.semaphore("s_tabid"))
        s_tabio = ctx.enter_context(nc.semaphore("s_tabio"))
        s_sub = [[ctx.enter_context(nc.semaphore(f"s_sub{j}_{u}"))
                  for u in range(NSUB)] for j in range(NBUF)]
        s_store = [ctx.enter_context(nc.semaphore(f"s_store{j}"))
                   for j in range(OBUF)]
        s_mm = ctx.enter_context(nc.semaphore("s_mm"))      # matmul groups
        s_sv = ctx.enter_context(nc.semaphore("s_sv"))      # psum->sbuf copies
        s_T = ctx.enter_context(nc.semaphore("s_T"))        # transposes done
        s_comp = ctx.enter_context(nc.semaphore("s_comp"))  # DVE-encoded tiles
        s_dve = ctx.enter_context(nc.semaphore("s_dve"))    # chain ops done
        s_ac = ctx.enter_context(nc.semaphore("s_ac"))      # ACT-encoded tiles
        s_acq = ctx.enter_context(nc.semaphore("s_acq"))    # ACT square ops

        block = ctx.enter_context(nc.Block())

        @block.sync
        def _(sync: bass.BassEngine):
            CW = NCH * NG // NSUB   # columns per sub-DMA

            def load_group(g):
                j = g % NBUF
                for u in range(NSUB):
                    sync.dma_start(
                        out=in_t[j][:, CW * u:CW * (u + 1)],
                        in_=ab_d[g, :, CW * u:CW * (u + 1)],
                    ).then_inc(s_sub[j][u], 16)

            sync.dma_start(out=tabw_t[:], in_=tabw_d[:]).then_inc(s_tabw, 16)
            load_group(0)
            sync.dma_start(out=tabid_t[:], in_=tabid_d[:]).then_inc(
                s_tabid, 16)
            sync.dma_start(out=tabio_t[:], in_=tabio_d[:]).then_inc(
                s_tabio, 16)
            for g in range(1, G):
                if g == 1:
                    # give group 0 the full bandwidth (critical path)
                    for u in range(NSUB):
                        sync.wait_ge(s_sub[0][u], 16)
                if g >= INFLIGHT:
                    # bounded prefetch: group g-INFLIGHT fully landed first
                    gp_ = g - INFLIGHT
                    for u in range(NSUB):
                        sync.wait_ge(s_sub[gp_ % NBUF][u],
                                     16 * (gp_ // NBUF + 1))
                if g >= NBUF:
                    # slot reuse: matmuls of group g-NBUF consumed it
                    sync.wait_ge(s_mm, g - NBUF + 1)
                load_group(g)

        @block.tensor
        def _(tensor: bass.BassEngine):
            CS = NCH // NSUB
            tensor.wait_ge(s_tabw, 16)
            # clock-ramp warmup while the first input group is in flight
            warm_rhs = tabw_t[:, None, :].to_broadcast((P, 5, NCH * 6))
            for _w in range(WARMUP_MM):
                tensor.matmul(out=pv[1][:, 0:5 * NCH * 6], lhsT=tabw_t[:, 0:6],
                              rhs=warm_rhs, start=True, stop=True)
            for g in range(G + 1):
                q = g - 1
                if q >= 0:
                    if q == 0:
                        tensor.wait_ge(s_tabid, 16)
                    tensor.wait_ge(s_sv, q + 1)
                    if q >= 2:
                        # pt[q%2] freed once the pt->iv copy of the last
                        # tile of group q-2 retired
                        tensor.wait_ge(s_dve, _op1_count(4 * (q - 2) + 3))
                    for k in range(4):
                        tensor.transpose(
                            out=pt[q % 2][:, 6 * k:6 * (k + 1)],
                            in_=sval[q % 2][:, P * k:P * (k + 1)],
                            identity=tabid_t[:],
                        ).then_inc(s_T, 1)
                if g < G:
                    j = g % NBUF
                    if g >= 2:
                        # pv[g%2] freed once ScalarE copied group g-2
                        tensor.wait_ge(s_sv, g - 1)
                    for c in range(NCH):
                        if c % CS == 0:
                            tensor.wait_ge(s_sub[j][c // CS],
                                           16 * (g // NBUF + 1))
                        ins = tensor.matmul(
                            out=pv[g % 2][:, :],
                            lhsT=tabw_t[:, 6 * c:6 * (c + 1)],
                            rhs=in_t[j][:, NG * c:NG * (c + 1)],
                            start=(c == 0),
                            stop=(c == NCH - 1),
                        )
                        if c == NCH - 1:
                            ins.then_inc(s_mm, 1)

        @block.scalar
        def _(scalar: bass.BassEngine):
            acq = 0
            # hoist the implicit ACT_TABLE_LOAD off the critical path: the
            # first LUT activation triggers it, so run a dummy early
            scalar.wait_ge(s_tabio, 16)
            scalar.activation(
                out=actsc[:], in_=tabio_t[:, 0:1],
                func=mybir.ActivationFunctionType.Square)
            for g in range(G + 1):
                if g < G:
                    scalar.wait_ge(s_mm, g + 1)
                    if g >= 2:
                        # sval[g%2] freed once transposes of group g-2 done
                        scalar.wait_ge(s_T, 4 * (g - 1))
                    scalar.activation(
                        out=sval[g % 2][:, :], in_=pv[g % 2][:, :],
                        func=mybir.ActivationFunctionType.Copy,
                    ).then_inc(s_sv, 1)
                # ScalarE-encoded bytes for the tiles of group g-1, two
                # tiles interleaved (ACT ops need sems for same-engine RAW)
                qe = g - 1
                if 0 <= qe < G and _act_bytes(qe):
                    joq = qe % OBUF
                    if qe >= OBUF:
                        scalar.wait_ge(s_store[joq], 128 * (qe // OBUF))
                    if qe >= 1:
                        # tmpa WAR: previous group's Relus retired
                        scalar.wait_ge(s_ac, _CUM_AC[4 * (qe - 1) + 3])
                    scalar.wait_ge(s_dve, _chain_count(4 * qe + 3))
                    for pr in range(4):
                        for i, e in enumerate(_act_bytes(qe)):
                            scalar.activation(
                                out=tmpa[pr][i][:], in_=tabio_t[:],
                                func=mybir.ActivationFunctionType.Square,
                                bias=idxf[pr][:, PERM[e]:PERM[e] + 1],
                                scale=-1.0,
                            ).then_inc(s_acq, 1)
                            acq += 1
                    scalar.wait_ge(s_acq, acq)
                    for pr in range(4):
                        for i, e in enumerate(_act_bytes(qe)):
                            scalar.activation(
                                out=og[joq][:, 2048 * pr + 256 * e:
                                            2048 * pr + 256 * (e + 1)],
                                in_=tmpa[pr][i][:],
                                func=mybir.ActivationFunctionType.Relu,
                                bias=1.0, scale=-1.0,
                            ).then_inc(s_ac, 1)

        @block.vector
        def _(vector: bass.BassEngine):
            n = 0  # statically tracked s_dve count

            def chain_op(ins):
                nonlocal n
                ins.then_inc(s_dve, 1)
                n += 1

            PRS = (0, 1, 2, 3)
            for q in range(G):
                jo = q % OBUF
                if q == 0:
                    vector.wait_ge(s_tabio, 16)
                vector.wait_ge(s_T, 4 * (q + 1))
                if q >= 1 and _act_bytes(q - 1):
                    # idxf reuse: ScalarE read group q-1 (squares done)
                    vector.wait_ge(s_acq, _CUM_AC[4 * (q - 1) + 3])
                if q >= OBUF:
                    vector.wait_ge(s_store[jo], 128 * (q // OBUF))
                # interleaved chains: each wait's producers are >=4 ops back
                # iv = [a_lo a_hi b_lo b_hi s_lo_raw s_hi | x_lo x_hi]
                for pr in PRS:
                    chain_op(vector.tensor_copy(
                        iv[pr][:, 0:6], pt[q % 2][:, 6 * pr:6 * pr + 6]))
                vector.wait_ge(s_dve, n)
                for pr in PRS:
                    chain_op(vector.tensor_tensor(
                        out=iv[pr][:, 6:8], in0=iv[pr][:, 0:2],
                        in1=iv[pr][:, 2:4], op=mybir.AluOpType.bitwise_xor))
                    # fold the 2^16 carry into s_hi (s_lo_raw keeps bit 16;
                    # the &255 byte masks strip it later)
                    chain_op(vector.scalar_tensor_tensor(
                        out=iv[pr][:, 5:6], in0=iv[pr][:, 4:5], scalar=65536,
                        in1=iv[pr][:, 5:6],
                        op0=mybir.AluOpType.is_ge, op1=mybir.AluOpType.add))
                vector.wait_ge(s_dve, n)
                for pr in PRS:
                    # byte extract (fused shift+mask); idx8 holds the bytes
                    # in [s0 s2 x0 x2 | s1 s3 x1 x3] order
                    chain_op(vector.tensor_scalar(
                        out=idx8[pr][:, 0:4], in0=iv[pr][:, 4:8], scalar1=255,
                        scalar2=None, op0=mybir.AluOpType.bitwise_and))
                    chain_op(vector.tensor_scalar(
                        out=idx8[pr][:, 4:8], in0=iv[pr][:, 4:8], scalar1=8,
                        scalar2=255,
                        op0=mybir.AluOpType.logical_shift_right,
                        op1=mybir.AluOpType.bitwise_and))
                vector.wait_ge(s_dve, n)
                for pr in PRS:
                    chain_op(vector.tensor_copy(idxf[pr][:], idx8[pr][:]))
                vector.wait_ge(s_dve, n)
                # encode: single-src is_equal against the iota table, one op
                # per output byte, per-partition scalar = that byte's value
                dbytes = _dve_bytes(q)
                for pr in PRS:
                    for i, e in enumerate(dbytes):
                        ins = vector.tensor_scalar(
                            out=og[jo][:, 2048 * pr + 256 * e:
                                       2048 * pr + 256 * (e + 1)],
                            in0=tabio_t[:],
                            scalar1=idxf[pr][:, PERM[e]:PERM[e] + 1],
                            scalar2=None,
                            op0=mybir.AluOpType.is_equal,
                        )
                        if i == len(dbytes) - 1:
                            ins.then_inc(s_comp, 1)

        @block.gpsimd
        def _(gp: bass.BassEngine):
            for t in range(N_TILES):
                q = t // 4
                k = t % 4
                jo = q % OBUF
                r0 = t * P
                gp.wait_ge(s_comp, t + 1)
                if _act_bytes(q):
                    gp.wait_ge(s_ac, _CUM_AC[t])
                gp.dma_start(
                    out=out_d[0, r0:r0 + P, :],
                    in_=og[jo][:, 2048 * k:2048 * k + ROW],
                ).then_inc(s_store[jo], 16)
                gp.dma_start(
                    out=out_d[1, r0:r0 + P, :],
                    in_=og[jo][:, 2048 * k + ROW:2048 * k + 2 * ROW],
                ).then_inc(s_store[jo], 16)

    return nc


def _make_tables():
    pos = np.arange(P, dtype=np.float64)
    w = np.zeros((NCH, P, 6), np.float64)
    for s in range(8):
        col = s // 2 if s < 4 else 2 + (s - 4) // 2
        scol = 4 + (s // 2) % 2
        mul = 1.0 if (s % 2 == 0) else 256.0
        for h in range(2):
            c = 2 * s + h
            v = (pos + 128.0 * h) * mul
            w[c, :, col] = v
            w[c, :, scol] = v
    tabw = w.transpose(1, 0, 2).reshape(P, NCH * 6).astype(ml_dtypes.bfloat16)
    tabio = np.tile(np.arange(256).astype(ml_dtypes.bfloat16)[None, :],
                    (P, 1))
    tabid = np.eye(6, dtype=np.float32)
    return tabw, tabio, tabid


def _pack_core(abt, lo):
    """[NCH, P, B] fp8 slab-chunks -> core block [G, P, NCH*NG]."""
    blk = abt[:, :, lo:lo + B_LOC].reshape(NCH, P, G, NG)
    return np.ascontiguousarray(
        blk.transpose(2, 1, 0, 3).reshape(G, P, NCH * NG))


_NC_CACHE = {}


def _get_nc(variant: str = "main"):
    if variant not in _NC_CACHE:
        _NC_CACHE[variant] = _build_nc()
    return _NC_CACHE[variant]


def _run(a: np.ndarray, b: np.ndarray, **spmd_kwargs):
    assert a.shape == (B, 4, 256) and b.shape == (B, 4, 256)
    a_t = np.ascontiguousarray(
        np.asarray(a, np.float32).reshape(B, 4, 256).transpose(1, 2, 0)
    ).astype(ml_dtypes.float8_e4m3)
    b_t = np.ascontiguousarray(
        np.asarray(b, np.float32).reshape(B, 4, 256).transpose(1, 2, 0)
    ).astype(ml_dtypes.float8_e4m3)
    abt = np.concatenate([a_t.reshape(NCH // 2, P, B),
                          b_t.reshape(NCH // 2, P, B)], axis=0)
    tabw, tabio, tabid = _make_tables()
    in_maps = [
        {
            "abt": _pack_core(abt, i * B_LOC),
            "tabw": tabw,
            "tabio": tabio,
            "tabid": tabid,
        }
        for i in range(N_CORES)
    ]
    nc = _get_nc()
    kr = run_bass_kernel_spmd(nc, in_maps, list(range(N_CORES)), **spmd_kwargs)
    shards = [kr.results[i]["out"] for i in range(N_CORES)]
    out = np.concatenate(shards, axis=1).astype(np.float32)
    return out.reshape(2, B, 4, 256), kr


def kernel(a: np.ndarray, b: np.ndarray) -> np.ndarray:
    out, _ = _run(a, b)
    return out


def run_sim():
    """CoreSim one core vs numpy oracle (invoked by test.py --sim)."""
    from concourse.bass_interp import CoreSim

    rng = np.random.default_rng(1)
    Bl = B_LOC
    ai = rng.integers(0, 256, (Bl, 4))
    bi = rng.integers(0, 256, (Bl, 4))
    ai[0] = [255] * 4
    bi[0] = [255] * 4
    ai[1] = [255, 255, 255, 255]
    bi[1] = [1, 0, 0, 0]
    a = np.zeros((Bl, 4, 256), np.float32)
    b = np.zeros((Bl, 4, 256), np.float32)
    r = np.arange(Bl)[:, None]
    j = np.arange(4)[None, :]
    a[r, j, ai] = 1.0
    b[r, j, bi] = 1.0

    a_t = np.ascontiguousarray(a.transpose(1, 2, 0)).astype(
        ml_dtypes.float8_e4m3)
    b_t = np.ascontiguousarray(b.transpose(1, 2, 0)).astype(
        ml_dtypes.float8_e4m3)
    abt = np.concatenate([a_t.reshape(NCH // 2, P, Bl),
                          b_t.reshape(NCH // 2, P, Bl)], axis=0)
    tabw, tabio, tabid = _make_tables()

    nc = _get_nc()
    sim = CoreSim(nc)
    sim.tensor("abt")[:] = _pack_core(abt, 0)
    sim.tensor("tabw")[:] = tabw
    sim.tensor("tabio")[:] = tabio
    sim.tensor("tabid")[:] = tabid
    sim.simulate()
    out = np.array(sim.tensor("out")).astype(np.float32).reshape(2, Bl, 4, 256)

    # numpy oracle
    pw = (256 ** np.arange(4)).astype(np.int64)
    a32 = (ai * pw).sum(-1)
    b32 = (bi * pw).sum(-1)
    s32 = (a32 + b32) % (2 ** 32)
    x32 = a32 ^ b32
    sb_ = np.stack([(s32 >> (8 * i)) & 255 for i in range(4)], -1)
    xb_ = np.stack([(x32 >> (8 * i)) & 255 for i in range(4)], -1)
    exp = np.zeros((2, Bl, 4, 256), np.float32)
    exp[0, r, j, sb_] = 1.0
    exp[1, r, j, xb_] = 1.0
    err = np.abs(out - exp).max()
    print(f"SIM max abs err: {err}")
    assert err == 0.0, "sim mismatch"
    print("SIM PASS")
